# revision 1
# baseline (speedup 1.0000x reference)
"""Bass/Trainium2 SPMD kernel for nn_Block3D (8 NeuronCores).

Sharding: spatial z-shard (24 planes -> 3 per core, host-prepped halo slabs),
channels on partitions. kernel_net GEMV1 row-sharded / GEMV2 K-sharded with a
single AllReduce; halo exchange of the LN2 output via paired AllGathers made
branch-free with per-core mask inputs. Depthwise 3x3x3 convs run as shifted-AP
scalar_tensor_tensor chains on DVE, with a tunable subset of MLP channel tiles
offloaded to the PE via host-precomputed diagonal weight matrices.
"""

import os
from contextlib import ExitStack

import numpy as np
import ml_dtypes

import concourse.bass as bass
import concourse.bacc as bacc
import concourse.tile as tile
from concourse import mybir
from concourse.bass_utils import run_bass_kernel_spmd

BF = ml_dtypes.bfloat16
F32 = mybir.dt.float32
BF16 = mybir.dt.bfloat16

C = 768
G = 12
GD = 64
S = 24
HID = 4 * C
KK = 27
V = S * S * S
EPS = 1e-5
NCORES = 8
ZP = S // NCORES
PL = S * S
VC = ZP * PL
Z5 = ZP + 2
PPL = 26 * 26
PADN = Z5 * PPL
CT = C // 128
HT = HID // 128
W1R = HID // NCORES
W2K = W1R
NB = 288
KFLAT = C * KK

PE_TILES = frozenset(int(x) for x in os.environ.get(
    "BLK3D_PE_TILES", ",".join(str(i) for i in range(24))).split(",") if x != "")

TAPS = [(dz, dy, dx) for dz in (-1, 0, 1) for dy in (-1, 0, 1) for dx in (-1, 0, 1)]

_CACHE = {}

Copy = mybir.ActivationFunctionType.Copy
Iden = mybir.ActivationFunctionType.Identity
Gelu = mybir.ActivationFunctionType.Gelu
Sigmoid = mybir.ActivationFunctionType.Sigmoid
Square = mybir.ActivationFunctionType.Square
Sqrt = mybir.ActivationFunctionType.Sqrt
Relu = mybir.ActivationFunctionType.Relu
ADD = mybir.AluOpType.add
SUB = mybir.AluOpType.subtract
MULT = mybir.AluOpType.mult

(S_Y, S_LB1, S_LB2, S_TPB, S_ABV, S_ABO, S_OPB, S_N2W, S_N2B, S_N3W, S_N3B,
 S_GNG, S_GNB, S_MODB) = range(14)


def _pad_off(dz, dy, dx):
    return (1 + dz) * PPL + (1 + dy) * 26 + (1 + dx)


def build_program():
    nc = bacc.Bacc("TRN2", target_bir_lowering=False)

    def dram_in(name, shape, dtype=F32):
        return nc.declare_dram_parameter(name, list(shape), dtype, isOutput=False)

    x_halo = dram_in("x_halo", [C, Z5 * PL], BF16)
    smalls = dram_in("smalls", [C, 16])
    knb1 = dram_in("knb1", [W1R])
    knb2r = dram_in("knb2r", [KFLAT])
    halo_masks = dram_in("halo_masks", [128, 18])
    gind6 = dram_in("gind6", [CT, 128, G])
    gexpT = dram_in("gexpT", [G, C])
    ident = dram_in("ident", [128, 128], BF16)
    onesc = dram_in("onesc", [128, 1], BF16)
    loraW1T = dram_in("loraW1T", [C, C], BF16)
    loraW2T = dram_in("loraW2T", [C, C], BF16)
    tpWT = dram_in("tpWT", [C, C], BF16)
    avWT = dram_in("avWT", [C, C], BF16)
    aoWT = dram_in("aoWT", [C, C], BF16)
    modWT = dram_in("modWT", [2 * C, C], BF16)
    w1sT = dram_in("w1sT", [2 * C, W1R], BF16)
    w2sT = dram_in("w2sT", [W2K, KFLAT], BF16)
    opT = dram_in("opT", [C, C], BF16)
    wiT = dram_in("wiT", [HT, 128, CT * 128], BF16)
    woT = dram_in("woT", [CT, 128, (HID // 2 // 128) * 128], BF16)
    dmlp = dram_in("dmlp", [HT, 128, KK * 128], BF16)
    dwk_in = dram_in("dwk", [HID, KK])
    out = nc.declare_dram_parameter("out", [C, VC], F32, isOutput=True)

    with tile.TileContext(nc) as tc, ExitStack() as ctx:
        dram = ctx.enter_context(tc.tile_pool(name="dram", bufs=1, space="DRAM"))
        persist = ctx.enter_context(tc.tile_pool(name="persist", bufs=1))
        gpool = ctx.enter_context(tc.tile_pool(name="gemv", bufs=2))
        wpool = ctx.enter_context(tc.tile_pool(name="wstream", bufs=4))

        # ------------- persistent small tiles -------------
        sm = [persist.tile([128, 16], F32, name=f"sm{i}", tag=f"sm{i}")
              for i in range(CT)]
        for i in range(CT):
            nc.gpsimd.dma_start(sm[i][:], smalls[128 * i:128 * (i + 1), :])
        hm = persist.tile([128, 18], F32, name="hm", tag="hm")
        nc.gpsimd.dma_start(hm[:], halo_masks[:, :])
        id_t = persist.tile([128, 128], BF16, name="identt", tag="identt")
        nc.gpsimd.dma_start(id_t[:], ident[:, :])
        ones_t = persist.tile([128, 1], BF16, name="onest", tag="onest")
        nc.gpsimd.dma_start(ones_t[:], onesc[:, :])
        eps_t = persist.tile([128, 1], F32, name="epst", tag="epst")
        nc.vector.memset(eps_t[:], EPS)
        mid = {}
        for k in range(16):
            t = persist.tile([128, 128], BF16, name=f"mid{k}", tag=f"mid{k}")
            nc.vector.tensor_scalar_mul(t[:], id_t[:], hm[:, k:k + 1])
            mid[k] = t
        junk = persist.tile([128, VC], BF16, name="junk", tag="junk")
        _dwk = []
        for tt in range(HT):
            t = persist.tile([128, KK], F32, name=f"dwk{tt}", tag=f"dwk{tt}")
            nc.gpsimd.dma_start(t[:], dwk_in[128 * tt:128 * (tt + 1), :])
            _dwk.append(t)

        def conv_dve(dst, src_pad, src_pad1, ktile):
            p4 = src_pad.rearrange("p (z y x) -> p z y x", z=Z5, y=26, x=26)
            p4s = src_pad1.rearrange("p (z y x) -> p z y x", z=Z5, y=26, x=26)
            d4 = dst.rearrange("p (z y x) -> p z y x", z=ZP, y=S, x=S)
            for ti, (dz, dy, dx) in enumerate(TAPS):
                sc = ktile[:, ti:ti + 1]
                for z in range(ZP):
                    if _pad_off(dz, dy, dx) % 2 == 0:
                        sv = p4[:, 1 + dz + z, 1 + dy:1 + dy + S,
                                1 + dx:1 + dx + S]
                    else:
                        sv = p4s[:, 1 + dz + z, 1 + dy:1 + dy + S, dx:dx + S]
                    dv = d4[:, z]
                    if ti == 0:
                        nc.vector.tensor_scalar_mul(dv, sv, sc)
                    else:
                        nc.vector.scalar_tensor_tensor(dv, sv, sc, dv,
                                                       op0=MULT, op1=ADD)

        def ln_stats(tiles, tag):
            """Per-voxel mean/rstd over channels -> [128, VC] bf16 bcast tiles."""
            chunks = [(0, 512), (512, 512), (1024, 512), (1536, 192)]
            with (tc.tile_pool(name=f"{tag}ps", bufs=1, space="PSUM") as lps,
                  tc.tile_pool(name=f"{tag}sq", bufs=2) as sqp,
                  tc.tile_pool(name=f"{tag}rw", bufs=1) as rwp):
                ps1 = lps.tile([1, VC], F32, name="s1", tag="s1")
                ps2 = lps.tile([1, VC], F32, name="s2", tag="s2")
                for o, n in chunks:
                    sl = slice(o, o + n)
                    for k in range(CT):
                        nc.tensor.matmul(ps1[:, sl], ones_t[:], tiles[k][:, sl],
                                         start=(k == 0), stop=(k == CT - 1))
                    for k in range(CT):
                        sq = sqp.tile([128, 512], BF16, name="sq", tag="sq")
                        nc.scalar.activation(sq[:, 0:n], tiles[k][:, sl], Square)
                        nc.tensor.matmul(ps2[:, sl], ones_t[:], sq[:, 0:n],
                                         start=(k == 0), stop=(k == CT - 1))
                row = rwp.tile([1, 2 * VC], F32, name="row", tag="row")
                nc.scalar.activation(row[:, 0:VC], ps1[:], Copy, scale=1.0 / C)
                nc.scalar.activation(row[:, VC:2 * VC], ps2[:], Copy,
                                     scale=1.0 / C)
                w = VC // 96  # 18
                rs = rwp.tile([96, 2 * w], F32, name="rs", tag="rs")
                nc.gpsimd.dma_start(rs[:, 0:w], row[:, 0:VC])
                nc.gpsimd.dma_start(rs[:, w:2 * w], row[:, VC:2 * VC])
                m2 = rwp.tile([96, w], F32, name="m2", tag="m2")
                nc.scalar.square(m2[:], rs[:, 0:w])
                vr = rwp.tile([96, w], F32, name="vr", tag="vr")
                nc.vector.tensor_sub(vr[:], rs[:, w:2 * w], m2[:])
                nc.scalar.activation(vr[:], vr[:], Sqrt, bias=eps_t[0:96, 0:1])
                nc.vector.reciprocal(vr[:], vr[:])
                mrow = rwp.tile([1, 2 * VC], BF16, name="mrow", tag="mrow")
                nc.gpsimd.dma_start(mrow[:, 0:VC], rs[:, 0:w])
                nc.gpsimd.dma_start(mrow[:, VC:2 * VC], vr[:])
                drow = dram.tile([2 * VC], BF16, name=f"{tag}drow",
                                 tag=f"{tag}drow")
                nc.gpsimd.dma_start(drow[:], mrow[:])
            muB = persist.tile([128, VC], BF16, name=f"{tag}muB", tag=f"{tag}muB")
            rsB = persist.tile([128, VC], BF16, name=f"{tag}rsB", tag=f"{tag}rsB")
            nc.gpsimd.dma_start(
                muB[:], bass.AP(tensor=drow[:].tensor, offset=drow[:].offset,
                                ap=[[0, 128], [1, VC]]))
            nc.gpsimd.dma_start(
                rsB[:], bass.AP(tensor=drow[:].tensor,
                                offset=drow[:].offset + VC,
                                ap=[[0, 128], [1, VC]]))
            return muB, rsB

        xbp = ctx.enter_context(tc.tile_pool(name="xbp", bufs=1))
        gatep = ctx.enter_context(tc.tile_pool(name="gatep", bufs=1))
        xb = [xbp.tile([128, VC], BF16, name=f"xb{i}", tag=f"xb{i}")
              for i in range(CT)]
        gate = [gatep.tile([128, VC], BF16, name=f"gate{j}", tag=f"gate{j}")
                for j in range(HT // 2)]

        with tc.tile_pool(name="actp", bufs=1) as actp:
            xs = [actp.tile([128, Z5 * PL], BF16, name=f"xs{i}", tag=f"xs{i}")
                  for i in range(CT)]
            for i in range(CT):
                nc.gpsimd.dma_start(xs[i][:], x_halo[128 * i:128 * (i + 1), :])
            dyn = [actp.tile([128, VC], BF16, name=f"dyn{i}", tag=f"dyn{i}")
                   for i in range(CT)]

            # ---- phase A: vc partial sums + AR1 ----
            vcs = persist.tile([128, CT], F32, name="vcs", tag="vcs")
            for i in range(CT):
                nc.scalar.activation(junk[:], xs[i][:, PL:PL + VC], Copy,
                                     accum_out=vcs[:, i:i + 1])
            ar1_in = dram.tile([C], F32, name="ar1i", tag="ar1i")
            ar1_out = dram.tile([C], F32, name="ar1o", tag="ar1o", addr_space="Shared")
            nc.gpsimd.dma_start(
                bass.AP(tensor=ar1_in[:].tensor, offset=ar1_in[:].offset,
                        ap=[[1, 128], [128, CT]]), vcs[:])
            nc.gpsimd.collective_compute(
                "AllReduce", ADD, replica_groups=[list(range(NCORES))],
                ins=[ar1_in[:]], outs=[ar1_out[:]])

            # ---- phases B-E ----
            with tc.tile_pool(name="psA", bufs=2, space="PSUM") as psA:

                def load_w(wdram, k0, m0, tag):
                    t = wpool.tile([128, 128], BF16, name=tag, tag=tag)
                    nc.gpsimd.dma_start(t[:], wdram[k0:k0 + 128, m0:m0 + 128])
                    return t

                def gemv(wdram, in_cols, nk, nm, act, bias_cols, tag, scale=1.0,
                         odt=BF16):
                    outs = []
                    for m in range(nm):
                        ps = psA.tile([128, 1], F32, name="ps_small",
                                      tag="ps_small")
                        for k in range(nk):
                            w = load_w(wdram, 128 * k, 128 * m, tag)
                            nc.tensor.matmul(ps[:], w[:], in_cols[k][:],
                                             start=(k == 0), stop=(k == nk - 1))
                        o = gpool.tile([128, 1], odt, name=f"{tag}o{m}",
                                       tag=f"{tag}o{m}")
                        bias = bias_cols[m] if bias_cols is not None else 0.0
                        nc.scalar.activation(o[:], ps[:], act, bias=bias,
                                             scale=scale)
                        outs.append(o)
                    return outs

                t_cols = []
                for i in range(CT):
                    t = gpool.tile([128, 1], BF16, name=f"tc{i}", tag=f"tc{i}")
                    nc.scalar.activation(t[:], sm[i][:, S_Y:S_Y + 1], Copy)
                    t_cols.append(t)
                h1 = gemv(loraW1T, t_cols, CT, CT, Relu,
                          [sm[i][:, S_LB1:S_LB1 + 1] for i in range(CT)], "lw1")
                h2 = gemv(loraW2T, h1, CT, CT, Iden,
                          [sm[i][:, S_LB2:S_LB2 + 1] for i in range(CT)], "lw2")
                tp = gemv(tpWT, h2, CT, CT, Iden,
                          [sm[i][:, S_TPB:S_TPB + 1] for i in range(CT)], "tpw")
                av = gemv(avWT, tp, CT, CT, Iden,
                          [sm[i][:, S_ABV:S_ABV + 1] for i in range(CT)], "avw")
                attn = gemv(aoWT, av, CT, CT, Iden,
                            [sm[i][:, S_ABO:S_ABO + 1] for i in range(CT)],
                            "aow")

                comb = []
                for i in range(CT):
                    cb = gpool.tile([128, 1], BF16, name=f"cmb{i}",
                                    tag=f"cmb{i}")
                    col = persist.tile([128, 1], F32, name=f"vcc{i}",
                                       tag=f"vcc{i}")
                    nc.gpsimd.dma_start(
                        col[:], bass.AP(tensor=ar1_out[:].tensor,
                                        offset=ar1_out[:].offset + 128 * i,
                                        ap=[[1, 128], [128, 1]]))
                    nc.scalar.activation(cb[:], col[:], Copy, scale=1.0 / V)
                    comb.append(cb)
                comb += attn

                mod = gemv(modWT, comb, 2 * CT, CT, Sigmoid,
                           [sm[i][:, S_MODB:S_MODB + 1] for i in range(CT)],
                           "modw", odt=F32)

                knb1_t = persist.tile([128, 3], F32, name="knb1t", tag="knb1t")
                nc.gpsimd.dma_start(
                    knb1_t[:],
                    bass.AP(tensor=knb1, offset=0, ap=[[1, 128], [128, 3]]))
                kp1 = gemv(w1sT, comb, 2 * CT, 3, Relu,
                           [knb1_t[:, m:m + 1] for m in range(3)], "w1s")

                ar2_in = dram.tile([KFLAT], F32, name="ar2i", tag="ar2i")
                ar2_out = dram.tile([KFLAT], F32, name="ar2o", tag="ar2o", addr_space="Shared")
                with tc.tile_pool(name="kseq", bufs=2) as kseq:
                    for t in range(KK):
                        wts = []
                        for k in range(3):
                            wt = kseq.tile([128, C], BF16, name=f"w2s{k}",
                                           tag=f"w2s{k}")
                            nc.sync.dma_start(
                                wt[:], w2sT[128 * k:128 * (k + 1),
                                            C * t:C * (t + 1)])
                            wts.append(wt)
                        ps = psA.tile([128, CT], F32, name="g2ps", tag="g2ps")
                        for m in range(CT):
                            for k in range(3):
                                nc.tensor.matmul(
                                    ps[:, m:m + 1],
                                    wts[k][:, 128 * m:128 * (m + 1)],
                                    kp1[k][:], start=(m == 0 and k == 0),
                                    stop=(m == CT - 1 and k == 2))
                        ko = kseq.tile([128, CT], F32, name="g2o", tag="g2o")
                        nc.scalar.activation(ko[:], ps[:], Copy)
                        for m in range(CT):
                            nc.gpsimd.dma_start(
                                bass.AP(tensor=ar2_in[:].tensor,
                                        offset=(ar2_in[:].offset + C * t
                                                + 128 * m),
                                        ap=[[1, 128], [1, 1]]),
                                ko[:, m:m + 1])
                nc.gpsimd.collective_compute(
                    "AllReduce", ADD, replica_groups=[list(range(NCORES))],
                    ins=[ar2_in[:]], outs=[ar2_out[:]])

                kern = []
                for i in range(CT):
                    kt = persist.tile([128, KK], F32, name=f"kern{i}",
                                      tag=f"kern{i}")
                    nc.gpsimd.dma_start(
                        kt[:], bass.AP(tensor=ar2_out[:].tensor,
                                       offset=ar2_out[:].offset + 128 * i,
                                       ap=[[1, 128], [C, KK]]))
                    kb = persist.tile([128, KK], F32, name=f"kernb{i}",
                                      tag=f"kernb{i}")
                    nc.gpsimd.dma_start(
                        kb[:], bass.AP(tensor=knb2r, offset=128 * i,
                                       ap=[[1, 128], [C, KK]]))
                    nc.vector.tensor_add(kt[:], kt[:], kb[:])
                    kern.append(kt)

                # ---- phase E: mv + dynamic conv + GN stats ----
                ar3_in = dram.tile([G, 2], F32, name="ar3i", tag="ar3i")
                ar3_out = dram.tile([G, 2], F32, name="ar3o", tag="ar3o", addr_space="Shared")
                gsb = persist.tile([G, 2], F32, name="gsb", tag="gsb")
                with (tc.tile_pool(name="mvp", bufs=2) as mvp,
                      tc.tile_pool(name="gnps", bufs=1, space="PSUM") as gnps):
                    gps = gnps.tile([G, 2], F32, name="gps", tag="gps")
                    for i in range(CT):
                        mp = mvp.tile([128, PADN], BF16, name="mvpad",
                                      tag="mvpad")
                        mp1 = mvp.tile([128, PADN], BF16, name="mvpad1",
                                       tag="mvpad1")
                        nc.gpsimd.memset(mp[:], 0.0)
                        nc.gpsimd.memset(mp1[:], 0.0)
                        m4 = mp.rearrange("p (z y x) -> p z y x", z=Z5, y=26,
                                          x=26)
                        x4 = xs[i].rearrange("p (z y x) -> p z y x", z=Z5, y=S,
                                             x=S)
                        for z in range(Z5):
                            nc.scalar.activation(m4[:, z, 1:25, 1:25], x4[:, z],
                                                 Copy, scale=mod[i][:, 0:1])
                        nc.scalar.activation(mp1[:, 0:PADN - 1], mp[:, 1:PADN],
                                             Copy)
                        conv_dve(dyn[i][:], mp[:], mp1[:], kern[i])
                        st = mvp.tile([128, 2], F32, name="gnst", tag="gnst")
                        nc.scalar.activation(junk[:], dyn[i][:], Copy,
                                             accum_out=st[:, 0:1])
                        nc.scalar.activation(junk[:], dyn[i][:], Square,
                                             accum_out=st[:, 1:2])
                        gi = mvp.tile([128, G], F32, name="gind", tag="gind")
                        nc.gpsimd.dma_start(gi[:], gind6[i, :, :])
                        nc.tensor.matmul(gps[:], gi[:], st[:], start=(i == 0),
                                         stop=(i == CT - 1))
                    nc.scalar.activation(gsb[:], gps[:], Copy)
                nc.gpsimd.dma_start(ar3_in[:], gsb[:])
                nc.gpsimd.collective_compute(
                    "AllReduce", ADD, replica_groups=[list(range(NCORES))],
                    ins=[ar3_in[:]], outs=[ar3_out[:]])
                gstat = persist.tile([G, 2], F32, name="gstat", tag="gstat")
                nc.gpsimd.dma_start(gstat[:], ar3_out[:])
                NGRP = float(GD * V)
                gmr = persist.tile([G, 2], F32, name="gmr", tag="gmr")
                nc.scalar.activation(gmr[:, 0:1], gstat[:, 0:1], Copy,
                                     scale=1.0 / NGRP)
                musq = persist.tile([G, 1], F32, name="musq", tag="musq")
                nc.scalar.square(musq[:], gmr[:, 0:1])
                var = persist.tile([G, 1], F32, name="gvar", tag="gvar")
                nc.vector.tensor_scalar(var[:], gstat[:, 1:2], 1.0 / NGRP, None,
                                        op0=MULT)
                nc.vector.tensor_sub(var[:], var[:], musq[:])
                nc.scalar.activation(var[:], var[:], Sqrt, bias=eps_t[0:G, 0:1])
                nc.vector.reciprocal(gmr[:, 1:2], var[:])

                opT_t = [persist.tile([128, C], BF16, name=f"opT{i}",
                                      tag=f"opT{i}") for i in range(CT)]
                for i in range(CT):
                    nc.sync.dma_start(opT_t[i][:],
                                        opT[128 * i:128 * (i + 1), :])
                cafm_shift = []
                gsc = []
                for i in range(CT):
                    ge = gpool.tile([G, 128], F32, name=f"gexp{i}",
                                    tag=f"gexp{i}")
                    nc.gpsimd.dma_start(ge[:], gexpT[:, 128 * i:128 * (i + 1)])
                    ps = psA.tile([128, 2], F32, name="ps_sm2", tag="ps_sm2")
                    nc.tensor.matmul(ps[:], ge[:], gmr[:], start=True, stop=True)
                    mu_c = persist.tile([128, 2], F32, name=f"muc{i}",
                                        tag=f"muc{i}")
                    nc.scalar.activation(mu_c[:], ps[:], Copy)
                    a = persist.tile([128, 1], F32, name=f"gsc{i}",
                                     tag=f"gsc{i}")
                    nc.vector.tensor_mul(a[:], sm[i][:, S_GNG:S_GNG + 1],
                                         mu_c[:, 1:2])
                    b = persist.tile([128, 1], F32, name=f"gsh{i}",
                                     tag=f"gsh{i}")
                    nc.vector.tensor_mul(b[:], mu_c[:, 0:1], a[:])
                    nc.vector.tensor_sub(b[:], sm[i][:, S_GNB:S_GNB + 1], b[:])
                    gsc.append(a)
                    bb = gpool.tile([128, 1], BF16, name=f"gshb{i}",
                                    tag=f"gshb{i}")
                    nc.scalar.activation(bb[:], b[:], Copy)
                    cafm_shift.append(bb)
                cb_cols = []
                for m in range(CT):
                    ps = psA.tile([128, 1], F32, name="ps_small",
                                  tag="ps_small")
                    for k in range(CT):
                        nc.tensor.matmul(ps[:],
                                         opT_t[k][:, 128 * m:128 * (m + 1)],
                                         cafm_shift[k][:], start=(k == 0),
                                         stop=(k == CT - 1))
                    o = persist.tile([128, 1], F32, name=f"cbc{m}",
                                     tag=f"cbc{m}")
                    nc.scalar.activation(o[:], ps[:], Iden,
                                         bias=sm[m][:, S_OPB:S_OPB + 1])
                    cb_cols.append(o)
                for i in range(CT):
                    nc.vector.tensor_scalar_mul(opT_t[i][:], opT_t[i][:],
                                                gsc[i][:])

            # ---- phase F1: cafm matmul + xb ----
            CH4 = [(0, 512), (512, 512), (1024, 512), (1536, 192)]
            with tc.tile_pool(name="opwps", bufs=2, space="PSUM") as opwps:
                for m in range(CT):
                    for o, n in CH4:
                        sl = slice(o, o + n)
                        ps = opwps.tile([128, 512], F32, name="opw", tag="opw")
                        for k in range(CT):
                            nc.tensor.matmul(
                                ps[:, 0:n], opT_t[k][:, 128 * m:128 * (m + 1)],
                                dyn[k][:, sl], start=(k == 0),
                                stop=(k == CT - 1))
                        nc.vector.scalar_tensor_tensor(
                            xb[m][:, sl], ps[:, 0:n], cb_cols[m][:],
                            xs[m][:, PL + o:PL + o + n],
                            op0=ADD, op1=MULT)

        # ---- phase F2: LN2 + halo exchange ----
        muB, rsB = ln_stats(xb, "ln2")
        with tc.tile_pool(name="xlnp", bufs=1) as xlnp:
            xln = [xlnp.tile([128, Z5 * PL], BF16, name=f"xln{i}",
                             tag=f"xln{i}") for i in range(CT)]
            with tc.tile_pool(name="glueF", bufs=2) as glueF:
                for i in range(CT):
                    t1 = glueF.tile([128, VC], BF16, name="lnt1", tag="lnt1")
                    nc.vector.tensor_sub(t1[:], xb[i][:], muB[:])
                    nc.vector.tensor_mul(t1[:], t1[:], rsB[:])
                    nc.vector.tensor_scalar(xln[i][:, PL:PL + VC], t1[:],
                                            sm[i][:, S_N2W:S_N2W + 1],
                                            sm[i][:, S_N2B:S_N2B + 1],
                                            op0=MULT, op1=ADD)

            agi = dram.tile([2, C, PL], BF16, name="agi", tag="agi")
            ago = dram.tile([8, 2, C, PL], BF16, name="ago", tag="ago", addr_space="Shared")
            for i in range(CT):
                cs = slice(128 * i, 128 * (i + 1))
                nc.gpsimd.dma_start(agi[0, cs, :], xln[i][:, PL:2 * PL])
                nc.gpsimd.dma_start(agi[1, cs, :], xln[i][:, 3 * PL:4 * PL])
            nc.gpsimd.collective_compute(
                "AllGather", mybir.AluOpType.bypass,
                replica_groups=[list(range(NCORES))],
                ins=[agi[:]], outs=[ago[:]])
            with (tc.tile_pool(name="halo_ps", bufs=2, space="PSUM") as hps,
                  tc.tile_pool(name="hterm", bufs=2) as htp):
                for i in range(CT):
                    cs = slice(128 * i, 128 * (i + 1))
                    # lower halo <- senders' top face (idx 1), masks 0..7;
                    # upper halo <- senders' bottom face (idx 0), masks 8..15
                    for face, dst0, m0 in [(1, 0, 0), (0, 4 * PL, 8)]:
                        terms = []
                        for j in range(NCORES):
                            t = htp.tile([128, PL], BF16, name=f"ht{j}",
                                         tag=f"ht{j}")
                            nc.sync.dma_start(t[:], ago[j, face, cs, :])
                            terms.append((m0 + j, t))
                        for nb in range(PL // NB):
                            sl = slice(NB * nb, NB * (nb + 1))
                            ps = hps.tile([128, NB], F32, name="hh", tag="hh")
                            for ti, (mc, t) in enumerate(terms):
                                nc.tensor.matmul(ps[:], mid[mc][:], t[:, sl],
                                                 start=(ti == 0),
                                                 stop=(ti == NCORES - 1))
                            nc.scalar.activation(
                                xln[i][:, dst0 + NB * nb:dst0 + NB * (nb + 1)],
                                ps[:], Copy)

            # ---- phase G: MLP ----
            with (tc.tile_pool(name="hpadp", bufs=2) as hpad_pool,
                  tc.tile_pool(name="hpad1p", bufs=1) as hpad1_pool,
                  tc.tile_pool(name="wiw", bufs=2) as wiw,
                  tc.tile_pool(name="diag", bufs=1) as dpool,
                  tc.tile_pool(name="glueG", bufs=1) as glueG,
                  tc.tile_pool(name="wips", bufs=2, space="PSUM") as wips,
                  tc.tile_pool(name="cvps", bufs=1, space="PSUM") as cvps):

                def mlp_tile(tt, conv_out):
                    wall = wiw.tile([128, CT * 128], BF16, name="wiall",
                                    tag="wiall")
                    nc.sync.dma_start(wall[:], wiT[tt, :, :])
                    wts = [wall[:, 128 * k:128 * (k + 1)] for k in range(CT)]
                    hp = hpad_pool.tile([128, PADN], BF16, name="hpad",
                                        tag="hpad")
                    nc.gpsimd.memset(hp[:], 0.0)
                    h4 = hp.rearrange("p (z y x) -> p z y x", z=Z5, y=26, x=26)
                    for nb in range(Z5 * PL // NB):
                        sl = slice(NB * nb, NB * (nb + 1))
                        ps = wips.tile([128, NB], F32, name="wi_ps",
                                       tag="wi_ps")
                        for k in range(CT):
                            nc.tensor.matmul(ps[:], wts[k], xln[k][:, sl],
                                             start=(k == 0), stop=(k == CT - 1))
                        z, y0 = nb // 2, (nb % 2) * 12
                        nc.scalar.activation(h4[:, z, 1 + y0:13 + y0, 1:25],
                                             ps[:], Copy)
                    if tt in PE_TILES:
                        p4 = hp.rearrange("p (z y x) -> p z y x", z=Z5, y=26,
                                          x=26)
                        cps = [cvps.tile([128, NB], F32, name=f"cv{nb}",
                                         tag=f"cv{nb}")
                               for nb in range(VC // NB)]
                        dga = dpool.tile([128, KK * 128], BF16, name="dgall",
                                         tag="dgall")
                        nc.sync.dma_start(dga[:], dmlp[tt, :, :])
                        for ti, (dz, dy, dx) in enumerate(TAPS):
                            dg = dga[:, 128 * ti:128 * (ti + 1)]
                            for nb in range(VC // NB):
                                z, y0 = nb // 2, (nb % 2) * 12
                                sv = p4[:, 1 + z + dz,
                                        1 + y0 + dy:13 + y0 + dy,
                                        1 + dx:25 + dx]
                                nc.tensor.matmul(cps[nb][:], dg, sv,
                                                 start=(ti == 0),
                                                 stop=(ti == KK - 1))
                        return cps
                    hp1 = hpad1_pool.tile([128, PADN], BF16, name="hpad1",
                                          tag="hpad1")
                    nc.gpsimd.memset(hp1[:], 0.0)
                    nc.scalar.activation(hp1[:, 0:PADN - 1], hp[:, 1:PADN],
                                         Copy)
                    conv_dve(conv_out[:], hp[:], hp1[:], _dwk[tt])
                    return None

                for j in range(HT // 2):
                    c1 = glueG.tile([128, VC], BF16, name="conv1", tag="conv1")
                    p1 = mlp_tile(j, c1)
                    g1 = glueG.tile([128, VC], BF16, name="gelu1", tag="gelu1")
                    if p1 is not None:
                        for nb in range(VC // NB):
                            nc.scalar.activation(g1[:, NB * nb:NB * (nb + 1)],
                                                 p1[nb][:], Gelu)
                    else:
                        nc.scalar.activation(g1[:], c1[:], Gelu)
                    c2 = glueG.tile([128, VC], BF16, name="conv2", tag="conv2")
                    p2 = mlp_tile(j + HT // 2, c2)
                    if p2 is not None:
                        for nb in range(VC // NB):
                            nc.scalar.activation(c2[:, NB * nb:NB * (nb + 1)],
                                                 p2[nb][:], Copy)
                    nc.vector.tensor_mul(gate[j][:], g1[:], c2[:])

        # ---- phase H: Wo + residual ----
        ytp = ctx.enter_context(tc.tile_pool(name="ytp", bufs=1))
        y_t = [ytp.tile([128, VC], BF16, name=f"y{i}", tag=f"y{i}")
               for i in range(CT)]
        with (tc.tile_pool(name="wow", bufs=2) as wow,
              tc.tile_pool(name="wops", bufs=2, space="PSUM") as wops):
            for m in range(CT):
                wall = wow.tile([128, (HT // 2) * 128], BF16, name="woall",
                                tag="woall")
                nc.sync.dma_start(wall[:], woT[m, :, :])
                wts = [wall[:, 128 * k:128 * (k + 1)] for k in range(HT // 2)]
                for o, n in [(0, 512), (512, 512), (1024, 512), (1536, 192)]:
                    sl = slice(o, o + n)
                    ps = wops.tile([128, 512], F32, name="wo_ps", tag="wo_ps")
                    for k in range(HT // 2):
                        nc.tensor.matmul(ps[:, 0:n], wts[k], gate[k][:, sl],
                                         start=(k == 0), stop=False)
                    nc.tensor.matmul(ps[:, 0:n], id_t[:], xb[m][:, sl],
                                     start=False, stop=True)
                    nc.scalar.activation(y_t[m][:, sl], ps[:, 0:n], Copy)

        # ---- LN3 + output ----
        muB3, rsB3 = ln_stats(y_t, "ln3")
        with tc.tile_pool(name="glueH", bufs=2) as glueH:
            for i in range(CT):
                t1 = glueH.tile([128, VC], BF16, name="ln3t", tag="ln3t")
                nc.vector.tensor_sub(t1[:], y_t[i][:], muB3[:])
                nc.vector.tensor_mul(t1[:], t1[:], rsB3[:])
                of = glueH.tile([128, VC], F32, name="outf", tag="outf")
                nc.vector.tensor_scalar(of[:], t1[:],
                                        sm[i][:, S_N3W:S_N3W + 1],
                                        sm[i][:, S_N3B:S_N3B + 1],
                                        op0=MULT, op1=ADD)
                nc.gpsimd.dma_start(out[128 * i:128 * (i + 1), :], of[:])

    nc.compile()
    return nc


def _prep(inputs):
    bf = lambda a: np.ascontiguousarray(a).astype(BF)
    f32 = lambda a: np.ascontiguousarray(a, dtype=np.float32)
    x = f32(inputs["x"][0])
    xf = x.reshape(C, S, PL)

    smalls = np.zeros((C, 16), np.float32)
    smalls[:, 0] = f32(inputs["y"][0, 0])
    for i, k in enumerate(["lora_b1", "lora_b2", "tp_b", "attn_bv", "attn_bo",
                           "op_b", "n2_w", "n2_b", "n3_w", "n3_b", "gn_g",
                           "gn_b", "mod_b"]):
        smalls[:, i + 1] = f32(inputs[k])

    gind6 = np.zeros((CT, 128, G), np.float32)
    for j in range(CT):
        for p in range(128):
            gind6[j, p, (128 * j + p) // GD] = 1.0
    gexpT = np.zeros((G, C), np.float32)
    for c in range(C):
        gexpT[c // GD, c] = 1.0

    kn_W2 = f32(inputs["kn_W2"])
    w2r = kn_W2.reshape(C, KK, HID).transpose(1, 0, 2).reshape(KFLAT, HID)
    w2T = np.ascontiguousarray(w2r.T)
    knb2r = f32(inputs["kn_b2"]).reshape(C, KK).T.copy().reshape(-1)
    kn_W1 = f32(inputs["kn_W1"])

    mlp_dw = f32(inputs["mlp_dw"]).reshape(HID, KK)
    dmlp = np.zeros((HT, 128, KK, 128), np.float32)
    idx = np.arange(128)
    for tt in range(HT):
        for ti in range(KK):
            dmlp[tt, idx, ti, idx] = mlp_dw[128 * tt:128 * (tt + 1), ti]
    dmlp = dmlp.reshape(HT, 128, KK * 128)

    com = dict(
        smalls=smalls, knb2r=knb2r,
        gind6=gind6, gexpT=gexpT,
        ident=bf(np.eye(128, dtype=np.float32)),
        onesc=bf(np.ones((128, 1), np.float32)),
        loraW1T=bf(f32(inputs["lora_W1"]).T),
        loraW2T=bf(f32(inputs["lora_W2"]).T),
        tpWT=bf(f32(inputs["tp_W"]).T), avWT=bf(f32(inputs["attn_Wv"]).T),
        aoWT=bf(f32(inputs["attn_Wo"]).T), modWT=bf(f32(inputs["mod_W"]).T),
        opT=bf(f32(inputs["op_W"]).T),
        wiT=bf(f32(inputs["mlp_Wi"]).T.reshape(CT, 128, HT, 128)
               .transpose(2, 1, 0, 3).reshape(HT, 128, CT * 128)),
        woT=bf(f32(inputs["mlp_Wo"]).T.reshape(HT // 2, 128, CT, 128)
               .transpose(2, 1, 0, 3).reshape(CT, 128, (HT // 2) * 128)),
        dmlp=bf(dmlp), dwk=mlp_dw,
    )

    in_maps = []
    for i in range(NCORES):
        z0 = ZP * i
        xh = np.zeros((C, Z5, PL), np.float32)
        lo, hi = max(z0 - 1, 0), min(z0 + ZP + 1, S)
        xh[:, lo - (z0 - 1):lo - (z0 - 1) + (hi - lo)] = xf[:, lo:hi]
        hmask = np.zeros((128, 18), np.float32)
        if i > 0:
            hmask[:, i - 1] = 1.0
        if i < NCORES - 1:
            hmask[:, 8 + i + 1] = 1.0
        m = dict(com)
        m.update(
            x_halo=xh.reshape(C, Z5 * PL).astype(BF),
            knb1=f32(inputs["kn_b1"][W1R * i:W1R * (i + 1)]),
            halo_masks=hmask,
            w1sT=bf(kn_W1[W1R * i:W1R * (i + 1), :].T),
            w2sT=bf(w2T[W2K * i:W2K * (i + 1), :]),
        )
        in_maps.append(m)
    return in_maps


def kernel(**inputs) -> np.ndarray:
    if "nc" not in _CACHE:
        _CACHE["nc"] = build_program()
    nc = _CACHE["nc"]
    in_maps = _prep(inputs)
    res = run_bass_kernel_spmd(nc, in_maps, list(range(NCORES)))
    outs = [res.results[i]["out"].reshape(C, ZP, PL) for i in range(NCORES)]
    full = np.concatenate(outs, axis=1)
    return full.reshape(1, C, S, S, S).astype(np.float32)



# revision 31
# speedup vs baseline: 1.8739x; 1.8739x over previous
"""Bass/Trainium2 SPMD kernel for nn_Block3D (8 NeuronCores).

Spatial z-shard (24 planes -> 3 per core) with a 2-plane host-prepped halo:
each core computes cafm/xb/xln on 5 planes locally, so the depthwise MLP conv
needs no on-device halo exchange (no AllGather). The per-channel `mod` gate is
folded into the dynamic conv kernels. Depthwise 3x3x3 convs are split between
the PE (diagonal-weight matmuls) and DVE (tensor_scalar 4x + tensor_tensor 2x
chains). All DMAs ride the HWDGE path (nc.sync); kernel_net output is staged
as one [128,162] tile around a single AllReduce.
"""

from contextlib import ExitStack

import numpy as np
import ml_dtypes

import concourse.bass as bass
import concourse.bacc as bacc
import concourse.tile as tile
from concourse import mybir
from concourse.bass_utils import run_bass_kernel_spmd

BF = ml_dtypes.bfloat16
F32 = mybir.dt.float32
BF16 = mybir.dt.bfloat16

C = 768
G = 12
GD = 64
S = 24
HID = 4 * C
KK = 27
EPS = 1e-5
NCORES = 8
ZP = S // NCORES          # 3 own planes
PL = S * S                # 576
Z7 = ZP + 4               # 7 input planes (2-halo each side)
Z5 = ZP + 2               # 5 computed planes (1-halo each side)
PPL = 26 * 26             # 676 padded plane
XPN = Z7 * PPL            # 4732
HPN = Z5 * PPL            # 3380
V5 = Z5 * PL              # 2880
VC = ZP * PL              # 1728
CT = C // 128             # 6
HT = HID // 128           # 24
W1R = HID // NCORES       # 384 hidden rows per core
KFLAT = C * KK
NTOT = float(S * S * S)

TAPS = [(dz, dy, dx) for dz in (-1, 0, 1) for dy in (-1, 0, 1) for dx in (-1, 0, 1)]

# engine split for the depthwise convs (PE diag-matmul vs DVE chains)
DYN_PE = (0, 1, 2, 3)
MLP_PE = tuple(range(0, 13))

_CACHE = {}
DEBUG = False

Copy = mybir.ActivationFunctionType.Copy
Iden = mybir.ActivationFunctionType.Identity
Gelu = mybir.ActivationFunctionType.Gelu
Sigmoid = mybir.ActivationFunctionType.Sigmoid
Square = mybir.ActivationFunctionType.Square
Sqrt = mybir.ActivationFunctionType.Sqrt
Relu = mybir.ActivationFunctionType.Relu
ADD = mybir.AluOpType.add
SUB = mybir.AluOpType.subtract
MULT = mybir.AluOpType.mult

(S_Y, S_LB1, S_LB2, S_TPB, S_ABV, S_ABO, S_OPB, S_N2W, S_N2B, S_N3W, S_N3B,
 S_GNG, S_GNB, S_MODB, S_MLO, S_MHI) = range(16)


def build_program():
    nc = bacc.Bacc("TRN2", target_bir_lowering=False)

    def dram_in(name, shape, dtype=F32):
        return nc.declare_dram_parameter(name, list(shape), dtype, isOutput=False)

    xpad_in = dram_in("xpad", [C, XPN], BF16)
    smalls = dram_in("smalls", [C, 16])
    knb1 = dram_in("knb1", [W1R])
    knb2t = dram_in("knb2t", [C, KK])
    ident = dram_in("ident", [128, 128], BF16)
    onesc = dram_in("onesc", [128, 1], BF16)
    onesr_in = dram_in("onesr", [1, 128], BF16)
    gind_in = dram_in("gind6", [CT, 128, G])
    gexpT = dram_in("gexpT", [G, C])
    gemvW = dram_in("gemvW", [6, C, C], BF16)     # lora1,lora2,tp,av,ao,op (all .T)
    modWT = dram_in("modWT", [2 * C, C], BF16)
    w1sT = dram_in("w1sT", [2 * C, W1R], BF16)
    w2sT = dram_in("w2sT", [W1R, KFLAT], BF16)
    wiT = dram_in("wiT", [HT, 128, CT * 128], BF16)
    woT = dram_in("woT", [CT, 128, (HT // 2) * 128], BF16)
    dmlp = dram_in("dmlp", [HT, 128, KK * 128], BF16)
    dwk_in = dram_in("dwk", [HID, KK])
    out = nc.declare_dram_parameter("out", [C, VC], F32, isOutput=True)
    if DEBUG:
        dbg_mod = nc.declare_dram_parameter("dbg_mod", [C, 1], F32, isOutput=True)
        dbg_kern = nc.declare_dram_parameter("dbg_kern", [C, KK], F32, isOutput=True)
        dbg_dyn = nc.declare_dram_parameter("dbg_dyn", [C, V5], F32, isOutput=True)
        dbg_xb = nc.declare_dram_parameter("dbg_xb", [C, V5], F32, isOutput=True)
        dbg_xln = nc.declare_dram_parameter("dbg_xln", [C, V5], F32, isOutput=True)
        dbg_gate = nc.declare_dram_parameter("dbg_gate", [128, VC], F32, isOutput=True)
        dbg_h = nc.declare_dram_parameter("dbg_h", [128, HPN], F32, isOutput=True)
        dbg_c1 = nc.declare_dram_parameter("dbg_c1", [128, VC], F32, isOutput=True)
        dbg_y = nc.declare_dram_parameter("dbg_y", [C, VC], F32, isOutput=True)

    with tile.TileContext(nc) as tc, ExitStack() as ctx:
        persist = ctx.enter_context(tc.tile_pool(name="persist", bufs=1))
        dram = ctx.enter_context(tc.tile_pool(name="dram", bufs=1, space="DRAM"))
        gpool = ctx.enter_context(tc.tile_pool(name="gemv", bufs=2))
        psA = ctx.enter_context(tc.tile_pool(name="psA", bufs=1, space="PSUM"))
        xbpool = ctx.enter_context(tc.tile_pool(name="xbpool", bufs=1))
        ytpool = ctx.enter_context(tc.tile_pool(name="ytpool", bufs=1))

        xpool_cm = tc.tile_pool(name="xpool", bufs=1)
        xpool = xpool_cm.__enter__()
        xp = [xpool.tile([128, XPN], BF16, name=f"xp{i}", tag=f"xp{i}")
              for i in range(CT)]
        for i in range(CT):
            nc.sync.dma_start(xp[i][:], xpad_in[128 * i:128 * (i + 1), :])
        x4 = [t.rearrange("p (z y x) -> p z y x", z=Z7, y=26, x=26) for t in xp]

        sm = [persist.tile([128, 16], F32, name=f"sm{i}", tag=f"sm{i}")
              for i in range(CT)]
        for i in range(CT):
            nc.sync.dma_start(sm[i][:], smalls[128 * i:128 * (i + 1), :])
        id_t = persist.tile([128, 128], BF16, name="identt", tag="identt")
        nc.sync.dma_start(id_t[:], ident[:, :])
        ones_t = persist.tile([128, 1], BF16, name="onest", tag="onest")
        nc.sync.dma_start(ones_t[:], onesc[:, :])
        onesr_t = persist.tile([1, 128], BF16, name="onesrt", tag="onesrt")
        nc.sync.dma_start(onesr_t[:], onesr_in[:, :])
        eps_t = persist.tile([128, 1], F32, name="epst", tag="epst")
        nc.vector.memset(eps_t[:], EPS)
        junk = xpool.tile([128, VC], BF16, name="junk", tag="junk")
        junk5 = junk.rearrange("p (z y x) -> p z y x", z=ZP, y=24, x=24)
        opT = persist.tile([128, CT * C], BF16, name="opT", tag="opT")
        nc.sync.dma_start(
            opT[:], bass.AP(tensor=gemvW, offset=5 * C * C,
                            ap=[[C, 128], [128 * C, CT], [1, C]]))
        gi_all = persist.tile([128, CT * G], F32, name="giall", tag="giall")
        nc.sync.dma_start(
            gi_all[:], bass.AP(tensor=gind_in, offset=0,
                               ap=[[G, 128], [128 * G, CT], [1, G]]))
        knb1_t = persist.tile([128, 3], F32, name="knb1t", tag="knb1t")
        nc.sync.dma_start(
            knb1_t[:], bass.AP(tensor=knb1, offset=0, ap=[[1, 128], [128, 3]]))
        kb2 = [persist.tile([128, KK], F32, name=f"kb2{i}", tag=f"kb2{i}")
               for i in range(CT)]
        for i in range(CT):
            nc.sync.dma_start(kb2[i][:], knb2t[128 * i:128 * (i + 1), :])
        dwk_map = {}
        for t in range(HT):
            if t not in MLP_PE:
                d = persist.tile([128, KK], F32, name=f"dwk{t}", tag=f"dwk{t}")
                nc.sync.dma_start(d[:], dwk_in[128 * t:128 * (t + 1), :])
                dwk_map[t] = d

        gwpool_cm = tc.tile_pool(name="gwpool", bufs=2)
        gwpool = gwpool_cm.__enter__()
        w1pool_cm = tc.tile_pool(name="w1pool", bufs=1)
        w1pool = w1pool_cm.__enter__()

        def load_gwbuf(dram_t, off):
            t = gwpool.tile([128, CT * C], BF16, name="gwbuf", tag="gwbuf")
            nc.sync.dma_start(
                t[:], bass.AP(tensor=dram_t, offset=off,
                              ap=[[C, 128], [128 * C, CT], [1, C]]))
            return t

        w1t = [w1pool.tile([128, CT * W1R], BF16, name=f"w1t{h}",
                           tag=f"w1t{h}") for h in range(2)]
        for h in range(2):
            nc.sync.dma_start(
                w1t[h][:], bass.AP(tensor=w1sT, offset=h * C * W1R,
                                   ap=[[W1R, 128], [128 * W1R, CT], [1, W1R]]))

        # ------------ phase A: vc partial sums + AR1 ------------
        vcs = persist.tile([128, CT], F32, name="vcs", tag="vcs")
        for i in range(CT):
            nc.scalar.activation(junk5[:, :, :, :], x4[i][:, 2:5, 1:25, 1:25],
                                 Copy, accum_out=vcs[:, i:i + 1])
        ar1_in = dram.tile([C], F32, name="ar1i", tag="ar1i")
        ar1_out = dram.tile([C], F32, name="ar1o", tag="ar1o",
                            addr_space="Shared")
        nc.sync.dma_start(
            bass.AP(tensor=ar1_in[:].tensor, offset=ar1_in[:].offset,
                    ap=[[1, 128], [128, CT]]), vcs[:])
        nc.gpsimd.collective_compute(
            "AllReduce", ADD, replica_groups=[list(range(NCORES))],
            ins=[ar1_in[:]], outs=[ar1_out[:]])

        # ------------ phase B: text gemv chain (overlaps AR1) ------------
        def gemv(wt, in_cols, nm, act, bias_cols, tag, scale=1.0, odt=BF16):
            outs = []
            for m in range(nm):
                ps = psA.tile([128, 1], F32, name="ps_small", tag="ps_small")
                for k in range(CT):
                    nc.tensor.matmul(
                        ps[:], wt[:, C * k + 128 * m:C * k + 128 * m + 128],
                        in_cols[k][:], start=(k == 0), stop=(k == CT - 1))
                o = gpool.tile([128, 1], odt, name=f"{tag}o{m}",
                               tag=f"{tag}o{m}")
                bias = bias_cols[m] if bias_cols is not None else 0.0
                nc.scalar.activation(o[:], ps[:], act, bias=bias, scale=scale)
                outs.append(o)
            return outs

        t_cols = []
        for i in range(CT):
            t = gpool.tile([128, 1], BF16, name=f"tc{i}", tag=f"tc{i}")
            nc.scalar.activation(t[:], sm[i][:, S_Y:S_Y + 1], Copy)
            t_cols.append(t)
        h1 = gemv(load_gwbuf(gemvW, 0 * C * C), t_cols, CT, Relu,
                  [sm[i][:, S_LB1:S_LB1 + 1] for i in range(CT)], "lw1")
        h2 = gemv(load_gwbuf(gemvW, 1 * C * C), h1, CT, Iden,
                  [sm[i][:, S_LB2:S_LB2 + 1] for i in range(CT)], "lw2")
        tp = gemv(load_gwbuf(gemvW, 2 * C * C), h2, CT, Iden,
                  [sm[i][:, S_TPB:S_TPB + 1] for i in range(CT)], "tpw")
        av = gemv(load_gwbuf(gemvW, 3 * C * C), tp, CT, Iden,
                  [sm[i][:, S_ABV:S_ABV + 1] for i in range(CT)], "avw")
        attn = gemv(load_gwbuf(gemvW, 4 * C * C), av, CT, Iden,
                    [sm[i][:, S_ABO:S_ABO + 1] for i in range(CT)], "aow")
        mw = [load_gwbuf(modWT, h * C * C) for h in range(2)]

        # ------------ phase C: post-AR1 gemvs ------------
        vc_cols = []
        for i in range(CT):
            col = gpool.tile([128, 1], F32, name=f"vcc{i}", tag=f"vcc{i}")
            nc.sync.dma_start(
                col[:], bass.AP(tensor=ar1_out[:].tensor,
                                offset=ar1_out[:].offset + 128 * i,
                                ap=[[1, 128], [128, 1]]))
            cb = gpool.tile([128, 1], BF16, name=f"cmb{i}", tag=f"cmb{i}")
            nc.scalar.activation(cb[:], col[:], Copy, scale=1.0 / NTOT)
            vc_cols.append(cb)
        halves = [vc_cols, attn]

        def gemv2h(wts, stride, nm, act, bias_cols, tag, odt=F32):
            outs = []
            for m in range(nm):
                ps = psA.tile([128, 1], F32, name="ps_small", tag="ps_small")
                for h in range(2):
                    for k in range(CT):
                        nc.tensor.matmul(
                            ps[:], wts[h][:, k * stride + 128 * m:
                                          k * stride + 128 * m + 128],
                            halves[h][k][:],
                            start=(h == 0 and k == 0),
                            stop=(h == 1 and k == CT - 1))
                o = gpool.tile([128, 1], odt, name=f"{tag}o{m}",
                               tag=f"{tag}o{m}")
                nc.scalar.activation(o[:], ps[:], act, bias=bias_cols[m])
                outs.append(o)
            return outs

        mod = gemv2h(mw, C, CT, Sigmoid,
                     [sm[i][:, S_MODB:S_MODB + 1] for i in range(CT)], "modw")
        kp1 = gemv2h(w1t, W1R, 3, Relu,
                     [knb1_t[:, m:m + 1] for m in range(3)], "w1s", odt=BF16)

        # ------------ phase D: kernel_net gemv2 + AR2 ------------
        ko_all = persist.tile([128, KK * CT], F32, name="koall", tag="koall")
        with (tc.tile_pool(name="kseq", bufs=2) as kseq,
              tc.tile_pool(name="kps", bufs=2, space="PSUM") as kps):
            for t in range(KK):
                wt = kseq.tile([128, 3 * C], BF16, name="w2t", tag="w2t")
                nc.sync.dma_start(
                    wt[:], bass.AP(tensor=w2sT, offset=t * C,
                                   ap=[[KFLAT, 128], [128 * KFLAT, 3],
                                       [1, C]]))
                ps = kps.tile([128, CT], F32, name="g2ps", tag="g2ps")
                for m in range(CT):
                    for k in range(3):
                        nc.tensor.matmul(
                            ps[:, m:m + 1],
                            wt[:, k * C + 128 * m:k * C + 128 * m + 128],
                            kp1[k][:], start=(m == 0 and k == 0),
                            stop=(m == CT - 1 and k == 2))
                nc.scalar.activation(ko_all[:, CT * t:CT * (t + 1)], ps[:],
                                     Copy)
        ar2_in = dram.tile([128, KK * CT], F32, name="ar2i", tag="ar2i")
        ar2_out = dram.tile([128, KK * CT], F32, name="ar2o", tag="ar2o",
                            addr_space="Shared")
        nc.sync.dma_start(ar2_in[:, :], ko_all[:])
        nc.gpsimd.collective_compute(
            "AllReduce", ADD, replica_groups=[list(range(NCORES))],
            ins=[ar2_in[:]], outs=[ar2_out[:]])
        kraw = persist.tile([128, KK * CT], F32, name="kraw", tag="kraw")
        nc.sync.dma_start(kraw[:], ar2_out[:, :])
        kraw3 = kraw.rearrange("p (t i) -> p t i", t=KK, i=CT)

        w1pool_cm.__exit__(None, None, None)
        gwpool_cm.__exit__(None, None, None)

        # kernels: +bias, fold mod
        kernm = []
        for i in range(CT):
            km = persist.tile([128, KK], F32, name=f"kernm{i}", tag=f"kernm{i}")
            nc.vector.tensor_tensor(km[:], kraw3[:, :, i], kb2[i][:], op=ADD)
            nc.vector.tensor_scalar_mul(km[:], km[:], mod[i][:, 0:1])
            kernm.append(km)

        if DEBUG:
            for i in range(CT):
                nc.gpsimd.dma_start(dbg_mod[128 * i:128 * (i + 1), :], mod[i][:])
                nc.gpsimd.dma_start(dbg_kern[128 * i:128 * (i + 1), :], kernm[i][:])

        # ------------ phase E: dynamic depthwise conv + GN stats ------------
        dynpool_cm = tc.tile_pool(name="dynpool", bufs=1)
        dynpool = dynpool_cm.__enter__()
        dyn = [dynpool.tile([128, V5], BF16, name=f"dyn{i}", tag=f"dyn{i}")
               for i in range(CT)]
        dyn4 = [t.rearrange("p (z y x) -> p z y x", z=Z5, y=24, x=24)
                for t in dyn]

        dgpool_cm = tc.tile_pool(name="dgpool", bufs=2)
        dgpool = dgpool_cm.__enter__()

        def build_diag(i):
            d = dgpool.tile([128, KK * 128], BF16, name="dg", tag="dg")
            for t in range(KK):
                nc.vector.tensor_scalar_mul(d[:, 128 * t:128 * (t + 1)],
                                            id_t[:], kernm[i][:, t:t + 1])
            return d

        def dyn_pe_planes(i, dgt, zos, cpool):
            for zo in zos:
                for half in range(2):
                    y0 = 12 * half
                    cp = cpool.tile([128, 288], F32, name=f"dcp{half}",
                                    tag=f"dcp{half}")
                    for ti, (dz, dy, dx) in enumerate(TAPS):
                        nc.tensor.matmul(
                            cp[:], dgt[:, 128 * ti:128 * (ti + 1)],
                            x4[i][:, zo + 1 + dz, 1 + y0 + dy:13 + y0 + dy,
                                  1 + dx:25 + dx],
                            start=(ti == 0), stop=(ti == KK - 1))
                    nc.scalar.activation(
                        dyn4[i][:, zo, y0:y0 + 12, :], cp[:], Copy)

        def dyn_dve_planes(i, zo0, nz, tpool):
            dst = dyn4[i][:, zo0:zo0 + nz, :, :]
            for ti, (dz, dy, dx) in enumerate(TAPS):
                src = x4[i][:, zo0 + 1 + dz:zo0 + 1 + dz + nz,
                            1 + dy:25 + dy, 1 + dx:25 + dx]
                if ti == 0:
                    nc.vector.tensor_scalar_mul(dst, src, kernm[i][:, 0:1])
                else:
                    tmp = tpool.tile([128, 3 * PL], BF16, name="dtmp",
                                     tag="dtmp")
                    t4 = tmp.rearrange("p (z y x) -> p z y x", z=3, y=24,
                                       x=24)[:, 0:nz, :, :]
                    nc.vector.tensor_scalar_mul(t4, src,
                                                kernm[i][:, ti:ti + 1])
                    nc.vector.tensor_tensor(dst, dst, t4, op=ADD)

        gst = persist.tile([128, 2 * CT], F32, name="gst", tag="gst")
        ar3_in = dram.tile([G, 2], F32, name="ar3i", tag="ar3i")
        ar3_out = dram.tile([G, 2], F32, name="ar3o", tag="ar3o",
                            addr_space="Shared")
        gsb = persist.tile([G, 2], F32, name="gsb", tag="gsb")
        with (tc.tile_pool(name="dcpool", bufs=1, space="PSUM") as dcpool,
              tc.tile_pool(name="dtpool", bufs=2) as dtpool,
              tc.tile_pool(name="gnps", bufs=1, space="PSUM") as gnps):
            # own planes (zo 1..3) first, stats, then halo planes under AR3
            dg_live = {}
            for i in range(CT):
                if i in DYN_PE:
                    dg_live[i] = build_diag(i)
                    dyn_pe_planes(i, dg_live[i], (1, 2, 3), dcpool)
                else:
                    dyn_dve_planes(i, 1, 3, dtpool)
                nc.scalar.activation(junk[:], dyn[i][:, PL:4 * PL],
                                     Copy, accum_out=gst[:, 2 * i:2 * i + 1])
                nc.scalar.activation(junk[:], dyn[i][:, PL:4 * PL],
                                     Square,
                                     accum_out=gst[:, 2 * i + 1:2 * i + 2])
            gps = gnps.tile([G, 2], F32, name="gps", tag="gps")
            for i in range(CT):
                nc.tensor.matmul(gps[:], gi_all[:, G * i:G * (i + 1)],
                                 gst[:, 2 * i:2 * i + 2], start=(i == 0),
                                 stop=(i == CT - 1))
            nc.scalar.activation(gsb[:], gps[:], Copy)
            nc.sync.dma_start(ar3_in[:, :], gsb[:])
            nc.gpsimd.collective_compute(
                "AllReduce", ADD, replica_groups=[list(range(NCORES))],
                ins=[ar3_in[:]], outs=[ar3_out[:]])
            for i in range(CT):
                if i in DYN_PE:
                    dg2 = build_diag(i)
                    dyn_pe_planes(i, dg2, (0, 4), dcpool)
                else:
                    dyn_dve_planes(i, 0, 1, dtpool)
                    dyn_dve_planes(i, 4, 1, dtpool)
        dgpool_cm.__exit__(None, None, None)

        if DEBUG:
            for i in range(CT):
                nc.gpsimd.dma_start(dbg_dyn[128 * i:128 * (i + 1), :], dyn[i][:])

        # ------------ GN scale/shift + fold into opT ------------
        gstat = persist.tile([G, 2], F32, name="gstat", tag="gstat")
        nc.sync.dma_start(gstat[:], ar3_out[:, :])
        NGRP = float(GD) * NTOT
        gmr = persist.tile([G, 2], F32, name="gmr", tag="gmr")
        nc.scalar.activation(gmr[:, 0:1], gstat[:, 0:1], Copy, scale=1.0 / NGRP)
        musq = persist.tile([G, 1], F32, name="musq", tag="musq")
        nc.scalar.square(musq[:], gmr[:, 0:1])
        var = persist.tile([G, 1], F32, name="gvar", tag="gvar")
        nc.vector.tensor_scalar(var[:], gstat[:, 1:2], 1.0 / NGRP, None,
                                op0=MULT)
        nc.vector.tensor_sub(var[:], var[:], musq[:])
        nc.scalar.activation(var[:], var[:], Sqrt, bias=eps_t[0:G, 0:1])
        nc.vector.reciprocal(gmr[:, 1:2], var[:])

        cafm_shift = []
        gsc = []
        for i in range(CT):
            ge = gpool.tile([G, 128], F32, name=f"gexp{i}", tag=f"gexp{i}")
            nc.sync.dma_start(ge[:], gexpT[:, 128 * i:128 * (i + 1)])
            ps = psA.tile([128, 2], F32, name="ps_sm2", tag="ps_sm2")
            nc.tensor.matmul(ps[:], ge[:], gmr[:], start=True, stop=True)
            mu_c = persist.tile([128, 2], F32, name=f"muc{i}", tag=f"muc{i}")
            nc.scalar.activation(mu_c[:], ps[:], Copy)
            a = persist.tile([128, 1], F32, name=f"gsc{i}", tag=f"gsc{i}")
            nc.vector.tensor_mul(a[:], sm[i][:, S_GNG:S_GNG + 1], mu_c[:, 1:2])
            b = persist.tile([128, 1], F32, name=f"gsh{i}", tag=f"gsh{i}")
            nc.vector.tensor_mul(b[:], mu_c[:, 0:1], a[:])
            nc.vector.tensor_sub(b[:], sm[i][:, S_GNB:S_GNB + 1], b[:])
            gsc.append(a)
            bb = gpool.tile([128, 1], BF16, name=f"gshb{i}", tag=f"gshb{i}")
            nc.scalar.activation(bb[:], b[:], Copy)
            cafm_shift.append(bb)
        cb_cols = []
        for m in range(CT):
            ps = psA.tile([128, 1], F32, name="ps_small", tag="ps_small")
            for k in range(CT):
                nc.tensor.matmul(ps[:], opT[:, 768 * k + 128 * m:
                                            768 * k + 128 * m + 128],
                                 cafm_shift[k][:], start=(k == 0),
                                 stop=(k == CT - 1))
            o = persist.tile([128, 1], F32, name=f"cbc{m}", tag=f"cbc{m}")
            nc.scalar.activation(o[:], ps[:], Iden,
                                 bias=sm[m][:, S_OPB:S_OPB + 1])
            cb_cols.append(o)
        for k in range(CT):
            nc.vector.tensor_scalar_mul(opT[:, 768 * k:768 * (k + 1)],
                                        opT[:, 768 * k:768 * (k + 1)],
                                        gsc[k][:])

        # ------------ phase F1: cafm matmul + xb ------------
        xb = [xbpool.tile([128, V5], BF16, name=f"xb{m}", tag=f"xb{m}")
              for m in range(CT)]
        xb4 = [t.rearrange("p (z y x) -> p z y x", z=Z5, y=24, x=24)
               for t in xb]
        CH6 = [(o, min(512, V5 - o)) for o in range(0, V5, 512)]
        with (tc.tile_pool(name="f1ps", bufs=2, space="PSUM") as f1ps,
              tc.tile_pool(name="f1t", bufs=2) as f1t):
            for m in range(CT):
                tca = f1t.tile([128, V5], BF16, name="tcafm", tag="tcafm")
                for o, n in CH6:
                    ps = f1ps.tile([128, 512], F32, name="f1p", tag="f1p")
                    for k in range(CT):
                        nc.tensor.matmul(ps[:, 0:n],
                                         opT[:, 768 * k + 128 * m:
                                             768 * k + 128 * m + 128],
                                         dyn[k][:, o:o + n], start=(k == 0),
                                         stop=(k == CT - 1))
                    nc.scalar.activation(tca[:, o:o + n], ps[:, 0:n], Iden,
                                         bias=cb_cols[m][:])
                t4 = tca.rearrange("p (z y x) -> p z y x", z=Z5, y=24, x=24)
                nc.vector.tensor_tensor(xb4[m][:, :, :, :], t4[:, :, :, :],
                                        x4[m][:, 1:6, 1:25, 1:25], op=MULT)
        if DEBUG:
            for m in range(CT):
                nc.gpsimd.dma_start(dbg_xb[128 * m:128 * (m + 1), :], xb[m][:])
        dynpool_cm.__exit__(None, None, None)
        xpool_cm.__exit__(None, None, None)

        # ------------ LN helper ------------
        def ln_stats(tiles, nv, tag, mupool):
            """Per-voxel mean/rstd over channels -> [128, nv] bf16 tiles.
            ones_t carries 1/C so the matmuls produce mean / E[x^2]."""
            nch = [(o, min(480, nv - o)) for o in range(0, nv, 480)]
            muB = mupool.tile([128, nv], BF16, name=f"{tag}muB",
                              tag=f"{tag}muB")
            rsB = mupool.tile([128, nv], BF16, name=f"{tag}rsB",
                              tag=f"{tag}rsB")
            with (tc.tile_pool(name=f"{tag}ps", bufs=1, space="PSUM") as lps,
                  tc.tile_pool(name=f"{tag}sq", bufs=2) as sqp,
                  tc.tile_pool(name=f"{tag}rw", bufs=2) as rwp):
                for o, n in nch:
                    p1 = lps.tile([1, 480], F32, name="s1", tag="s1")
                    p2 = lps.tile([1, 480], F32, name="s2", tag="s2")
                    for k in range(CT):
                        nc.tensor.matmul(p1[:, 0:n], ones_t[:],
                                         tiles[k][:, o:o + n],
                                         start=(k == 0), stop=(k == CT - 1))
                    for k in range(CT):
                        q = sqp.tile([128, 480], BF16, name="sqc", tag="sqc")
                        nc.scalar.activation(q[:, 0:n], tiles[k][:, o:o + n],
                                             Square)
                        nc.tensor.matmul(p2[:, 0:n], ones_t[:], q[:, 0:n],
                                         start=(k == 0), stop=(k == CT - 1))
                    mubf = rwp.tile([1, 480], BF16, name="mubf", tag="mubf")
                    nc.scalar.activation(mubf[:, 0:n], p1[:, 0:n], Copy)
                    m2 = rwp.tile([1, 480], F32, name="m2", tag="m2")
                    nc.scalar.activation(m2[:, 0:n], p1[:, 0:n], Square)
                    vr = rwp.tile([1, 480], F32, name="vr", tag="vr")
                    nc.vector.tensor_sub(vr[:, 0:n], p2[:, 0:n], m2[:, 0:n])
                    nc.scalar.activation(vr[:, 0:n], vr[:, 0:n], Sqrt,
                                         bias=eps_t[0:1, 0:1])
                    rbf = rwp.tile([1, 480], BF16, name="rbf", tag="rbf")
                    with nc.allow_low_precision(reason="rstd bcast in bf16"):
                        nc.vector.reciprocal(rbf[:, 0:n], vr[:, 0:n])
                    pb = lps.tile([128, 480], F32, name="bc", tag="bc")
                    nc.tensor.matmul(pb[:, 0:n], onesr_t[:], mubf[:, 0:n],
                                     start=True, stop=True)
                    nc.scalar.activation(muB[:, o:o + n], pb[:, 0:n], Copy)
                    pb2 = lps.tile([128, 480], F32, name="bc2", tag="bc2")
                    nc.tensor.matmul(pb2[:, 0:n], onesr_t[:], rbf[:, 0:n],
                                     start=True, stop=True)
                    nc.scalar.activation(rsB[:, o:o + n], pb2[:, 0:n], Copy)
            return muB, rsB

        # ------------ LN2 + edge masks ------------
        gatepool_cm = tc.tile_pool(name="gatepool", bufs=1)
        gatepool = gatepool_cm.__enter__()
        xlnpool_cm = tc.tile_pool(name="xlnpool", bufs=1)
        xlnpool = xlnpool_cm.__enter__()
        xln = [xlnpool.tile([128, V5], BF16, name=f"xln{k}", tag=f"xln{k}")
               for k in range(CT)]
        mupool_cm = tc.tile_pool(name="mupool", bufs=1)
        mupool = mupool_cm.__enter__()
        muB, rsB = ln_stats(xb, V5, "ln2", mupool)
        with tc.tile_pool(name="lnap", bufs=2) as lnap:
            for k in range(CT):
                t1 = lnap.tile([128, V5], BF16, name="lnt1", tag="lnt1")
                nc.vector.tensor_sub(t1[:], xb[k][:], muB[:])
                nc.vector.tensor_mul(t1[:], t1[:], rsB[:])
                nc.vector.tensor_scalar(xln[k][:], t1[:],
                                        sm[k][:, S_N2W:S_N2W + 1],
                                        sm[k][:, S_N2B:S_N2B + 1],
                                        op0=MULT, op1=ADD)
                nc.vector.tensor_scalar_mul(xln[k][:, 0:PL], xln[k][:, 0:PL],
                                            sm[k][:, S_MLO:S_MLO + 1])
                nc.vector.tensor_scalar_mul(xln[k][:, 4 * PL:5 * PL],
                                            xln[k][:, 4 * PL:5 * PL],
                                            sm[k][:, S_MHI:S_MHI + 1])

        # ------------ MLP: Wi + depthwise conv + gate ------------
        if DEBUG:
            for k in range(CT):
                nc.gpsimd.dma_start(dbg_xln[128 * k:128 * (k + 1), :], xln[k][:])
        mupool_cm.__exit__(None, None, None)
        gate = [gatepool.tile([128, VC], BF16, name=f"gate{j}", tag=f"gate{j}")
                for j in range(HT // 2)]
        hppool_cm = tc.tile_pool(name="hppool", bufs=1)
        hppool = hppool_cm.__enter__()
        hpads = [hppool.tile([128, HPN], BF16, name=f"hpad{b}", tag=f"hpad{b}")
                 for b in range(2)]
        for b in range(2):
            nc.vector.memset(hpads[b][:], 0.0)
        hp4s = [t.rearrange("p (z y x) -> p z y x", z=Z5, y=26, x=26)
                for t in hpads]

        with (tc.tile_pool(name="wiw", bufs=2) as wiw,
              tc.tile_pool(name="dga", bufs=2) as dgap,
              tc.tile_pool(name="wips", bufs=1, space="PSUM") as wips,
              tc.tile_pool(name="cvps", bufs=1, space="PSUM") as cvps,
              tc.tile_pool(name="mlpt", bufs=1) as mlpt):

            def mlp_tile(t, half, dst, act):
                """conv(dw3(Wi_t @ xln)) own 3 planes -> act(conv) into dst
                (PE: per-block psum evac; DVE: acc chain + final act)."""
                hp, hp4 = hpads[half], hp4s[half]
                wall = wiw.tile([128, CT * 128], BF16, name="wiall",
                                tag="wiall")
                nc.sync.dma_start(wall[:], wiT[t, :, :])
                for z in range(Z5):
                    for halfy in range(2):
                        y0 = 12 * halfy
                        pz = wips.tile([128, 288], F32, name=f"wip{halfy}",
                                       tag=f"wip{halfy}")
                        for k in range(CT):
                            nc.tensor.matmul(
                                pz[:],
                                wall[:, 128 * k:128 * (k + 1)],
                                xln[k][:, z * PL + y0 * 24:
                                       z * PL + y0 * 24 + 288],
                                start=(k == 0), stop=(k == CT - 1))
                        nc.scalar.activation(
                            hp4[:, z, 1 + y0:13 + y0, 1:25],
                            pz.rearrange("p (y x) -> p y x", y=12, x=24)[
                                :, :, :], Copy)
                if DEBUG and t == 0:
                    nc.gpsimd.dma_start(dbg_h[:, :], hp[:])
                if t in MLP_PE:
                    dgt = dgap.tile([128, KK * 128], BF16, name="dgall",
                                    tag="dgall")
                    nc.sync.dma_start(dgt[:], dmlp[t, :, :])
                    for zo in range(3):
                        for halfy in range(2):
                            b = 2 * zo + halfy
                            y0 = 12 * halfy
                            cp = cvps.tile([128, 288], F32,
                                           name=f"mcp{b % 2}",
                                           tag=f"mcp{b % 2}")
                            for ti, (dz, dy, dx) in enumerate(TAPS):
                                nc.tensor.matmul(
                                    cp[:], dgt[:, 128 * ti:128 * (ti + 1)],
                                    hp4[:, 1 + zo + dz,
                                        1 + y0 + dy:13 + y0 + dy,
                                        1 + dx:25 + dx],
                                    start=(ti == 0), stop=(ti == KK - 1))
                            nc.scalar.activation(
                                dst[:, 288 * b:288 * (b + 1)], cp[:], act)
                    return dst
                acc = mlpt.tile([128, VC], BF16, name=f"macc{half}",
                                tag=f"macc{half}")
                a4 = acc.rearrange("p (z y x) -> p z y x", z=3, y=24, x=24)
                kw = dwk_map[t]
                for ti, (dz, dy, dx) in enumerate(TAPS):
                    src = hp4[:, 1 + dz:4 + dz, 1 + dy:25 + dy, 1 + dx:25 + dx]
                    if ti == 0:
                        nc.vector.tensor_scalar_mul(a4[:, :, :, :], src,
                                                    kw[:, 0:1])
                    else:
                        tmp = mlpt.tile([128, VC], BF16, name="mtmp",
                                        tag="mtmp")
                        t4 = tmp.rearrange("p (z y x) -> p z y x", z=3, y=24,
                                           x=24)
                        nc.vector.tensor_scalar_mul(t4[:, :, :, :], src,
                                                    kw[:, ti:ti + 1])
                        nc.vector.tensor_tensor(a4[:, :, :, :], a4[:, :, :, :],
                                                t4[:, :, :, :], op=ADD)
                if act is Gelu:
                    nc.scalar.activation(dst[:], acc[:], Gelu)
                    return dst
                return acc

            for j in range(HT // 2):
                mlp_tile(j, 0, gate[j], Gelu)
                if DEBUG and j == 0:
                    nc.gpsimd.dma_start(dbg_c1[:, :], gate[0][:])
                c2b = mlpt.tile([128, VC], BF16, name="conv2", tag="conv2")
                c2 = mlp_tile(j + HT // 2, 1, c2b, Copy)
                nc.vector.tensor_mul(gate[j][:], gate[j][:], c2[:])
        if DEBUG:
            nc.gpsimd.dma_start(dbg_gate[:, :], gate[0][:])
        hppool_cm.__exit__(None, None, None)
        xlnpool_cm.__exit__(None, None, None)

        # ------------ Wo + residual ------------
        y_t = [ytpool.tile([128, VC], BF16, name=f"y{m}", tag=f"y{m}")
               for m in range(CT)]
        CH4 = [(0, 512), (512, 512), (1024, 512), (1536, 192)]
        with (tc.tile_pool(name="wow", bufs=2) as wow,
              tc.tile_pool(name="wops", bufs=2, space="PSUM") as wops):
            for m in range(CT):
                wall = wow.tile([128, (HT // 2) * 128], BF16, name="woall",
                                tag="woall")
                nc.sync.dma_start(wall[:], woT[m, :, :])
                for o, n in CH4:
                    ps = wops.tile([128, 512], F32, name="wo_ps", tag="wo_ps")
                    for k in range(HT // 2):
                        nc.tensor.matmul(ps[:, 0:n],
                                         wall[:, 128 * k:128 * (k + 1)],
                                         gate[k][:, o:o + n],
                                         start=(k == 0), stop=False)
                    nc.tensor.matmul(ps[:, 0:n], id_t[:],
                                     xb[m][:, PL + o:PL + o + n],
                                     start=False, stop=True)
                    nc.scalar.activation(y_t[m][:, o:o + n], ps[:, 0:n], Copy)

        if DEBUG:
            for m in range(CT):
                nc.gpsimd.dma_start(dbg_y[128 * m:128 * (m + 1), :], y_t[m][:])

        # ------------ LN3 + output ------------
        gatepool_cm.__exit__(None, None, None)
        mu3_cm = tc.tile_pool(name="mupool3", bufs=1)
        mupool3 = mu3_cm.__enter__()
        muB3, rsB3 = ln_stats(y_t, VC, "ln3", mupool3)
        with tc.tile_pool(name="lnap3", bufs=2) as lnap3:
            for k in range(CT):
                t1 = lnap3.tile([128, VC], BF16, name="ln3t", tag="ln3t")
                nc.vector.tensor_sub(t1[:], y_t[k][:], muB3[:])
                nc.vector.tensor_mul(t1[:], t1[:], rsB3[:])
                of = lnap3.tile([128, VC], F32, name="outf", tag="outf")
                nc.vector.tensor_scalar(of[:], t1[:],
                                        sm[k][:, S_N3W:S_N3W + 1],
                                        sm[k][:, S_N3B:S_N3B + 1],
                                        op0=MULT, op1=ADD)
                nc.sync.dma_start(out[128 * k:128 * (k + 1), :], of[:])
        mu3_cm.__exit__(None, None, None)

    nc.compile()
    return nc


def _prep(inputs):
    bf = lambda a: np.ascontiguousarray(a).astype(BF)
    f32 = lambda a: np.ascontiguousarray(a, dtype=np.float32)
    x = f32(inputs["x"][0]).reshape(C, S, S, S)

    smalls_c = np.zeros((C, 16), np.float32)
    smalls_c[:, 0] = f32(inputs["y"][0, 0])
    for i, k in enumerate(["lora_b1", "lora_b2", "tp_b", "attn_bv", "attn_bo",
                           "op_b", "n2_w", "n2_b", "n3_w", "n3_b", "gn_g",
                           "gn_b", "mod_b"]):
        smalls_c[:, i + 1] = f32(inputs[k])

    gind6 = np.zeros((CT, 128, G), np.float32)
    for j in range(CT):
        for p in range(128):
            gind6[j, p, (128 * j + p) // GD] = 1.0
    gexpT = np.zeros((G, C), np.float32)
    for c in range(C):
        gexpT[c // GD, c] = 1.0

    gemv_stack = np.stack([
        f32(inputs["lora_W1"]).T, f32(inputs["lora_W2"]).T,
        f32(inputs["tp_W"]).T, f32(inputs["attn_Wv"]).T,
        f32(inputs["attn_Wo"]).T, f32(inputs["op_W"]).T])

    kn_W2 = f32(inputs["kn_W2"])
    w2r = kn_W2.reshape(C, KK, HID).transpose(1, 0, 2).reshape(KFLAT, HID)
    w2T = np.ascontiguousarray(w2r.T)          # [HID, KFLAT]
    kn_W1 = f32(inputs["kn_W1"])

    mlp_dw = f32(inputs["mlp_dw"]).reshape(HID, KK)
    dmlp = np.zeros((HT, 128, KK, 128), np.float32)
    idx = np.arange(128)
    for tt in range(HT):
        for ti in range(KK):
            dmlp[tt, idx, ti, idx] = mlp_dw[128 * tt:128 * (tt + 1), ti]
    dmlp = dmlp.reshape(HT, 128, KK * 128)

    com = dict(
        knb2t=f32(inputs["kn_b2"]).reshape(C, KK),
        gind6=gind6, gexpT=gexpT,
        ident=bf(np.eye(128, dtype=np.float32)),
        onesc=bf(np.full((128, 1), 1.0 / C, np.float32)),
        onesr=bf(np.ones((1, 128), np.float32)),
        gemvW=bf(gemv_stack),
        modWT=bf(f32(inputs["mod_W"]).T),
        wiT=bf(f32(inputs["mlp_Wi"]).T.reshape(CT, 128, HT, 128)
               .transpose(2, 1, 0, 3).reshape(HT, 128, CT * 128)),
        woT=bf(f32(inputs["mlp_Wo"]).T.reshape(HT // 2, 128, CT, 128)
               .transpose(2, 1, 0, 3).reshape(CT, 128, (HT // 2) * 128)),
        dmlp=bf(dmlp), dwk=mlp_dw,
    )

    in_maps = []
    for i in range(NCORES):
        z0 = ZP * i
        xh = np.zeros((C, Z7, 26, 26), np.float32)
        lo, hi = max(z0 - 2, 0), min(z0 + ZP + 2, S)
        xh[:, lo - (z0 - 2):lo - (z0 - 2) + (hi - lo), 1:25, 1:25] = \
            x[:, lo:hi]
        smalls = smalls_c.copy()
        smalls[:, S_MLO] = 0.0 if i == 0 else 1.0
        smalls[:, S_MHI] = 0.0 if i == NCORES - 1 else 1.0
        m = dict(com)
        m.update(
            xpad=xh.reshape(C, XPN).astype(BF),
            smalls=smalls,
            knb1=f32(inputs["kn_b1"][W1R * i:W1R * (i + 1)]),
            w1sT=bf(kn_W1[W1R * i:W1R * (i + 1), :].T),
            w2sT=bf(w2T[W1R * i:W1R * (i + 1), :]),
        )
        in_maps.append(m)
    return in_maps


def kernel(**inputs) -> np.ndarray:
    if "nc" not in _CACHE:
        _CACHE["nc"] = build_program()
    nc = _CACHE["nc"]
    in_maps = _prep(inputs)
    res = run_bass_kernel_spmd(nc, in_maps, list(range(NCORES)))
    outs = [res.results[i]["out"].reshape(C, ZP, PL) for i in range(NCORES)]
    full = np.concatenate(outs, axis=1)
    return full.reshape(1, C, S, S, S).astype(np.float32)


# revision 47
# speedup vs baseline: 2.1157x; 1.1291x over previous
"""Bass/Trainium2 SPMD kernel for nn_Block3D (8 NeuronCores).

Spatial z-shard (24 planes -> 3 per core) with a 2-plane host-prepped halo:
each core computes cafm/xb/xln on 5 planes locally, so the depthwise MLP conv
needs no on-device halo exchange (no AllGather). The per-channel `mod` gate is
folded into the dynamic conv kernels. Depthwise 3x3x3 convs are split between
the PE (diagonal-weight matmuls) and DVE (tensor_scalar 4x + tensor_tensor 2x
chains). All DMAs ride the HWDGE path (nc.sync); kernel_net output is staged
as one [128,162] tile around a single AllReduce.
"""

from contextlib import ExitStack

import numpy as np
import ml_dtypes

import concourse.bass as bass
import concourse.bacc as bacc
import concourse.tile as tile
from concourse import mybir
from concourse.bass_utils import run_bass_kernel_spmd

BF = ml_dtypes.bfloat16
F32 = mybir.dt.float32
BF16 = mybir.dt.bfloat16

C = 768
G = 12
GD = 64
S = 24
HID = 4 * C
KK = 27
EPS = 1e-5
NCORES = 8
ZP = S // NCORES          # 3 own planes
PL = S * S                # 576
Z7 = ZP + 4               # 7 input planes (2-halo each side)
Z5 = ZP + 2               # 5 computed planes (1-halo each side)
PPL = 26 * 26             # 676 padded plane
XPN = Z7 * PPL            # 4732
HPN = Z5 * PPL            # 3380
V5 = Z5 * PL              # 2880
VC = ZP * PL              # 1728
CT = C // 128             # 6
HT = HID // 128           # 24
W1R = HID // NCORES       # 384 hidden rows per core
KFLAT = C * KK
NTOT = float(S * S * S)

TAPS = [(dz, dy, dx) for dz in (-1, 0, 1) for dy in (-1, 0, 1) for dx in (-1, 0, 1)]

# engine split for the depthwise convs (PE diag-matmul vs DVE chains)
DYN_PE = (0, 1, 2, 3)
MLP_PE = tuple([j for j in range(0, 12, 2)] + [j + 12 for j in range(1, 12, 2)])

_CACHE = {}
DEBUG = False

Copy = mybir.ActivationFunctionType.Copy
Iden = mybir.ActivationFunctionType.Identity
Gelu = mybir.ActivationFunctionType.Gelu
Sigmoid = mybir.ActivationFunctionType.Sigmoid
Square = mybir.ActivationFunctionType.Square
Sqrt = mybir.ActivationFunctionType.Sqrt
Relu = mybir.ActivationFunctionType.Relu
ADD = mybir.AluOpType.add
SUB = mybir.AluOpType.subtract
MULT = mybir.AluOpType.mult

(S_Y, S_LB1, S_LB2, S_TPB, S_ABV, S_ABO, S_OPB, S_N2W, S_N2B, S_N3W, S_N3B,
 S_GNG, S_GNB, S_MODB, S_MLO, S_MHI) = range(16)


def build_program():
    nc = bacc.Bacc("TRN2", target_bir_lowering=False)

    def dram_in(name, shape, dtype=F32):
        return nc.declare_dram_parameter(name, list(shape), dtype, isOutput=False)

    xpad_in = dram_in("xpad", [C, XPN], BF16)
    xown_in = dram_in("xown", [C, VC], BF16)
    smalls = dram_in("smalls", [C, 16])
    knb1 = dram_in("knb1", [W1R])
    knb2t = dram_in("knb2t", [C, KK])
    ident = dram_in("ident", [128, 128], BF16)
    onesc = dram_in("onesc", [128, 1], BF16)
    onesr_in = dram_in("onesr", [1, 128], BF16)
    gind_in = dram_in("gind6", [CT, 128, G])
    gexpT = dram_in("gexpT", [G, C])
    gemvW = dram_in("gemvW", [6, C, C], BF16)     # lora1,lora2,tp,av,ao,op (all .T)
    modWT = dram_in("modWT", [2 * C, C], BF16)
    w1sT = dram_in("w1sT", [2 * C, W1R], BF16)
    w2sT = dram_in("w2sT", [W1R, KFLAT], BF16)
    wiT = dram_in("wiT", [HT, 128, CT * 128], BF16)
    woT = dram_in("woT", [CT, 128, (HT // 2) * 128], BF16)
    dmlp = dram_in("dmlp", [HT, 128, KK * 128], BF16)
    dwk_in = dram_in("dwk", [HID, KK])
    out = nc.declare_dram_parameter("out", [C, VC], F32, isOutput=True)
    if DEBUG:
        dbg_mod = nc.declare_dram_parameter("dbg_mod", [C, 1], F32, isOutput=True)
        dbg_kern = nc.declare_dram_parameter("dbg_kern", [C, KK], F32, isOutput=True)
        dbg_dyn = nc.declare_dram_parameter("dbg_dyn", [C, V5], F32, isOutput=True)
        dbg_xb = nc.declare_dram_parameter("dbg_xb", [C, V5], F32, isOutput=True)
        dbg_xln = nc.declare_dram_parameter("dbg_xln", [C, V5], F32, isOutput=True)
        dbg_gate = nc.declare_dram_parameter("dbg_gate", [128, VC], F32, isOutput=True)
        dbg_h = nc.declare_dram_parameter("dbg_h", [128, HPN], F32, isOutput=True)
        dbg_c1 = nc.declare_dram_parameter("dbg_c1", [128, VC], F32, isOutput=True)
        dbg_y = nc.declare_dram_parameter("dbg_y", [C, VC], F32, isOutput=True)

    with tile.TileContext(nc) as tc, ExitStack() as ctx:
        persist = ctx.enter_context(tc.tile_pool(name="persist", bufs=1))
        dram = ctx.enter_context(tc.tile_pool(name="dram", bufs=1, space="DRAM"))
        xbpool = ctx.enter_context(tc.tile_pool(name="xbpool", bufs=1))
        ytpool = ctx.enter_context(tc.tile_pool(name="ytpool", bufs=1))

        psA_cm = tc.tile_pool(name="psA", bufs=1, space="PSUM")
        psA = psA_cm.__enter__()
        gpool_cm = tc.tile_pool(name="gemv", bufs=2)
        gpool = gpool_cm.__enter__()
        xpool_cm = tc.tile_pool(name="xpool", bufs=1)
        xpool = xpool_cm.__enter__()

        id_t = persist.tile([128, 128], BF16, name="identt", tag="identt")
        nc.sync.dma_start(id_t[:], ident[:, :])
        ones_t = persist.tile([128, 1], BF16, name="onest", tag="onest")
        nc.sync.dma_start(ones_t[:], onesc[:, :])
        onesr_t = persist.tile([1, 128], BF16, name="onesrt", tag="onesrt")
        nc.sync.dma_start(onesr_t[:], onesr_in[:, :])
        eps_t = persist.tile([128, 1], F32, name="epst", tag="epst")
        nc.vector.memset(eps_t[:], EPS)
        junk = xpool.tile([128, VC], BF16, name="junk", tag="junk")
        junk5 = junk.rearrange("p (z y x) -> p z y x", z=ZP, y=24, x=24)
        opT = persist.tile([128, CT * C], BF16, name="opT", tag="opT")
        nc.sync.dma_start(
            opT[:], bass.AP(tensor=gemvW, offset=5 * C * C,
                            ap=[[C, 128], [128 * C, CT], [1, C]]))
        gi_all = persist.tile([128, CT * G], F32, name="giall", tag="giall")
        nc.sync.dma_start(
            gi_all[:], bass.AP(tensor=gind_in, offset=0,
                               ap=[[G, 128], [128 * G, CT], [1, G]]))
        knb1_t = persist.tile([128, 3], F32, name="knb1t", tag="knb1t")
        nc.sync.dma_start(
            knb1_t[:], bass.AP(tensor=knb1, offset=0, ap=[[1, 128], [128, 3]]))
        kb2 = [persist.tile([128, KK], F32, name=f"kb2{i}", tag=f"kb2{i}")
               for i in range(CT)]
        for i in range(CT):
            nc.sync.dma_start(kb2[i][:], knb2t[128 * i:128 * (i + 1), :])
        dwk_map = {}
        for t in range(HT):
            if t not in MLP_PE:
                d = persist.tile([128, KK], F32, name=f"dwk{t}", tag=f"dwk{t}")
                nc.sync.dma_start(d[:], dwk_in[128 * t:128 * (t + 1), :])
                dwk_map[t] = d

        gwpool_cm = tc.tile_pool(name="gwpool", bufs=2)
        gwpool = gwpool_cm.__enter__()
        w1pool_cm = tc.tile_pool(name="w1pool", bufs=1)
        w1pool = w1pool_cm.__enter__()

        # ------------ phase A: vc partial sums + AR1 (issue-first) ------------
        xvc = [w1pool.tile([128, VC], BF16, name=f"xvc{i}", tag=f"xvc{i}")
               for i in range(CT)]
        for i in range(CT):
            nc.sync.dma_start(xvc[i][:], xown_in[128 * i:128 * (i + 1), :])
        vcs = persist.tile([128, CT], F32, name="vcs", tag="vcs")
        for i in range(CT):
            nc.vector.tensor_reduce(vcs[:, i:i + 1], xvc[i][:],
                                    axis=mybir.AxisListType.X, op=ADD)
        ar1_in = dram.tile([C], F32, name="ar1i", tag="ar1i")
        ar1_out = dram.tile([C], F32, name="ar1o", tag="ar1o",
                            addr_space="Shared")
        nc.sync.dma_start(
            bass.AP(tensor=ar1_in[:].tensor, offset=ar1_in[:].offset,
                    ap=[[1, 128], [128, CT]]), vcs[:])
        nc.gpsimd.collective_compute(
            "AllReduce", ADD, replica_groups=[list(range(NCORES))],
            ins=[ar1_in[:]], outs=[ar1_out[:]])
        sm = [persist.tile([128, 16], F32, name=f"sm{i}", tag=f"sm{i}")
              for i in range(CT)]
        for i in range(CT):
            nc.sync.dma_start(sm[i][:], smalls[128 * i:128 * (i + 1), :])
        xp = [xpool.tile([128, XPN], BF16, name=f"xp{i}", tag=f"xp{i}")
              for i in range(CT)]
        for i in range(CT):
            nc.sync.dma_start(xp[i][:], xpad_in[128 * i:128 * (i + 1), :])
        x4 = [t.rearrange("p (z y x) -> p z y x", z=Z7, y=26, x=26) for t in xp]


        def load_gwbuf(dram_t, off):
            t = gwpool.tile([128, CT * C], BF16, name="gwbuf", tag="gwbuf")
            nc.scalar.dma_start(
                t[:], bass.AP(tensor=dram_t, offset=off,
                              ap=[[C, 128], [128 * C, CT], [1, C]]))
            return t

        w1t = [w1pool.tile([128, CT * W1R], BF16, name=f"w1t{h}",
                           tag=f"w1t{h}") for h in range(2)]
        for h in range(2):
            nc.scalar.dma_start(
                w1t[h][:], bass.AP(tensor=w1sT, offset=h * C * W1R,
                                   ap=[[W1R, 128], [128 * W1R, CT], [1, W1R]]))


        # ------------ phase B: text gemv chain (overlaps AR1) ------------
        def gemv(wt, in_cols, nm, act, bias_cols, tag, scale=1.0, odt=BF16):
            outs = []
            for m in range(nm):
                ps = psA.tile([128, 1], F32, name="ps_small", tag="ps_small")
                for k in range(CT):
                    nc.tensor.matmul(
                        ps[:], wt[:, C * k + 128 * m:C * k + 128 * m + 128],
                        in_cols[k][:], start=(k == 0), stop=(k == CT - 1))
                o = gpool.tile([128, 1], odt, name=f"{tag}o{m}",
                               tag=f"{tag}o{m}")
                bias = bias_cols[m] if bias_cols is not None else 0.0
                nc.scalar.activation(o[:], ps[:], act, bias=bias, scale=scale)
                outs.append(o)
            return outs

        t_cols = []
        for i in range(CT):
            t = gpool.tile([128, 1], BF16, name=f"tc{i}", tag=f"tc{i}")
            nc.scalar.activation(t[:], sm[i][:, S_Y:S_Y + 1], Copy)
            t_cols.append(t)
        h1 = gemv(load_gwbuf(gemvW, 0 * C * C), t_cols, CT, Relu,
                  [sm[i][:, S_LB1:S_LB1 + 1] for i in range(CT)], "lw1")
        h2 = gemv(load_gwbuf(gemvW, 1 * C * C), h1, CT, Iden,
                  [sm[i][:, S_LB2:S_LB2 + 1] for i in range(CT)], "lw2")
        tp = gemv(load_gwbuf(gemvW, 2 * C * C), h2, CT, Iden,
                  [sm[i][:, S_TPB:S_TPB + 1] for i in range(CT)], "tpw")
        av = gemv(load_gwbuf(gemvW, 3 * C * C), tp, CT, Iden,
                  [sm[i][:, S_ABV:S_ABV + 1] for i in range(CT)], "avw")
        attn = gemv(load_gwbuf(gemvW, 4 * C * C), av, CT, Iden,
                    [sm[i][:, S_ABO:S_ABO + 1] for i in range(CT)], "aow")
        mw = [load_gwbuf(modWT, h * C * C) for h in range(2)]

        # ------------ phase C: post-AR1 gemvs ------------
        vc_cols = []
        for i in range(CT):
            col = gpool.tile([128, 1], F32, name=f"vcc{i}", tag=f"vcc{i}")
            nc.sync.dma_start(
                col[:], bass.AP(tensor=ar1_out[:].tensor,
                                offset=ar1_out[:].offset + 128 * i,
                                ap=[[1, 128], [128, 1]]))
            cb = gpool.tile([128, 1], BF16, name=f"cmb{i}", tag=f"cmb{i}")
            nc.scalar.activation(cb[:], col[:], Copy, scale=1.0 / NTOT)
            vc_cols.append(cb)
        halves = [vc_cols, attn]

        def gemv2h(wts, stride, nm, act, bias_cols, tag, odt=F32):
            outs = []
            for m in range(nm):
                ps = psA.tile([128, 1], F32, name="ps_small", tag="ps_small")
                for h in range(2):
                    for k in range(CT):
                        nc.tensor.matmul(
                            ps[:], wts[h][:, k * stride + 128 * m:
                                          k * stride + 128 * m + 128],
                            halves[h][k][:],
                            start=(h == 0 and k == 0),
                            stop=(h == 1 and k == CT - 1))
                o = gpool.tile([128, 1], odt, name=f"{tag}o{m}",
                               tag=f"{tag}o{m}")
                nc.scalar.activation(o[:], ps[:], act, bias=bias_cols[m])
                outs.append(o)
            return outs

        mod = gemv2h(mw, C, CT, Sigmoid,
                     [sm[i][:, S_MODB:S_MODB + 1] for i in range(CT)], "modw")
        kp1 = gemv2h(w1t, W1R, 3, Relu,
                     [knb1_t[:, m:m + 1] for m in range(3)], "w1s", odt=BF16)

        w1pool_cm.__exit__(None, None, None)
        gwpool_cm.__exit__(None, None, None)

        # ------------ phase D: kernel_net gemv2 + AR2 ------------
        ko_all = persist.tile([128, KK * CT], F32, name="koall", tag="koall")
        with (tc.tile_pool(name="kseq", bufs=13) as kseq,
              tc.tile_pool(name="kps", bufs=2, space="PSUM") as kps):
            for t in range(KK):
                wt = kseq.tile([128, 3 * C], BF16, name="w2t", tag="w2t")
                nc.gpsimd.dma_start(
                    wt[:], bass.AP(tensor=w2sT, offset=t * C,
                                   ap=[[KFLAT, 128], [128 * KFLAT, 3],
                                       [1, C]]))
                ps = kps.tile([128, CT], F32, name="g2ps", tag="g2ps")
                for m in range(CT):
                    for k in range(3):
                        nc.tensor.matmul(
                            ps[:, m:m + 1],
                            wt[:, k * C + 128 * m:k * C + 128 * m + 128],
                            kp1[k][:], start=(m == 0 and k == 0),
                            stop=(m == CT - 1 and k == 2))
                nc.scalar.activation(ko_all[:, CT * t:CT * (t + 1)], ps[:],
                                     Copy)
        ar2_in = dram.tile([128, KK * CT], F32, name="ar2i", tag="ar2i")
        ar2_out = dram.tile([128, KK * CT], F32, name="ar2o", tag="ar2o",
                            addr_space="Shared")
        nc.sync.dma_start(ar2_in[:, :], ko_all[:])
        nc.gpsimd.collective_compute(
            "AllReduce", ADD, replica_groups=[list(range(NCORES))],
            ins=[ar2_in[:]], outs=[ar2_out[:]])
        kraw = persist.tile([128, KK * CT], F32, name="kraw", tag="kraw")
        nc.sync.dma_start(kraw[:], ar2_out[:, :])
        kraw3 = kraw.rearrange("p (t i) -> p t i", t=KK, i=CT)


        # kernels: +bias, fold mod
        kernm = []
        for i in range(CT):
            km = persist.tile([128, KK], F32, name=f"kernm{i}", tag=f"kernm{i}")
            nc.vector.tensor_tensor(km[:], kraw3[:, :, i], kb2[i][:], op=ADD)
            nc.vector.tensor_scalar_mul(km[:], km[:], mod[i][:, 0:1])
            kernm.append(km)

        if DEBUG:
            for i in range(CT):
                nc.gpsimd.dma_start(dbg_mod[128 * i:128 * (i + 1), :], mod[i][:])
                nc.gpsimd.dma_start(dbg_kern[128 * i:128 * (i + 1), :], kernm[i][:])

        # ------------ phase E: dynamic depthwise conv + GN stats ------------
        dynpool_cm = tc.tile_pool(name="dynpool", bufs=1)
        dynpool = dynpool_cm.__enter__()
        dyn = [dynpool.tile([128, V5], BF16, name=f"dyn{i}", tag=f"dyn{i}")
               for i in range(CT)]
        dyn4 = [t.rearrange("p (z y x) -> p z y x", z=Z5, y=24, x=24)
                for t in dyn]

        dgpool_cm = tc.tile_pool(name="dgpool", bufs=2)
        dgpool = dgpool_cm.__enter__()

        def build_diag(i):
            d = dgpool.tile([128, KK * 128], BF16, name="dg", tag="dg")
            for t in range(KK):
                nc.vector.tensor_scalar_mul(d[:, 128 * t:128 * (t + 1)],
                                            id_t[:], kernm[i][:, t:t + 1])
            return d

        def dyn_pe_planes(i, dgt, zos, cpool):
            for zo in zos:
                for half in range(2):
                    y0 = 12 * half
                    cp = cpool.tile([128, 288], F32, name=f"dcp{half}",
                                    tag=f"dcp{half}")
                    for ti, (dz, dy, dx) in enumerate(TAPS):
                        nc.tensor.matmul(
                            cp[:], dgt[:, 128 * ti:128 * (ti + 1)],
                            x4[i][:, zo + 1 + dz, 1 + y0 + dy:13 + y0 + dy,
                                  1 + dx:25 + dx],
                            start=(ti == 0), stop=(ti == KK - 1))
                    nc.scalar.activation(
                        dyn4[i][:, zo, y0:y0 + 12, :], cp[:], Copy)

        def dyn_dve_planes(i, zo0, nz, tpool):
            dst = dyn4[i][:, zo0:zo0 + nz, :, :]
            for ti, (dz, dy, dx) in enumerate(TAPS):
                src = x4[i][:, zo0 + 1 + dz:zo0 + 1 + dz + nz,
                            1 + dy:25 + dy, 1 + dx:25 + dx]
                if ti == 0:
                    nc.vector.tensor_scalar_mul(dst, src, kernm[i][:, 0:1])
                else:
                    tmp = tpool.tile([128, 3 * PL], BF16, name="dtmp",
                                     tag="dtmp")
                    t4 = tmp.rearrange("p (z y x) -> p z y x", z=3, y=24,
                                       x=24)[:, 0:nz, :, :]
                    nc.vector.tensor_scalar_mul(t4, src,
                                                kernm[i][:, ti:ti + 1])
                    nc.vector.tensor_tensor(dst, dst, t4, op=ADD)

        gst = persist.tile([128, 2 * CT], F32, name="gst", tag="gst")
        ar3_in = dram.tile([G, 2], F32, name="ar3i", tag="ar3i")
        ar3_out = dram.tile([G, 2], F32, name="ar3o", tag="ar3o",
                            addr_space="Shared")
        gsb = persist.tile([G, 2], F32, name="gsb", tag="gsb")
        with (tc.tile_pool(name="dcpool", bufs=1, space="PSUM") as dcpool,
              tc.tile_pool(name="dtpool", bufs=2) as dtpool,
              tc.tile_pool(name="gnps", bufs=1, space="PSUM") as gnps):
            # own planes (zo 1..3) first, stats, then halo planes under AR3
            dg_live = {}
            for i in range(CT):
                if i in DYN_PE:
                    dg_live[i] = build_diag(i)
                    dyn_pe_planes(i, dg_live[i], (1, 2, 3), dcpool)
                else:
                    dyn_dve_planes(i, 1, 3, dtpool)
                nc.scalar.activation(junk[:], dyn[i][:, PL:4 * PL],
                                     Copy, accum_out=gst[:, 2 * i:2 * i + 1])
                nc.scalar.activation(junk[:], dyn[i][:, PL:4 * PL],
                                     Square,
                                     accum_out=gst[:, 2 * i + 1:2 * i + 2])
            gps = gnps.tile([G, 2], F32, name="gps", tag="gps")
            for i in range(CT):
                nc.tensor.matmul(gps[:], gi_all[:, G * i:G * (i + 1)],
                                 gst[:, 2 * i:2 * i + 2], start=(i == 0),
                                 stop=(i == CT - 1))
            nc.scalar.activation(gsb[:], gps[:], Copy)
            nc.sync.dma_start(ar3_in[:, :], gsb[:])
            nc.gpsimd.collective_compute(
                "AllReduce", ADD, replica_groups=[list(range(NCORES))],
                ins=[ar3_in[:]], outs=[ar3_out[:]])
            for i in range(CT):
                if i in DYN_PE:
                    dg2 = build_diag(i)
                    dyn_pe_planes(i, dg2, (0, 4), dcpool)
                else:
                    dyn_dve_planes(i, 0, 1, dtpool)
                    dyn_dve_planes(i, 4, 1, dtpool)
        dgpool_cm.__exit__(None, None, None)

        if DEBUG:
            for i in range(CT):
                nc.gpsimd.dma_start(dbg_dyn[128 * i:128 * (i + 1), :], dyn[i][:])

        # ------------ GN scale/shift + fold into opT ------------
        gstat = persist.tile([G, 2], F32, name="gstat", tag="gstat")
        nc.sync.dma_start(gstat[:], ar3_out[:, :])
        NGRP = float(GD) * NTOT
        gmr = persist.tile([G, 2], F32, name="gmr", tag="gmr")
        nc.scalar.activation(gmr[:, 0:1], gstat[:, 0:1], Copy, scale=1.0 / NGRP)
        musq = persist.tile([G, 1], F32, name="musq", tag="musq")
        nc.scalar.square(musq[:], gmr[:, 0:1])
        var = persist.tile([G, 1], F32, name="gvar", tag="gvar")
        nc.vector.tensor_scalar(var[:], gstat[:, 1:2], 1.0 / NGRP, None,
                                op0=MULT)
        nc.vector.tensor_sub(var[:], var[:], musq[:])
        nc.scalar.activation(var[:], var[:], Sqrt, bias=eps_t[0:G, 0:1])
        nc.vector.reciprocal(gmr[:, 1:2], var[:])

        cafm_shift = []
        gsc = []
        for i in range(CT):
            ge = gpool.tile([G, 128], F32, name=f"gexp{i}", tag=f"gexp{i}")
            nc.sync.dma_start(ge[:], gexpT[:, 128 * i:128 * (i + 1)])
            ps = psA.tile([128, 2], F32, name="ps_sm2", tag="ps_sm2")
            nc.tensor.matmul(ps[:], ge[:], gmr[:], start=True, stop=True)
            mu_c = persist.tile([128, 2], F32, name=f"muc{i}", tag=f"muc{i}")
            nc.scalar.activation(mu_c[:], ps[:], Copy)
            a = persist.tile([128, 1], F32, name=f"gsc{i}", tag=f"gsc{i}")
            nc.vector.tensor_mul(a[:], sm[i][:, S_GNG:S_GNG + 1], mu_c[:, 1:2])
            b = persist.tile([128, 1], F32, name=f"gsh{i}", tag=f"gsh{i}")
            nc.vector.tensor_mul(b[:], mu_c[:, 0:1], a[:])
            nc.vector.tensor_sub(b[:], sm[i][:, S_GNB:S_GNB + 1], b[:])
            gsc.append(a)
            bb = gpool.tile([128, 1], BF16, name=f"gshb{i}", tag=f"gshb{i}")
            nc.scalar.activation(bb[:], b[:], Copy)
            cafm_shift.append(bb)
        cb_cols = []
        for m in range(CT):
            ps = psA.tile([128, 1], F32, name="ps_small", tag="ps_small")
            for k in range(CT):
                nc.tensor.matmul(ps[:], opT[:, 768 * k + 128 * m:
                                            768 * k + 128 * m + 128],
                                 cafm_shift[k][:], start=(k == 0),
                                 stop=(k == CT - 1))
            o = persist.tile([128, 1], F32, name=f"cbc{m}", tag=f"cbc{m}")
            nc.scalar.activation(o[:], ps[:], Iden,
                                 bias=sm[m][:, S_OPB:S_OPB + 1])
            cb_cols.append(o)
        for k in range(CT):
            nc.vector.tensor_scalar_mul(opT[:, 768 * k:768 * (k + 1)],
                                        opT[:, 768 * k:768 * (k + 1)],
                                        gsc[k][:])

        # ------------ phase F1: cafm matmul + xb ------------
        xb = [xbpool.tile([128, V5], BF16, name=f"xb{m}", tag=f"xb{m}")
              for m in range(CT)]
        xb4 = [t.rearrange("p (z y x) -> p z y x", z=Z5, y=24, x=24)
               for t in xb]
        CH6 = [(o, min(512, V5 - o)) for o in range(0, V5, 512)]
        with (tc.tile_pool(name="f1ps", bufs=2, space="PSUM") as f1ps,
              tc.tile_pool(name="f1t", bufs=2) as f1t):
            for m in range(CT):
                tca = f1t.tile([128, V5], BF16, name="tcafm", tag="tcafm")
                for o, n in CH6:
                    ps = f1ps.tile([128, 512], F32, name="f1p", tag="f1p")
                    for k in range(CT):
                        nc.tensor.matmul(ps[:, 0:n],
                                         opT[:, 768 * k + 128 * m:
                                             768 * k + 128 * m + 128],
                                         dyn[k][:, o:o + n], start=(k == 0),
                                         stop=(k == CT - 1))
                    nc.scalar.activation(tca[:, o:o + n], ps[:, 0:n], Iden,
                                         bias=cb_cols[m][:])
                t4 = tca.rearrange("p (z y x) -> p z y x", z=Z5, y=24, x=24)
                nc.vector.tensor_tensor(xb4[m][:, :, :, :], t4[:, :, :, :],
                                        x4[m][:, 1:6, 1:25, 1:25], op=MULT)
        if DEBUG:
            for m in range(CT):
                nc.gpsimd.dma_start(dbg_xb[128 * m:128 * (m + 1), :], xb[m][:])
        dynpool_cm.__exit__(None, None, None)
        xpool_cm.__exit__(None, None, None)
        gpool_cm.__exit__(None, None, None)
        psA_cm.__exit__(None, None, None)

        # ------------ LN helper ------------
        def ln_stats(tiles, nv, tag, mupool):
            """Per-voxel mean/rstd over channels -> [128, nv] bf16 tiles.
            ones_t carries 1/C so the matmuls produce mean / E[x^2]."""
            nch = [(o, min(480, nv - o)) for o in range(0, nv, 480)]
            muB = mupool.tile([128, nv], BF16, name=f"{tag}muB",
                              tag=f"{tag}muB")
            rsB = mupool.tile([128, nv], BF16, name=f"{tag}rsB",
                              tag=f"{tag}rsB")
            with (tc.tile_pool(name=f"{tag}ps", bufs=1, space="PSUM") as lps,
                  tc.tile_pool(name=f"{tag}sq", bufs=2) as sqp,
                  tc.tile_pool(name=f"{tag}rw", bufs=2) as rwp):
                for o, n in nch:
                    p1 = lps.tile([1, 480], F32, name="s1", tag="s1")
                    p2 = lps.tile([1, 480], F32, name="s2", tag="s2")
                    for k in range(CT):
                        nc.tensor.matmul(p1[:, 0:n], ones_t[:],
                                         tiles[k][:, o:o + n],
                                         start=(k == 0), stop=(k == CT - 1))
                    for k in range(CT):
                        q = sqp.tile([128, 480], BF16, name="sqc", tag="sqc")
                        if k % 2 == 0:
                            nc.scalar.activation(q[:, 0:n],
                                                 tiles[k][:, o:o + n], Square)
                        else:
                            nc.vector.tensor_mul(q[:, 0:n],
                                                 tiles[k][:, o:o + n],
                                                 tiles[k][:, o:o + n])
                        nc.tensor.matmul(p2[:, 0:n], ones_t[:], q[:, 0:n],
                                         start=(k == 0), stop=(k == CT - 1))
                    mubf = rwp.tile([1, 480], BF16, name="mubf", tag="mubf")
                    nc.scalar.activation(mubf[:, 0:n], p1[:, 0:n], Copy)
                    m2 = rwp.tile([1, 480], F32, name="m2", tag="m2")
                    nc.scalar.activation(m2[:, 0:n], p1[:, 0:n], Square)
                    vr = rwp.tile([1, 480], F32, name="vr", tag="vr")
                    nc.vector.tensor_sub(vr[:, 0:n], p2[:, 0:n], m2[:, 0:n])
                    nc.scalar.activation(vr[:, 0:n], vr[:, 0:n], Sqrt,
                                         bias=eps_t[0:1, 0:1])
                    rbf = rwp.tile([1, 480], BF16, name="rbf", tag="rbf")
                    with nc.allow_low_precision(reason="rstd bcast in bf16"):
                        nc.vector.reciprocal(rbf[:, 0:n], vr[:, 0:n])
                    pb = lps.tile([128, 480], F32, name="bc", tag="bc")
                    nc.tensor.matmul(pb[:, 0:n], onesr_t[:], mubf[:, 0:n],
                                     start=True, stop=True)
                    nc.scalar.activation(muB[:, o:o + n], pb[:, 0:n], Copy)
                    pb2 = lps.tile([128, 480], F32, name="bc2", tag="bc2")
                    nc.tensor.matmul(pb2[:, 0:n], onesr_t[:], rbf[:, 0:n],
                                     start=True, stop=True)
                    nc.scalar.activation(rsB[:, o:o + n], pb2[:, 0:n], Copy)
            return muB, rsB

        # ------------ LN2 + edge masks ------------
        gatepool_cm = tc.tile_pool(name="gatepool", bufs=1)
        gatepool = gatepool_cm.__enter__()
        xlnpool_cm = tc.tile_pool(name="xlnpool", bufs=1)
        xlnpool = xlnpool_cm.__enter__()
        xln = [xlnpool.tile([128, V5], BF16, name=f"xln{k}", tag=f"xln{k}")
               for k in range(CT)]
        mupool_cm = tc.tile_pool(name="mupool", bufs=1)
        mupool = mupool_cm.__enter__()
        muB, rsB = ln_stats(xb, V5, "ln2", mupool)
        with tc.tile_pool(name="lnap", bufs=2) as lnap:
            for k in range(CT):
                t1 = lnap.tile([128, V5], BF16, name="lnt1", tag="lnt1")
                nc.vector.tensor_sub(t1[:], xb[k][:], muB[:])
                nc.vector.tensor_mul(t1[:], t1[:], rsB[:])
                nc.vector.tensor_scalar(xln[k][:], t1[:],
                                        sm[k][:, S_N2W:S_N2W + 1],
                                        sm[k][:, S_N2B:S_N2B + 1],
                                        op0=MULT, op1=ADD)
                nc.vector.tensor_scalar_mul(xln[k][:, 0:PL], xln[k][:, 0:PL],
                                            sm[k][:, S_MLO:S_MLO + 1])
                nc.vector.tensor_scalar_mul(xln[k][:, 4 * PL:5 * PL],
                                            xln[k][:, 4 * PL:5 * PL],
                                            sm[k][:, S_MHI:S_MHI + 1])

        # ------------ MLP: Wi + depthwise conv + gate ------------
        if DEBUG:
            for k in range(CT):
                nc.gpsimd.dma_start(dbg_xln[128 * k:128 * (k + 1), :], xln[k][:])
        mupool_cm.__exit__(None, None, None)
        gate = [gatepool.tile([128, VC], BF16, name=f"gate{j}", tag=f"gate{j}")
                for j in range(HT // 2)]
        hppool_cm = tc.tile_pool(name="hppool", bufs=1)
        hppool = hppool_cm.__enter__()
        hpads = [hppool.tile([128, HPN], BF16, name=f"hpad{b}", tag=f"hpad{b}")
                 for b in range(4)]
        for b in range(4):
            nc.vector.memset(hpads[b][:], 0.0)
        hp4s = [t.rearrange("p (z y x) -> p z y x", z=Z5, y=26, x=26)
                for t in hpads]

        with (tc.tile_pool(name="wiw", bufs=2) as wiw,
              tc.tile_pool(name="dga", bufs=2) as dgap,
              tc.tile_pool(name="wips", bufs=2, space="PSUM") as wips,
              tc.tile_pool(name="cvps", bufs=1, space="PSUM") as cvps,
              tc.tile_pool(name="mlpt", bufs=1) as mlpt):

            def wi_pass(t, bi):
                hp4 = hp4s[bi]
                wall = wiw.tile([128, CT * 128], BF16, name="wiall",
                                tag="wiall")
                nc.gpsimd.dma_start(wall[:], wiT[t, :, :])
                for z in range(Z5):
                    for halfy in range(2):
                        y0 = 12 * halfy
                        pz = wips.tile([128, 288], F32, name=f"wip{halfy}",
                                       tag=f"wip{halfy}")
                        for k in range(CT):
                            nc.tensor.matmul(
                                pz[:],
                                wall[:, 128 * k:128 * (k + 1)],
                                xln[k][:, z * PL + y0 * 24:
                                       z * PL + y0 * 24 + 288],
                                start=(k == 0), stop=(k == CT - 1))
                        nc.scalar.activation(
                            hp4[:, z, 1 + y0:13 + y0, 1:25],
                            pz.rearrange("p (y x) -> p y x", y=12, x=24)[
                                :, :, :], Copy)
                if DEBUG and t == 0:
                    nc.gpsimd.dma_start(dbg_h[:, :], hpads[bi][:])

            PTAPS = 4

            def conv_pass(t, bi, dst, act):
                hp4 = hp4s[bi]
                if t in MLP_PE:
                    if dst is None:
                        dst = mlpt.tile([128, VC], BF16, name="conv2",
                                        tag="conv2")
                    dgt = dgap.tile([128, KK * 128], BF16, name="dgall",
                                    tag="dgall")
                    nc.gpsimd.dma_start(dgt[:], dmlp[t, :, :])
                    for zo in range(3):
                        for halfy in range(2):
                            b = 2 * zo + halfy
                            y0 = 12 * halfy
                            cp = cvps.tile([128, 288], F32,
                                           name=f"mcp{b % 2}",
                                           tag=f"mcp{b % 2}")
                            for ti, (dz, dy, dx) in enumerate(TAPS):
                                nc.tensor.matmul(
                                    cp[:], dgt[:, 128 * ti:128 * (ti + 1)],
                                    hp4[:, 1 + zo + dz,
                                        1 + y0 + dy:13 + y0 + dy,
                                        1 + dx:25 + dx],
                                    start=(ti == 0), stop=(ti == KK - 1))
                            nc.scalar.activation(
                                dst[:, 288 * b:288 * (b + 1)], cp[:], act)
                    return dst
                # PE computes the first PTAPS taps into a bf16 partial
                dgp = dgap.tile([128, PTAPS * 128], BF16, name="dgpart",
                                tag="dgpart")
                nc.gpsimd.dma_start(
                    dgp[:], bass.AP(tensor=dmlp, offset=t * 128 * KK * 128,
                                    ap=[[KK * 128, 128], [1, PTAPS * 128]]))
                pcv = mlpt.tile([128, VC], BF16, name="pconv", tag="pconv")
                for zo in range(3):
                    for halfy in range(2):
                        b = 2 * zo + halfy
                        y0 = 12 * halfy
                        cp = cvps.tile([128, 288], F32, name=f"mcp{b % 2}",
                                       tag=f"mcp{b % 2}")
                        for ti in range(PTAPS):
                            dz, dy, dx = TAPS[ti]
                            nc.tensor.matmul(
                                cp[:], dgp[:, 128 * ti:128 * (ti + 1)],
                                hp4[:, 1 + zo + dz, 1 + y0 + dy:13 + y0 + dy,
                                    1 + dx:25 + dx],
                                start=(ti == 0), stop=(ti == PTAPS - 1))
                        nc.scalar.activation(pcv[:, 288 * b:288 * (b + 1)],
                                             cp[:], Copy)
                acc = mlpt.tile([128, VC], BF16, name="macc", tag="macc")
                a4 = acc.rearrange("p (z y x) -> p z y x", z=3, y=24, x=24)
                kw = dwk_map[t]
                for ti in range(PTAPS, KK):
                    dz, dy, dx = TAPS[ti]
                    src = hp4[:, 1 + dz:4 + dz, 1 + dy:25 + dy, 1 + dx:25 + dx]
                    if ti == PTAPS:
                        nc.vector.tensor_scalar_mul(a4[:, :, :, :], src,
                                                    kw[:, ti:ti + 1])
                    else:
                        tmp = mlpt.tile([128, VC], BF16, name="mtmp",
                                        tag="mtmp")
                        t4 = tmp.rearrange("p (z y x) -> p z y x", z=3, y=24,
                                           x=24)
                        nc.vector.tensor_scalar_mul(t4[:, :, :, :], src,
                                                    kw[:, ti:ti + 1])
                        nc.vector.tensor_tensor(a4[:, :, :, :], a4[:, :, :, :],
                                                t4[:, :, :, :], op=ADD)
                nc.vector.tensor_tensor(acc[:], acc[:], pcv[:], op=ADD)
                if act is Gelu:
                    nc.scalar.activation(dst[:], acc[:], Gelu)
                    return dst
                return acc

            for j in range(HT // 2):
                b1, b2 = (2 * j) % 4, (2 * j + 1) % 4
                wi_pass(j, b1)
                wi_pass(j + HT // 2, b2)
                conv_pass(j, b1, gate[j], Gelu)
                if DEBUG and j == 0:
                    nc.gpsimd.dma_start(dbg_c1[:, :], gate[0][:])
                c2 = conv_pass(j + HT // 2, b2, None, Copy)
                nc.vector.tensor_mul(gate[j][:], gate[j][:], c2[:])
        if DEBUG:
            nc.gpsimd.dma_start(dbg_gate[:, :], gate[0][:])
        hppool_cm.__exit__(None, None, None)
        xlnpool_cm.__exit__(None, None, None)

        # ------------ Wo + residual + LN3 (chunk-major, pipelined) ------------
        y_t = [ytpool.tile([128, VC], BF16, name=f"y{m}", tag=f"y{m}")
               for m in range(CT)]
        CH3 = [(0, 480), (480, 480), (960, 480), (1440, 288)]
        with (tc.tile_pool(name="wow", bufs=1) as wow,
              tc.tile_pool(name="wops", bufs=2, space="PSUM") as wops,
              tc.tile_pool(name="ln3ps", bufs=1, space="PSUM") as l3ps,
              tc.tile_pool(name="ln3sq", bufs=2) as l3sq,
              tc.tile_pool(name="ln3rw", bufs=2) as l3rw,
              tc.tile_pool(name="mupool3", bufs=1) as mupool3,
              tc.tile_pool(name="lnap3", bufs=2) as lnap3):
            walls = [wow.tile([128, (HT // 2) * 128], BF16, name=f"wo{m}",
                              tag=f"wo{m}") for m in range(CT)]
            for m in range(CT):
                nc.gpsimd.dma_start(walls[m][:], woT[m, :, :])
            muB3 = mupool3.tile([128, VC], BF16, name="ln3muB", tag="ln3muB")
            rsB3 = mupool3.tile([128, VC], BF16, name="ln3rsB", tag="ln3rsB")
            for o, n in CH3:
                for m in range(CT):
                    ps = wops.tile([128, 512], F32, name="wo_ps", tag="wo_ps")
                    for k in range(HT // 2):
                        nc.tensor.matmul(ps[:, 0:n],
                                         walls[m][:, 128 * k:128 * (k + 1)],
                                         gate[k][:, o:o + n],
                                         start=(k == 0), stop=False)
                    nc.tensor.matmul(ps[:, 0:n], id_t[:],
                                     xb[m][:, PL + o:PL + o + n],
                                     start=False, stop=True)
                    nc.scalar.activation(y_t[m][:, o:o + n], ps[:, 0:n], Copy)
                # LN3 stats for this chunk
                p1 = l3ps.tile([1, 480], F32, name="l3s1", tag="l3s1")
                p2 = l3ps.tile([1, 480], F32, name="l3s2", tag="l3s2")
                for k in range(CT):
                    nc.tensor.matmul(p1[:, 0:n], ones_t[:], y_t[k][:, o:o + n],
                                     start=(k == 0), stop=(k == CT - 1))
                for k in range(CT):
                    q = l3sq.tile([128, 480], BF16, name="l3q", tag="l3q")
                    nc.vector.tensor_mul(q[:, 0:n], y_t[k][:, o:o + n],
                                         y_t[k][:, o:o + n])
                    nc.tensor.matmul(p2[:, 0:n], ones_t[:], q[:, 0:n],
                                     start=(k == 0), stop=(k == CT - 1))
                mubf = l3rw.tile([1, 480], BF16, name="l3mubf", tag="l3mubf")
                nc.scalar.activation(mubf[:, 0:n], p1[:, 0:n], Copy)
                m2 = l3rw.tile([1, 480], F32, name="l3m2", tag="l3m2")
                nc.scalar.activation(m2[:, 0:n], p1[:, 0:n], Square)
                vr = l3rw.tile([1, 480], F32, name="l3vr", tag="l3vr")
                nc.vector.tensor_sub(vr[:, 0:n], p2[:, 0:n], m2[:, 0:n])
                nc.scalar.activation(vr[:, 0:n], vr[:, 0:n], Sqrt,
                                     bias=eps_t[0:1, 0:1])
                rbf = l3rw.tile([1, 480], BF16, name="l3rbf", tag="l3rbf")
                with nc.allow_low_precision(reason="rstd bcast in bf16"):
                    nc.vector.reciprocal(rbf[:, 0:n], vr[:, 0:n])
                pb = l3ps.tile([128, 480], F32, name="l3bc", tag="l3bc")
                nc.tensor.matmul(pb[:, 0:n], onesr_t[:], mubf[:, 0:n],
                                 start=True, stop=True)
                nc.scalar.activation(muB3[:, o:o + n], pb[:, 0:n], Copy)
                pb2 = l3ps.tile([128, 480], F32, name="l3bc2", tag="l3bc2")
                nc.tensor.matmul(pb2[:, 0:n], onesr_t[:], rbf[:, 0:n],
                                 start=True, stop=True)
                nc.scalar.activation(rsB3[:, o:o + n], pb2[:, 0:n], Copy)
                # LN3 apply + store for this chunk
                for k in range(CT):
                    t1 = lnap3.tile([128, 480], BF16, name="ln3t", tag="ln3t")
                    nc.vector.tensor_sub(t1[:, 0:n], y_t[k][:, o:o + n],
                                         muB3[:, o:o + n])
                    nc.vector.tensor_mul(t1[:, 0:n], t1[:, 0:n],
                                         rsB3[:, o:o + n])
                    of = lnap3.tile([128, 480], F32, name="outf", tag="outf")
                    nc.vector.tensor_scalar(of[:, 0:n], t1[:, 0:n],
                                            sm[k][:, S_N3W:S_N3W + 1],
                                            sm[k][:, S_N3B:S_N3B + 1],
                                            op0=MULT, op1=ADD)
                    nc.sync.dma_start(out[128 * k:128 * (k + 1), o:o + n],
                                      of[:, 0:n])
        gatepool_cm.__exit__(None, None, None)

        if DEBUG:
            for m in range(CT):
                nc.gpsimd.dma_start(dbg_y[128 * m:128 * (m + 1), :], y_t[m][:])

    nc.compile()
    return nc


def _prep(inputs):
    bf = lambda a: np.ascontiguousarray(a).astype(BF)
    f32 = lambda a: np.ascontiguousarray(a, dtype=np.float32)
    x = f32(inputs["x"][0]).reshape(C, S, S, S)

    smalls_c = np.zeros((C, 16), np.float32)
    smalls_c[:, 0] = f32(inputs["y"][0, 0])
    for i, k in enumerate(["lora_b1", "lora_b2", "tp_b", "attn_bv", "attn_bo",
                           "op_b", "n2_w", "n2_b", "n3_w", "n3_b", "gn_g",
                           "gn_b", "mod_b"]):
        smalls_c[:, i + 1] = f32(inputs[k])

    gind6 = np.zeros((CT, 128, G), np.float32)
    for j in range(CT):
        for p in range(128):
            gind6[j, p, (128 * j + p) // GD] = 1.0
    gexpT = np.zeros((G, C), np.float32)
    for c in range(C):
        gexpT[c // GD, c] = 1.0

    gemv_stack = np.stack([
        f32(inputs["lora_W1"]).T, f32(inputs["lora_W2"]).T,
        f32(inputs["tp_W"]).T, f32(inputs["attn_Wv"]).T,
        f32(inputs["attn_Wo"]).T, f32(inputs["op_W"]).T])

    kn_W2 = f32(inputs["kn_W2"])
    w2r = kn_W2.reshape(C, KK, HID).transpose(1, 0, 2).reshape(KFLAT, HID)
    w2T = np.ascontiguousarray(w2r.T)          # [HID, KFLAT]
    kn_W1 = f32(inputs["kn_W1"])

    mlp_dw = f32(inputs["mlp_dw"]).reshape(HID, KK)
    dmlp = np.zeros((HT, 128, KK, 128), np.float32)
    idx = np.arange(128)
    for tt in range(HT):
        for ti in range(KK):
            dmlp[tt, idx, ti, idx] = mlp_dw[128 * tt:128 * (tt + 1), ti]
    dmlp = dmlp.reshape(HT, 128, KK * 128)

    com = dict(
        knb2t=f32(inputs["kn_b2"]).reshape(C, KK),
        gind6=gind6, gexpT=gexpT,
        ident=bf(np.eye(128, dtype=np.float32)),
        onesc=bf(np.full((128, 1), 1.0 / C, np.float32)),
        onesr=bf(np.ones((1, 128), np.float32)),
        gemvW=bf(gemv_stack),
        modWT=bf(f32(inputs["mod_W"]).T),
        wiT=bf(f32(inputs["mlp_Wi"]).T.reshape(CT, 128, HT, 128)
               .transpose(2, 1, 0, 3).reshape(HT, 128, CT * 128)),
        woT=bf(f32(inputs["mlp_Wo"]).T.reshape(HT // 2, 128, CT, 128)
               .transpose(2, 1, 0, 3).reshape(CT, 128, (HT // 2) * 128)),
        dmlp=bf(dmlp), dwk=mlp_dw,
    )

    in_maps = []
    for i in range(NCORES):
        z0 = ZP * i
        xh = np.zeros((C, Z7, 26, 26), np.float32)
        lo, hi = max(z0 - 2, 0), min(z0 + ZP + 2, S)
        xh[:, lo - (z0 - 2):lo - (z0 - 2) + (hi - lo), 1:25, 1:25] = \
            x[:, lo:hi]
        smalls = smalls_c.copy()
        smalls[:, S_MLO] = 0.0 if i == 0 else 1.0
        smalls[:, S_MHI] = 0.0 if i == NCORES - 1 else 1.0
        m = dict(com)
        m.update(
            xpad=xh.reshape(C, XPN).astype(BF),
            xown=np.ascontiguousarray(
                x[:, z0:z0 + ZP].reshape(C, VC)).astype(BF),
            smalls=smalls,
            knb1=f32(inputs["kn_b1"][W1R * i:W1R * (i + 1)]),
            w1sT=bf(kn_W1[W1R * i:W1R * (i + 1), :].T),
            w2sT=bf(w2T[W1R * i:W1R * (i + 1), :]),
        )
        in_maps.append(m)
    return in_maps


def kernel(**inputs) -> np.ndarray:
    if "nc" not in _CACHE:
        _CACHE["nc"] = build_program()
    nc = _CACHE["nc"]
    in_maps = _prep(inputs)
    res = run_bass_kernel_spmd(nc, in_maps, list(range(NCORES)))
    outs = [res.results[i]["out"].reshape(C, ZP, PL) for i in range(NCORES)]
    full = np.concatenate(outs, axis=1)
    return full.reshape(1, C, S, S, S).astype(np.float32)


# revision 50
# speedup vs baseline: 2.1228x; 1.0034x over previous
"""Bass/Trainium2 SPMD kernel for nn_Block3D (8 NeuronCores).

Spatial z-shard (24 planes -> 3 per core) with a 2-plane host-prepped halo:
each core computes cafm/xb/xln on 5 planes locally, so the depthwise MLP conv
needs no on-device halo exchange (no AllGather). The per-channel `mod` gate is
folded into the dynamic conv kernels. Depthwise 3x3x3 convs are split between
the PE (diagonal-weight matmuls) and DVE (tensor_scalar 4x + tensor_tensor 2x
chains). All DMAs ride the HWDGE path (nc.sync); kernel_net output is staged
as one [128,162] tile around a single AllReduce.
"""

from contextlib import ExitStack

import numpy as np
import ml_dtypes

import concourse.bass as bass
import concourse.bacc as bacc
import concourse.tile as tile
from concourse import mybir
from concourse.bass_utils import run_bass_kernel_spmd

BF = ml_dtypes.bfloat16
F32 = mybir.dt.float32
BF16 = mybir.dt.bfloat16

C = 768
G = 12
GD = 64
S = 24
HID = 4 * C
KK = 27
EPS = 1e-5
NCORES = 8
ZP = S // NCORES          # 3 own planes
PL = S * S                # 576
Z7 = ZP + 4               # 7 input planes (2-halo each side)
Z5 = ZP + 2               # 5 computed planes (1-halo each side)
PPL = 26 * 26             # 676 padded plane
XPN = Z7 * PPL            # 4732
HPN = Z5 * PPL            # 3380
V5 = Z5 * PL              # 2880
VC = ZP * PL              # 1728
CT = C // 128             # 6
HT = HID // 128           # 24
W1R = HID // NCORES       # 384 hidden rows per core
KFLAT = C * KK
NTOT = float(S * S * S)

TAPS = [(dz, dy, dx) for dz in (-1, 0, 1) for dy in (-1, 0, 1) for dx in (-1, 0, 1)]

# engine split for the depthwise convs (PE diag-matmul vs DVE chains)
DYN_PE = (0, 1, 2, 3)
MLP_PE = tuple([j for j in range(0, 12, 2)] + [j + 12 for j in range(1, 12, 2)])

_CACHE = {}
DEBUG = False

Copy = mybir.ActivationFunctionType.Copy
Iden = mybir.ActivationFunctionType.Identity
Gelu = mybir.ActivationFunctionType.Gelu
Sigmoid = mybir.ActivationFunctionType.Sigmoid
Square = mybir.ActivationFunctionType.Square
Sqrt = mybir.ActivationFunctionType.Sqrt
Relu = mybir.ActivationFunctionType.Relu
ADD = mybir.AluOpType.add
SUB = mybir.AluOpType.subtract
MULT = mybir.AluOpType.mult

(S_Y, S_LB1, S_LB2, S_TPB, S_ABV, S_ABO, S_OPB, S_N2W, S_N2B, S_N3W, S_N3B,
 S_GNG, S_GNB, S_MODB, S_MLO, S_MHI) = range(16)


def build_program():
    nc = bacc.Bacc("TRN2", target_bir_lowering=False)

    def dram_in(name, shape, dtype=F32):
        return nc.declare_dram_parameter(name, list(shape), dtype, isOutput=False)

    xpad_in = dram_in("xpad", [C, XPN], BF16)
    xown_in = dram_in("xown", [C, VC], BF16)
    smalls = dram_in("smalls", [C, 16])
    knb1 = dram_in("knb1", [W1R])
    knb2t = dram_in("knb2t", [C, KK])
    ident = dram_in("ident", [128, 128], BF16)
    onesc = dram_in("onesc", [128, 1], BF16)
    onesr_in = dram_in("onesr", [1, 128], BF16)
    gind_in = dram_in("gind6", [CT, 128, G])
    gexpT = dram_in("gexpT", [G, C])
    gemvW = dram_in("gemvW", [6, C, C], BF16)     # lora1,lora2,tp,av,ao,op (all .T)
    modWT = dram_in("modWT", [2 * C, C], BF16)
    w1sT = dram_in("w1sT", [2 * C, W1R], BF16)
    w2sT = dram_in("w2sT", [W1R, KFLAT], BF16)
    wiT = dram_in("wiT", [HT, 128, CT * 128], BF16)
    woT = dram_in("woT", [CT, 128, (HT // 2) * 128], BF16)
    dmlp = dram_in("dmlp", [HT, 128, KK * 128], BF16)
    dwk_in = dram_in("dwk", [HID, KK])
    out = nc.declare_dram_parameter("out", [C, VC], F32, isOutput=True)
    if DEBUG:
        dbg_mod = nc.declare_dram_parameter("dbg_mod", [C, 1], F32, isOutput=True)
        dbg_kern = nc.declare_dram_parameter("dbg_kern", [C, KK], F32, isOutput=True)
        dbg_dyn = nc.declare_dram_parameter("dbg_dyn", [C, V5], F32, isOutput=True)
        dbg_xb = nc.declare_dram_parameter("dbg_xb", [C, V5], F32, isOutput=True)
        dbg_xln = nc.declare_dram_parameter("dbg_xln", [C, V5], F32, isOutput=True)
        dbg_gate = nc.declare_dram_parameter("dbg_gate", [128, VC], F32, isOutput=True)
        dbg_h = nc.declare_dram_parameter("dbg_h", [128, HPN], F32, isOutput=True)
        dbg_c1 = nc.declare_dram_parameter("dbg_c1", [128, VC], F32, isOutput=True)
        dbg_y = nc.declare_dram_parameter("dbg_y", [C, VC], F32, isOutput=True)

    with tile.TileContext(nc) as tc, ExitStack() as ctx:
        persist = ctx.enter_context(tc.tile_pool(name="persist", bufs=1))
        dram = ctx.enter_context(tc.tile_pool(name="dram", bufs=1, space="DRAM"))
        xbpool = ctx.enter_context(tc.tile_pool(name="xbpool", bufs=1))
        ytpool = ctx.enter_context(tc.tile_pool(name="ytpool", bufs=1))

        psA_cm = tc.tile_pool(name="psA", bufs=1, space="PSUM")
        psA = psA_cm.__enter__()
        gpool_cm = tc.tile_pool(name="gemv", bufs=2)
        gpool = gpool_cm.__enter__()
        xpool_cm = tc.tile_pool(name="xpool", bufs=1)
        xpool = xpool_cm.__enter__()

        gwpool_cm = tc.tile_pool(name="gwpool", bufs=2)
        gwpool = gwpool_cm.__enter__()
        w1pool_cm = tc.tile_pool(name="w1pool", bufs=1)
        w1pool = w1pool_cm.__enter__()

        # ------------ phase A: vc partial sums + AR1 (issue-first) ------------
        xvc = [w1pool.tile([128, VC], BF16, name=f"xvc{i}", tag=f"xvc{i}")
               for i in range(CT)]
        for i in range(CT):
            nc.sync.dma_start(xvc[i][:], xown_in[128 * i:128 * (i + 1), :])
        vcs = persist.tile([128, CT], F32, name="vcs", tag="vcs")
        for i in range(CT):
            nc.vector.tensor_reduce(vcs[:, i:i + 1], xvc[i][:],
                                    axis=mybir.AxisListType.X, op=ADD)
        ar1_in = dram.tile([C], F32, name="ar1i", tag="ar1i")
        ar1_out = dram.tile([C], F32, name="ar1o", tag="ar1o",
                            addr_space="Shared")
        nc.sync.dma_start(
            bass.AP(tensor=ar1_in[:].tensor, offset=ar1_in[:].offset,
                    ap=[[1, 128], [128, CT]]), vcs[:])
        nc.gpsimd.collective_compute(
            "AllReduce", ADD, replica_groups=[list(range(NCORES))],
            ins=[ar1_in[:]], outs=[ar1_out[:]])
        sm = [persist.tile([128, 16], F32, name=f"sm{i}", tag=f"sm{i}")
              for i in range(CT)]
        for i in range(CT):
            nc.sync.dma_start(sm[i][:], smalls[128 * i:128 * (i + 1), :])
        xp = [xpool.tile([128, XPN], BF16, name=f"xp{i}", tag=f"xp{i}")
              for i in range(CT)]
        x4 = [t.rearrange("p (z y x) -> p z y x", z=Z7, y=26, x=26) for t in xp]
        id_t = persist.tile([128, 128], BF16, name="identt", tag="identt")
        nc.sync.dma_start(id_t[:], ident[:, :])
        ones_t = persist.tile([128, 1], BF16, name="onest", tag="onest")
        nc.sync.dma_start(ones_t[:], onesc[:, :])
        onesr_t = persist.tile([1, 128], BF16, name="onesrt", tag="onesrt")
        nc.sync.dma_start(onesr_t[:], onesr_in[:, :])
        eps_t = persist.tile([128, 1], F32, name="epst", tag="epst")
        nc.vector.memset(eps_t[:], EPS)
        junk = xpool.tile([128, VC], BF16, name="junk", tag="junk")
        junk5 = junk.rearrange("p (z y x) -> p z y x", z=ZP, y=24, x=24)
        opT = persist.tile([128, CT * C], BF16, name="opT", tag="opT")
        nc.sync.dma_start(
            opT[:], bass.AP(tensor=gemvW, offset=5 * C * C,
                            ap=[[C, 128], [128 * C, CT], [1, C]]))
        gi_all = persist.tile([128, CT * G], F32, name="giall", tag="giall")
        nc.sync.dma_start(
            gi_all[:], bass.AP(tensor=gind_in, offset=0,
                               ap=[[G, 128], [128 * G, CT], [1, G]]))
        knb1_t = persist.tile([128, 3], F32, name="knb1t", tag="knb1t")
        nc.sync.dma_start(
            knb1_t[:], bass.AP(tensor=knb1, offset=0, ap=[[1, 128], [128, 3]]))
        kb2 = [persist.tile([128, KK], F32, name=f"kb2{i}", tag=f"kb2{i}")
               for i in range(CT)]
        for i in range(CT):
            nc.sync.dma_start(kb2[i][:], knb2t[128 * i:128 * (i + 1), :])
        dwk_map = {}
        for t in range(HT):
            if t not in MLP_PE:
                d = persist.tile([128, KK], F32, name=f"dwk{t}", tag=f"dwk{t}")
                nc.sync.dma_start(d[:], dwk_in[128 * t:128 * (t + 1), :])
                dwk_map[t] = d



        def load_gwbuf(dram_t, off):
            t = gwpool.tile([128, CT * C], BF16, name="gwbuf", tag="gwbuf")
            nc.scalar.dma_start(
                t[:], bass.AP(tensor=dram_t, offset=off,
                              ap=[[C, 128], [128 * C, CT], [1, C]]))
            return t

        w1t = [w1pool.tile([128, CT * W1R], BF16, name=f"w1t{h}",
                           tag=f"w1t{h}") for h in range(2)]
        for h in range(2):
            nc.scalar.dma_start(
                w1t[h][:], bass.AP(tensor=w1sT, offset=h * C * W1R,
                                   ap=[[W1R, 128], [128 * W1R, CT], [1, W1R]]))


        # ------------ phase B: text gemv chain (overlaps AR1) ------------
        def gemv(wt, in_cols, nm, act, bias_cols, tag, scale=1.0, odt=BF16):
            outs = []
            for m in range(nm):
                ps = psA.tile([128, 1], F32, name="ps_small", tag="ps_small")
                for k in range(CT):
                    nc.tensor.matmul(
                        ps[:], wt[:, C * k + 128 * m:C * k + 128 * m + 128],
                        in_cols[k][:], start=(k == 0), stop=(k == CT - 1))
                o = gpool.tile([128, 1], odt, name=f"{tag}o{m}",
                               tag=f"{tag}o{m}")
                bias = bias_cols[m] if bias_cols is not None else 0.0
                nc.scalar.activation(o[:], ps[:], act, bias=bias, scale=scale)
                outs.append(o)
            return outs

        t_cols = []
        for i in range(CT):
            t = gpool.tile([128, 1], BF16, name=f"tc{i}", tag=f"tc{i}")
            nc.scalar.activation(t[:], sm[i][:, S_Y:S_Y + 1], Copy)
            t_cols.append(t)
        h1 = gemv(load_gwbuf(gemvW, 0 * C * C), t_cols, CT, Relu,
                  [sm[i][:, S_LB1:S_LB1 + 1] for i in range(CT)], "lw1")
        h2 = gemv(load_gwbuf(gemvW, 1 * C * C), h1, CT, Iden,
                  [sm[i][:, S_LB2:S_LB2 + 1] for i in range(CT)], "lw2")
        tp = gemv(load_gwbuf(gemvW, 2 * C * C), h2, CT, Iden,
                  [sm[i][:, S_TPB:S_TPB + 1] for i in range(CT)], "tpw")
        av = gemv(load_gwbuf(gemvW, 3 * C * C), tp, CT, Iden,
                  [sm[i][:, S_ABV:S_ABV + 1] for i in range(CT)], "avw")
        attn = gemv(load_gwbuf(gemvW, 4 * C * C), av, CT, Iden,
                    [sm[i][:, S_ABO:S_ABO + 1] for i in range(CT)], "aow")
        mw = [load_gwbuf(modWT, h * C * C) for h in range(2)]

        # ------------ phase C: post-AR1 gemvs ------------
        vc_cols = []
        for i in range(CT):
            col = gpool.tile([128, 1], F32, name=f"vcc{i}", tag=f"vcc{i}")
            nc.sync.dma_start(
                col[:], bass.AP(tensor=ar1_out[:].tensor,
                                offset=ar1_out[:].offset + 128 * i,
                                ap=[[1, 128], [128, 1]]))
            cb = gpool.tile([128, 1], BF16, name=f"cmb{i}", tag=f"cmb{i}")
            nc.scalar.activation(cb[:], col[:], Copy, scale=1.0 / NTOT)
            vc_cols.append(cb)
        halves = [vc_cols, attn]

        def gemv2h(wts, stride, nm, act, bias_cols, tag, odt=F32):
            outs = []
            for m in range(nm):
                ps = psA.tile([128, 1], F32, name="ps_small", tag="ps_small")
                for h in range(2):
                    for k in range(CT):
                        nc.tensor.matmul(
                            ps[:], wts[h][:, k * stride + 128 * m:
                                          k * stride + 128 * m + 128],
                            halves[h][k][:],
                            start=(h == 0 and k == 0),
                            stop=(h == 1 and k == CT - 1))
                o = gpool.tile([128, 1], odt, name=f"{tag}o{m}",
                               tag=f"{tag}o{m}")
                nc.scalar.activation(o[:], ps[:], act, bias=bias_cols[m])
                outs.append(o)
            return outs

        mod = gemv2h(mw, C, CT, Sigmoid,
                     [sm[i][:, S_MODB:S_MODB + 1] for i in range(CT)], "modw")
        kp1 = gemv2h(w1t, W1R, 3, Relu,
                     [knb1_t[:, m:m + 1] for m in range(3)], "w1s", odt=BF16)

        w1pool_cm.__exit__(None, None, None)
        gwpool_cm.__exit__(None, None, None)

        # ------------ phase D: kernel_net gemv2 + AR2 ------------
        ko_all = persist.tile([128, KK * CT], F32, name="koall", tag="koall")
        with (tc.tile_pool(name="kseq", bufs=13) as kseq,
              tc.tile_pool(name="kps", bufs=1, space="PSUM") as kps):
            for i in (0, 4, 5):
                nc.sync.dma_start(xp[i][:], xpad_in[128 * i:128 * (i + 1), :])
            kop = kps.tile([128, KK * CT], F32, name="g2ps", tag="g2ps")
            for t in range(KK):
                wt = kseq.tile([128, 3 * C], BF16, name="w2t", tag="w2t")
                nc.gpsimd.dma_start(
                    wt[:], bass.AP(tensor=w2sT, offset=t * C,
                                   ap=[[KFLAT, 128], [128 * KFLAT, 3],
                                       [1, C]]))
                for m in range(CT):
                    for k in range(3):
                        nc.tensor.matmul(
                            kop[:, CT * t + m:CT * t + m + 1],
                            wt[:, k * C + 128 * m:k * C + 128 * m + 128],
                            kp1[k][:], start=(k == 0), stop=(k == 2))
            nc.scalar.activation(ko_all[:], kop[:], Copy)
            for i in (1, 2, 3):
                nc.gpsimd.dma_start(xp[i][:], xpad_in[128 * i:128 * (i + 1), :])
        ar2_in = dram.tile([128, KK * CT], F32, name="ar2i", tag="ar2i")
        ar2_out = dram.tile([128, KK * CT], F32, name="ar2o", tag="ar2o",
                            addr_space="Shared")
        nc.sync.dma_start(ar2_in[:, :], ko_all[:])
        nc.gpsimd.collective_compute(
            "AllReduce", ADD, replica_groups=[list(range(NCORES))],
            ins=[ar2_in[:]], outs=[ar2_out[:]])
        kraw = persist.tile([128, KK * CT], F32, name="kraw", tag="kraw")
        nc.sync.dma_start(kraw[:], ar2_out[:, :])
        kraw3 = kraw.rearrange("p (t i) -> p t i", t=KK, i=CT)


        # kernels: +bias, fold mod
        kernm = []
        for i in range(CT):
            km = persist.tile([128, KK], F32, name=f"kernm{i}", tag=f"kernm{i}")
            nc.vector.tensor_tensor(km[:], kraw3[:, :, i], kb2[i][:], op=ADD)
            nc.vector.tensor_scalar_mul(km[:], km[:], mod[i][:, 0:1])
            kernm.append(km)

        if DEBUG:
            for i in range(CT):
                nc.gpsimd.dma_start(dbg_mod[128 * i:128 * (i + 1), :], mod[i][:])
                nc.gpsimd.dma_start(dbg_kern[128 * i:128 * (i + 1), :], kernm[i][:])

        # ------------ phase E: dynamic depthwise conv + GN stats ------------
        dynpool_cm = tc.tile_pool(name="dynpool", bufs=1)
        dynpool = dynpool_cm.__enter__()
        dyn = [dynpool.tile([128, V5], BF16, name=f"dyn{i}", tag=f"dyn{i}")
               for i in range(CT)]
        dyn4 = [t.rearrange("p (z y x) -> p z y x", z=Z5, y=24, x=24)
                for t in dyn]

        dgpool_cm = tc.tile_pool(name="dgpool", bufs=2)
        dgpool = dgpool_cm.__enter__()

        def build_diag(i):
            d = dgpool.tile([128, KK * 128], BF16, name="dg", tag="dg")
            for t in range(KK):
                nc.vector.tensor_scalar_mul(d[:, 128 * t:128 * (t + 1)],
                                            id_t[:], kernm[i][:, t:t + 1])
            return d

        def dyn_pe_planes(i, dgt, zos, cpool):
            for zo in zos:
                for half in range(2):
                    y0 = 12 * half
                    cp = cpool.tile([128, 288], F32, name=f"dcp{half}",
                                    tag=f"dcp{half}")
                    for ti, (dz, dy, dx) in enumerate(TAPS):
                        nc.tensor.matmul(
                            cp[:], dgt[:, 128 * ti:128 * (ti + 1)],
                            x4[i][:, zo + 1 + dz, 1 + y0 + dy:13 + y0 + dy,
                                  1 + dx:25 + dx],
                            start=(ti == 0), stop=(ti == KK - 1))
                    nc.scalar.activation(
                        dyn4[i][:, zo, y0:y0 + 12, :], cp[:], Copy)

        def dyn_dve_planes(i, zo0, nz, tpool):
            dst = dyn4[i][:, zo0:zo0 + nz, :, :]
            for ti, (dz, dy, dx) in enumerate(TAPS):
                src = x4[i][:, zo0 + 1 + dz:zo0 + 1 + dz + nz,
                            1 + dy:25 + dy, 1 + dx:25 + dx]
                if ti == 0:
                    nc.vector.tensor_scalar_mul(dst, src, kernm[i][:, 0:1])
                else:
                    tmp = tpool.tile([128, 3 * PL], BF16, name="dtmp",
                                     tag="dtmp")
                    t4 = tmp.rearrange("p (z y x) -> p z y x", z=3, y=24,
                                       x=24)[:, 0:nz, :, :]
                    nc.vector.tensor_scalar_mul(t4, src,
                                                kernm[i][:, ti:ti + 1])
                    nc.vector.tensor_tensor(dst, dst, t4, op=ADD)

        gst = persist.tile([128, 2 * CT], F32, name="gst", tag="gst")
        ar3_in = dram.tile([G, 2], F32, name="ar3i", tag="ar3i")
        ar3_out = dram.tile([G, 2], F32, name="ar3o", tag="ar3o",
                            addr_space="Shared")
        gsb = persist.tile([G, 2], F32, name="gsb", tag="gsb")
        with (tc.tile_pool(name="dcpool", bufs=1, space="PSUM") as dcpool,
              tc.tile_pool(name="dtpool", bufs=2) as dtpool,
              tc.tile_pool(name="gnps", bufs=1, space="PSUM") as gnps):
            # own planes (zo 1..3) first, stats, then halo planes under AR3
            dg_live = {}
            for i in range(CT):
                if i in DYN_PE:
                    dg_live[i] = build_diag(i)
                    dyn_pe_planes(i, dg_live[i], (1, 2, 3), dcpool)
                else:
                    dyn_dve_planes(i, 1, 3, dtpool)
                nc.scalar.activation(junk[:], dyn[i][:, PL:4 * PL],
                                     Copy, accum_out=gst[:, 2 * i:2 * i + 1])
                nc.scalar.activation(junk[:], dyn[i][:, PL:4 * PL],
                                     Square,
                                     accum_out=gst[:, 2 * i + 1:2 * i + 2])
            gps = gnps.tile([G, 2], F32, name="gps", tag="gps")
            for i in range(CT):
                nc.tensor.matmul(gps[:], gi_all[:, G * i:G * (i + 1)],
                                 gst[:, 2 * i:2 * i + 2], start=(i == 0),
                                 stop=(i == CT - 1))
            nc.scalar.activation(gsb[:], gps[:], Copy)
            nc.sync.dma_start(ar3_in[:, :], gsb[:])
            nc.gpsimd.collective_compute(
                "AllReduce", ADD, replica_groups=[list(range(NCORES))],
                ins=[ar3_in[:]], outs=[ar3_out[:]])
            for i in range(CT):
                if i in DYN_PE:
                    dg2 = build_diag(i)
                    dyn_pe_planes(i, dg2, (0, 4), dcpool)
                else:
                    dyn_dve_planes(i, 0, 1, dtpool)
                    dyn_dve_planes(i, 4, 1, dtpool)
        dgpool_cm.__exit__(None, None, None)

        if DEBUG:
            for i in range(CT):
                nc.gpsimd.dma_start(dbg_dyn[128 * i:128 * (i + 1), :], dyn[i][:])

        # ------------ GN scale/shift + fold into opT ------------
        gstat = persist.tile([G, 2], F32, name="gstat", tag="gstat")
        nc.sync.dma_start(gstat[:], ar3_out[:, :])
        NGRP = float(GD) * NTOT
        gmr = persist.tile([G, 2], F32, name="gmr", tag="gmr")
        nc.scalar.activation(gmr[:, 0:1], gstat[:, 0:1], Copy, scale=1.0 / NGRP)
        musq = persist.tile([G, 1], F32, name="musq", tag="musq")
        nc.scalar.square(musq[:], gmr[:, 0:1])
        var = persist.tile([G, 1], F32, name="gvar", tag="gvar")
        nc.vector.tensor_scalar(var[:], gstat[:, 1:2], 1.0 / NGRP, None,
                                op0=MULT)
        nc.vector.tensor_sub(var[:], var[:], musq[:])
        nc.scalar.activation(var[:], var[:], Sqrt, bias=eps_t[0:G, 0:1])
        nc.vector.reciprocal(gmr[:, 1:2], var[:])

        cafm_shift = []
        gsc = []
        for i in range(CT):
            ge = gpool.tile([G, 128], F32, name=f"gexp{i}", tag=f"gexp{i}")
            nc.sync.dma_start(ge[:], gexpT[:, 128 * i:128 * (i + 1)])
            ps = psA.tile([128, 2], F32, name="ps_sm2", tag="ps_sm2")
            nc.tensor.matmul(ps[:], ge[:], gmr[:], start=True, stop=True)
            mu_c = persist.tile([128, 2], F32, name=f"muc{i}", tag=f"muc{i}")
            nc.scalar.activation(mu_c[:], ps[:], Copy)
            a = persist.tile([128, 1], F32, name=f"gsc{i}", tag=f"gsc{i}")
            nc.vector.tensor_mul(a[:], sm[i][:, S_GNG:S_GNG + 1], mu_c[:, 1:2])
            b = persist.tile([128, 1], F32, name=f"gsh{i}", tag=f"gsh{i}")
            nc.vector.tensor_mul(b[:], mu_c[:, 0:1], a[:])
            nc.vector.tensor_sub(b[:], sm[i][:, S_GNB:S_GNB + 1], b[:])
            gsc.append(a)
            bb = gpool.tile([128, 1], BF16, name=f"gshb{i}", tag=f"gshb{i}")
            nc.scalar.activation(bb[:], b[:], Copy)
            cafm_shift.append(bb)
        cb_cols = []
        for m in range(CT):
            ps = psA.tile([128, 1], F32, name="ps_small", tag="ps_small")
            for k in range(CT):
                nc.tensor.matmul(ps[:], opT[:, 768 * k + 128 * m:
                                            768 * k + 128 * m + 128],
                                 cafm_shift[k][:], start=(k == 0),
                                 stop=(k == CT - 1))
            o = persist.tile([128, 1], F32, name=f"cbc{m}", tag=f"cbc{m}")
            nc.scalar.activation(o[:], ps[:], Iden,
                                 bias=sm[m][:, S_OPB:S_OPB + 1])
            cb_cols.append(o)
        for k in range(CT):
            nc.vector.tensor_scalar_mul(opT[:, 768 * k:768 * (k + 1)],
                                        opT[:, 768 * k:768 * (k + 1)],
                                        gsc[k][:])

        # ------------ phase F1: cafm matmul + xb ------------
        xb = [xbpool.tile([128, V5], BF16, name=f"xb{m}", tag=f"xb{m}")
              for m in range(CT)]
        xb4 = [t.rearrange("p (z y x) -> p z y x", z=Z5, y=24, x=24)
               for t in xb]
        CH6 = [(o, min(512, V5 - o)) for o in range(0, V5, 512)]
        with (tc.tile_pool(name="f1ps", bufs=2, space="PSUM") as f1ps,
              tc.tile_pool(name="f1t", bufs=2) as f1t):
            for m in range(CT):
                tca = f1t.tile([128, V5], BF16, name="tcafm", tag="tcafm")
                for o, n in CH6:
                    ps = f1ps.tile([128, 512], F32, name="f1p", tag="f1p")
                    for k in range(CT):
                        nc.tensor.matmul(ps[:, 0:n],
                                         opT[:, 768 * k + 128 * m:
                                             768 * k + 128 * m + 128],
                                         dyn[k][:, o:o + n], start=(k == 0),
                                         stop=(k == CT - 1))
                    nc.scalar.activation(tca[:, o:o + n], ps[:, 0:n], Iden,
                                         bias=cb_cols[m][:])
                t4 = tca.rearrange("p (z y x) -> p z y x", z=Z5, y=24, x=24)
                nc.vector.tensor_tensor(xb4[m][:, :, :, :], t4[:, :, :, :],
                                        x4[m][:, 1:6, 1:25, 1:25], op=MULT)
        if DEBUG:
            for m in range(CT):
                nc.gpsimd.dma_start(dbg_xb[128 * m:128 * (m + 1), :], xb[m][:])
        dynpool_cm.__exit__(None, None, None)
        xpool_cm.__exit__(None, None, None)
        gpool_cm.__exit__(None, None, None)
        psA_cm.__exit__(None, None, None)

        # ------------ LN helper ------------
        def ln_stats(tiles, nv, tag, mupool):
            """Per-voxel mean/rstd over channels -> [128, nv] bf16 tiles.
            ones_t carries 1/C so the matmuls produce mean / E[x^2]."""
            nch = [(o, min(480, nv - o)) for o in range(0, nv, 480)]
            muB = mupool.tile([128, nv], BF16, name=f"{tag}muB",
                              tag=f"{tag}muB")
            rsB = mupool.tile([128, nv], BF16, name=f"{tag}rsB",
                              tag=f"{tag}rsB")
            with (tc.tile_pool(name=f"{tag}ps", bufs=1, space="PSUM") as lps,
                  tc.tile_pool(name=f"{tag}sq", bufs=2) as sqp,
                  tc.tile_pool(name=f"{tag}rw", bufs=2) as rwp):
                for o, n in nch:
                    p1 = lps.tile([1, 480], F32, name="s1", tag="s1")
                    p2 = lps.tile([1, 480], F32, name="s2", tag="s2")
                    for k in range(CT):
                        nc.tensor.matmul(p1[:, 0:n], ones_t[:],
                                         tiles[k][:, o:o + n],
                                         start=(k == 0), stop=(k == CT - 1))
                    for k in range(CT):
                        q = sqp.tile([128, 480], BF16, name="sqc", tag="sqc")
                        if k % 2 == 0:
                            nc.scalar.activation(q[:, 0:n],
                                                 tiles[k][:, o:o + n], Square)
                        else:
                            nc.vector.tensor_mul(q[:, 0:n],
                                                 tiles[k][:, o:o + n],
                                                 tiles[k][:, o:o + n])
                        nc.tensor.matmul(p2[:, 0:n], ones_t[:], q[:, 0:n],
                                         start=(k == 0), stop=(k == CT - 1))
                    mubf = rwp.tile([1, 480], BF16, name="mubf", tag="mubf")
                    nc.scalar.activation(mubf[:, 0:n], p1[:, 0:n], Copy)
                    m2 = rwp.tile([1, 480], F32, name="m2", tag="m2")
                    nc.scalar.activation(m2[:, 0:n], p1[:, 0:n], Square)
                    vr = rwp.tile([1, 480], F32, name="vr", tag="vr")
                    nc.vector.tensor_sub(vr[:, 0:n], p2[:, 0:n], m2[:, 0:n])
                    nc.scalar.activation(vr[:, 0:n], vr[:, 0:n], Sqrt,
                                         bias=eps_t[0:1, 0:1])
                    rbf = rwp.tile([1, 480], BF16, name="rbf", tag="rbf")
                    with nc.allow_low_precision(reason="rstd bcast in bf16"):
                        nc.vector.reciprocal(rbf[:, 0:n], vr[:, 0:n])
                    pb = lps.tile([128, 480], F32, name="bc", tag="bc")
                    nc.tensor.matmul(pb[:, 0:n], onesr_t[:], mubf[:, 0:n],
                                     start=True, stop=True)
                    nc.scalar.activation(muB[:, o:o + n], pb[:, 0:n], Copy)
                    pb2 = lps.tile([128, 480], F32, name="bc2", tag="bc2")
                    nc.tensor.matmul(pb2[:, 0:n], onesr_t[:], rbf[:, 0:n],
                                     start=True, stop=True)
                    nc.scalar.activation(rsB[:, o:o + n], pb2[:, 0:n], Copy)
            return muB, rsB

        # ------------ LN2 + edge masks ------------
        gatepool_cm = tc.tile_pool(name="gatepool", bufs=1)
        gatepool = gatepool_cm.__enter__()
        xlnpool_cm = tc.tile_pool(name="xlnpool", bufs=1)
        xlnpool = xlnpool_cm.__enter__()
        xln = [xlnpool.tile([128, V5], BF16, name=f"xln{k}", tag=f"xln{k}")
               for k in range(CT)]
        mupool_cm = tc.tile_pool(name="mupool", bufs=1)
        mupool = mupool_cm.__enter__()
        muB, rsB = ln_stats(xb, V5, "ln2", mupool)
        with tc.tile_pool(name="lnap", bufs=2) as lnap:
            for k in range(CT):
                t1 = lnap.tile([128, V5], BF16, name="lnt1", tag="lnt1")
                nc.vector.tensor_sub(t1[:], xb[k][:], muB[:])
                nc.vector.tensor_mul(t1[:], t1[:], rsB[:])
                nc.vector.tensor_scalar(xln[k][:], t1[:],
                                        sm[k][:, S_N2W:S_N2W + 1],
                                        sm[k][:, S_N2B:S_N2B + 1],
                                        op0=MULT, op1=ADD)
                nc.vector.tensor_scalar_mul(xln[k][:, 0:PL], xln[k][:, 0:PL],
                                            sm[k][:, S_MLO:S_MLO + 1])
                nc.vector.tensor_scalar_mul(xln[k][:, 4 * PL:5 * PL],
                                            xln[k][:, 4 * PL:5 * PL],
                                            sm[k][:, S_MHI:S_MHI + 1])

        # ------------ MLP: Wi + depthwise conv + gate ------------
        if DEBUG:
            for k in range(CT):
                nc.gpsimd.dma_start(dbg_xln[128 * k:128 * (k + 1), :], xln[k][:])
        mupool_cm.__exit__(None, None, None)
        gate = [gatepool.tile([128, VC], BF16, name=f"gate{j}", tag=f"gate{j}")
                for j in range(HT // 2)]
        hppool_cm = tc.tile_pool(name="hppool", bufs=1)
        hppool = hppool_cm.__enter__()
        hpads = [hppool.tile([128, HPN], BF16, name=f"hpad{b}", tag=f"hpad{b}")
                 for b in range(4)]
        for b in range(4):
            nc.vector.memset(hpads[b][:], 0.0)
        hp4s = [t.rearrange("p (z y x) -> p z y x", z=Z5, y=26, x=26)
                for t in hpads]

        with (tc.tile_pool(name="wiw", bufs=2) as wiw,
              tc.tile_pool(name="dga", bufs=2) as dgap,
              tc.tile_pool(name="wips", bufs=2, space="PSUM") as wips,
              tc.tile_pool(name="cvps", bufs=1, space="PSUM") as cvps,
              tc.tile_pool(name="mlpt", bufs=1) as mlpt):

            def wi_pass(t, bi):
                hp4 = hp4s[bi]
                wall = wiw.tile([128, CT * 128], BF16, name="wiall",
                                tag="wiall")
                nc.gpsimd.dma_start(wall[:], wiT[t, :, :])
                for z in range(Z5):
                    for halfy in range(2):
                        y0 = 12 * halfy
                        pz = wips.tile([128, 288], F32, name=f"wip{halfy}",
                                       tag=f"wip{halfy}")
                        for k in range(CT):
                            nc.tensor.matmul(
                                pz[:],
                                wall[:, 128 * k:128 * (k + 1)],
                                xln[k][:, z * PL + y0 * 24:
                                       z * PL + y0 * 24 + 288],
                                start=(k == 0), stop=(k == CT - 1))
                        nc.scalar.activation(
                            hp4[:, z, 1 + y0:13 + y0, 1:25],
                            pz.rearrange("p (y x) -> p y x", y=12, x=24)[
                                :, :, :], Copy)
                if DEBUG and t == 0:
                    nc.gpsimd.dma_start(dbg_h[:, :], hpads[bi][:])

            PTAPS = 4

            def conv_pass(t, bi, dst, act):
                hp4 = hp4s[bi]
                if t in MLP_PE:
                    if dst is None:
                        dst = mlpt.tile([128, VC], BF16, name="conv2",
                                        tag="conv2")
                    dgt = dgap.tile([128, KK * 128], BF16, name="dgall",
                                    tag="dgall")
                    nc.gpsimd.dma_start(dgt[:], dmlp[t, :, :])
                    for zo in range(3):
                        for halfy in range(2):
                            b = 2 * zo + halfy
                            y0 = 12 * halfy
                            cp = cvps.tile([128, 288], F32,
                                           name=f"mcp{b % 2}",
                                           tag=f"mcp{b % 2}")
                            for ti, (dz, dy, dx) in enumerate(TAPS):
                                nc.tensor.matmul(
                                    cp[:], dgt[:, 128 * ti:128 * (ti + 1)],
                                    hp4[:, 1 + zo + dz,
                                        1 + y0 + dy:13 + y0 + dy,
                                        1 + dx:25 + dx],
                                    start=(ti == 0), stop=(ti == KK - 1))
                            nc.scalar.activation(
                                dst[:, 288 * b:288 * (b + 1)], cp[:], act)
                    return dst
                # PE computes the first PTAPS taps into a bf16 partial
                dgp = dgap.tile([128, PTAPS * 128], BF16, name="dgpart",
                                tag="dgpart")
                nc.gpsimd.dma_start(
                    dgp[:], bass.AP(tensor=dmlp, offset=t * 128 * KK * 128,
                                    ap=[[KK * 128, 128], [1, PTAPS * 128]]))
                pcv = mlpt.tile([128, VC], BF16, name="pconv", tag="pconv")
                for zo in range(3):
                    for halfy in range(2):
                        b = 2 * zo + halfy
                        y0 = 12 * halfy
                        cp = cvps.tile([128, 288], F32, name=f"mcp{b % 2}",
                                       tag=f"mcp{b % 2}")
                        for ti in range(PTAPS):
                            dz, dy, dx = TAPS[ti]
                            nc.tensor.matmul(
                                cp[:], dgp[:, 128 * ti:128 * (ti + 1)],
                                hp4[:, 1 + zo + dz, 1 + y0 + dy:13 + y0 + dy,
                                    1 + dx:25 + dx],
                                start=(ti == 0), stop=(ti == PTAPS - 1))
                        nc.scalar.activation(pcv[:, 288 * b:288 * (b + 1)],
                                             cp[:], Copy)
                acc = mlpt.tile([128, VC], BF16, name="macc", tag="macc")
                a4 = acc.rearrange("p (z y x) -> p z y x", z=3, y=24, x=24)
                kw = dwk_map[t]
                for ti in range(PTAPS, KK):
                    dz, dy, dx = TAPS[ti]
                    src = hp4[:, 1 + dz:4 + dz, 1 + dy:25 + dy, 1 + dx:25 + dx]
                    if ti == PTAPS:
                        nc.vector.tensor_scalar_mul(a4[:, :, :, :], src,
                                                    kw[:, ti:ti + 1])
                    else:
                        tmp = mlpt.tile([128, VC], BF16, name="mtmp",
                                        tag="mtmp")
                        t4 = tmp.rearrange("p (z y x) -> p z y x", z=3, y=24,
                                           x=24)
                        nc.vector.tensor_scalar_mul(t4[:, :, :, :], src,
                                                    kw[:, ti:ti + 1])
                        nc.vector.tensor_tensor(a4[:, :, :, :], a4[:, :, :, :],
                                                t4[:, :, :, :], op=ADD)
                nc.vector.tensor_tensor(acc[:], acc[:], pcv[:], op=ADD)
                if act is Gelu:
                    nc.scalar.activation(dst[:], acc[:], Gelu)
                    return dst
                return acc

            for j in range(HT // 2):
                b1, b2 = (2 * j) % 4, (2 * j + 1) % 4
                wi_pass(j, b1)
                wi_pass(j + HT // 2, b2)
                conv_pass(j, b1, gate[j], Gelu)
                if DEBUG and j == 0:
                    nc.gpsimd.dma_start(dbg_c1[:, :], gate[0][:])
                c2 = conv_pass(j + HT // 2, b2, None, Copy)
                nc.vector.tensor_mul(gate[j][:], gate[j][:], c2[:])
        if DEBUG:
            nc.gpsimd.dma_start(dbg_gate[:, :], gate[0][:])
        hppool_cm.__exit__(None, None, None)
        xlnpool_cm.__exit__(None, None, None)

        # ------------ Wo + residual + LN3 (chunk-major, pipelined) ------------
        y_t = [ytpool.tile([128, VC], BF16, name=f"y{m}", tag=f"y{m}")
               for m in range(CT)]
        CH3 = [(0, 480), (480, 480), (960, 480), (1440, 288)]
        with (tc.tile_pool(name="wow", bufs=1) as wow,
              tc.tile_pool(name="wops", bufs=2, space="PSUM") as wops,
              tc.tile_pool(name="ln3ps", bufs=1, space="PSUM") as l3ps,
              tc.tile_pool(name="ln3sq", bufs=2) as l3sq,
              tc.tile_pool(name="ln3rw", bufs=2) as l3rw,
              tc.tile_pool(name="mupool3", bufs=1) as mupool3,
              tc.tile_pool(name="lnap3", bufs=2) as lnap3):
            walls = [wow.tile([128, (HT // 2) * 128], BF16, name=f"wo{m}",
                              tag=f"wo{m}") for m in range(CT)]
            for m in range(CT):
                nc.gpsimd.dma_start(walls[m][:], woT[m, :, :])
            muB3 = mupool3.tile([128, VC], BF16, name="ln3muB", tag="ln3muB")
            rsB3 = mupool3.tile([128, VC], BF16, name="ln3rsB", tag="ln3rsB")
            for o, n in CH3:
                for m in range(CT):
                    ps = wops.tile([128, 512], F32, name="wo_ps", tag="wo_ps")
                    for k in range(HT // 2):
                        nc.tensor.matmul(ps[:, 0:n],
                                         walls[m][:, 128 * k:128 * (k + 1)],
                                         gate[k][:, o:o + n],
                                         start=(k == 0), stop=False)
                    nc.tensor.matmul(ps[:, 0:n], id_t[:],
                                     xb[m][:, PL + o:PL + o + n],
                                     start=False, stop=True)
                    nc.scalar.activation(y_t[m][:, o:o + n], ps[:, 0:n], Copy)
                # LN3 stats for this chunk
                p1 = l3ps.tile([1, 480], F32, name="l3s1", tag="l3s1")
                p2 = l3ps.tile([1, 480], F32, name="l3s2", tag="l3s2")
                for k in range(CT):
                    nc.tensor.matmul(p1[:, 0:n], ones_t[:], y_t[k][:, o:o + n],
                                     start=(k == 0), stop=(k == CT - 1))
                for k in range(CT):
                    q = l3sq.tile([128, 480], BF16, name="l3q", tag="l3q")
                    nc.vector.tensor_mul(q[:, 0:n], y_t[k][:, o:o + n],
                                         y_t[k][:, o:o + n])
                    nc.tensor.matmul(p2[:, 0:n], ones_t[:], q[:, 0:n],
                                     start=(k == 0), stop=(k == CT - 1))
                mubf = l3rw.tile([1, 480], BF16, name="l3mubf", tag="l3mubf")
                nc.scalar.activation(mubf[:, 0:n], p1[:, 0:n], Copy)
                m2 = l3rw.tile([1, 480], F32, name="l3m2", tag="l3m2")
                nc.scalar.activation(m2[:, 0:n], p1[:, 0:n], Square)
                vr = l3rw.tile([1, 480], F32, name="l3vr", tag="l3vr")
                nc.vector.tensor_sub(vr[:, 0:n], p2[:, 0:n], m2[:, 0:n])
                nc.scalar.activation(vr[:, 0:n], vr[:, 0:n], Sqrt,
                                     bias=eps_t[0:1, 0:1])
                rbf = l3rw.tile([1, 480], BF16, name="l3rbf", tag="l3rbf")
                with nc.allow_low_precision(reason="rstd bcast in bf16"):
                    nc.vector.reciprocal(rbf[:, 0:n], vr[:, 0:n])
                pb = l3ps.tile([128, 480], F32, name="l3bc", tag="l3bc")
                nc.tensor.matmul(pb[:, 0:n], onesr_t[:], mubf[:, 0:n],
                                 start=True, stop=True)
                nc.scalar.activation(muB3[:, o:o + n], pb[:, 0:n], Copy)
                pb2 = l3ps.tile([128, 480], F32, name="l3bc2", tag="l3bc2")
                nc.tensor.matmul(pb2[:, 0:n], onesr_t[:], rbf[:, 0:n],
                                 start=True, stop=True)
                nc.scalar.activation(rsB3[:, o:o + n], pb2[:, 0:n], Copy)
                # LN3 apply + store for this chunk
                for k in range(CT):
                    t1 = lnap3.tile([128, 480], BF16, name="ln3t", tag="ln3t")
                    nc.vector.tensor_sub(t1[:, 0:n], y_t[k][:, o:o + n],
                                         muB3[:, o:o + n])
                    nc.vector.tensor_mul(t1[:, 0:n], t1[:, 0:n],
                                         rsB3[:, o:o + n])
                    of = lnap3.tile([128, 480], F32, name="outf", tag="outf")
                    nc.vector.tensor_scalar(of[:, 0:n], t1[:, 0:n],
                                            sm[k][:, S_N3W:S_N3W + 1],
                                            sm[k][:, S_N3B:S_N3B + 1],
                                            op0=MULT, op1=ADD)
                    nc.sync.dma_start(out[128 * k:128 * (k + 1), o:o + n],
                                      of[:, 0:n])
        gatepool_cm.__exit__(None, None, None)

        if DEBUG:
            for m in range(CT):
                nc.gpsimd.dma_start(dbg_y[128 * m:128 * (m + 1), :], y_t[m][:])

    nc.compile()
    return nc


def _prep(inputs):
    bf = lambda a: np.ascontiguousarray(a).astype(BF)
    f32 = lambda a: np.ascontiguousarray(a, dtype=np.float32)
    x = f32(inputs["x"][0]).reshape(C, S, S, S)

    smalls_c = np.zeros((C, 16), np.float32)
    smalls_c[:, 0] = f32(inputs["y"][0, 0])
    for i, k in enumerate(["lora_b1", "lora_b2", "tp_b", "attn_bv", "attn_bo",
                           "op_b", "n2_w", "n2_b", "n3_w", "n3_b", "gn_g",
                           "gn_b", "mod_b"]):
        smalls_c[:, i + 1] = f32(inputs[k])

    gind6 = np.zeros((CT, 128, G), np.float32)
    for j in range(CT):
        for p in range(128):
            gind6[j, p, (128 * j + p) // GD] = 1.0
    gexpT = np.zeros((G, C), np.float32)
    for c in range(C):
        gexpT[c // GD, c] = 1.0

    gemv_stack = np.stack([
        f32(inputs["lora_W1"]).T, f32(inputs["lora_W2"]).T,
        f32(inputs["tp_W"]).T, f32(inputs["attn_Wv"]).T,
        f32(inputs["attn_Wo"]).T, f32(inputs["op_W"]).T])

    kn_W2 = f32(inputs["kn_W2"])
    w2r = kn_W2.reshape(C, KK, HID).transpose(1, 0, 2).reshape(KFLAT, HID)
    w2T = np.ascontiguousarray(w2r.T)          # [HID, KFLAT]
    kn_W1 = f32(inputs["kn_W1"])

    mlp_dw = f32(inputs["mlp_dw"]).reshape(HID, KK)
    dmlp = np.zeros((HT, 128, KK, 128), np.float32)
    idx = np.arange(128)
    for tt in range(HT):
        for ti in range(KK):
            dmlp[tt, idx, ti, idx] = mlp_dw[128 * tt:128 * (tt + 1), ti]
    dmlp = dmlp.reshape(HT, 128, KK * 128)

    com = dict(
        knb2t=f32(inputs["kn_b2"]).reshape(C, KK),
        gind6=gind6, gexpT=gexpT,
        ident=bf(np.eye(128, dtype=np.float32)),
        onesc=bf(np.full((128, 1), 1.0 / C, np.float32)),
        onesr=bf(np.ones((1, 128), np.float32)),
        gemvW=bf(gemv_stack),
        modWT=bf(f32(inputs["mod_W"]).T),
        wiT=bf(f32(inputs["mlp_Wi"]).T.reshape(CT, 128, HT, 128)
               .transpose(2, 1, 0, 3).reshape(HT, 128, CT * 128)),
        woT=bf(f32(inputs["mlp_Wo"]).T.reshape(HT // 2, 128, CT, 128)
               .transpose(2, 1, 0, 3).reshape(CT, 128, (HT // 2) * 128)),
        dmlp=bf(dmlp), dwk=mlp_dw,
    )

    in_maps = []
    for i in range(NCORES):
        z0 = ZP * i
        xh = np.zeros((C, Z7, 26, 26), np.float32)
        lo, hi = max(z0 - 2, 0), min(z0 + ZP + 2, S)
        xh[:, lo - (z0 - 2):lo - (z0 - 2) + (hi - lo), 1:25, 1:25] = \
            x[:, lo:hi]
        smalls = smalls_c.copy()
        smalls[:, S_MLO] = 0.0 if i == 0 else 1.0
        smalls[:, S_MHI] = 0.0 if i == NCORES - 1 else 1.0
        m = dict(com)
        m.update(
            xpad=xh.reshape(C, XPN).astype(BF),
            xown=np.ascontiguousarray(
                x[:, z0:z0 + ZP].reshape(C, VC)).astype(BF),
            smalls=smalls,
            knb1=f32(inputs["kn_b1"][W1R * i:W1R * (i + 1)]),
            w1sT=bf(kn_W1[W1R * i:W1R * (i + 1), :].T),
            w2sT=bf(w2T[W1R * i:W1R * (i + 1), :]),
        )
        in_maps.append(m)
    return in_maps


def kernel(**inputs) -> np.ndarray:
    if "nc" not in _CACHE:
        _CACHE["nc"] = build_program()
    nc = _CACHE["nc"]
    in_maps = _prep(inputs)
    res = run_bass_kernel_spmd(nc, in_maps, list(range(NCORES)))
    outs = [res.results[i]["out"].reshape(C, ZP, PL) for i in range(NCORES)]
    full = np.concatenate(outs, axis=1)
    return full.reshape(1, C, S, S, S).astype(np.float32)


# revision 54
# speedup vs baseline: 2.1800x; 1.0269x over previous
"""Bass/Trainium2 SPMD kernel for nn_Block3D (8 NeuronCores).

Spatial z-shard (24 planes -> 3 per core) with a 2-plane host-prepped halo:
each core computes cafm/xb/xln on 5 planes locally, so the depthwise MLP conv
needs no on-device halo exchange (no AllGather). The per-channel `mod` gate is
folded into the dynamic conv kernels. Depthwise 3x3x3 convs are split between
the PE (diagonal-weight matmuls) and DVE (tensor_scalar 4x + tensor_tensor 2x
chains). All DMAs ride the HWDGE path (nc.sync); kernel_net output is staged
as one [128,162] tile around a single AllReduce.
"""

from contextlib import ExitStack

import numpy as np
import ml_dtypes

import concourse.bass as bass
import concourse.bacc as bacc
import concourse.tile as tile
from concourse import mybir
from concourse.bass_utils import run_bass_kernel_spmd

BF = ml_dtypes.bfloat16
F32 = mybir.dt.float32
BF16 = mybir.dt.bfloat16

C = 768
G = 12
GD = 64
S = 24
HID = 4 * C
KK = 27
EPS = 1e-5
NCORES = 8
ZP = S // NCORES          # 3 own planes
PL = S * S                # 576
Z7 = ZP + 4               # 7 input planes (2-halo each side)
Z5 = ZP + 2               # 5 computed planes (1-halo each side)
PPL = 26 * 26             # 676 padded plane
XPN = Z7 * PPL            # 4732
HPN = Z5 * PPL            # 3380
V5 = Z5 * PL              # 2880
VC = ZP * PL              # 1728
CT = C // 128             # 6
HT = HID // 128           # 24
W1R = HID // NCORES       # 384 hidden rows per core
KFLAT = C * KK
NTOT = float(S * S * S)

TAPS = [(dz, dy, dx) for dz in (-1, 0, 1) for dy in (-1, 0, 1) for dx in (-1, 0, 1)]

# engine split for the depthwise convs (PE diag-matmul vs DVE chains)
DYN_PE = (0, 1, 2, 3)
MLP_PE = tuple([j for j in range(0, 12, 2)] + [j + 12 for j in range(1, 12, 2)])

_CACHE = {}
DEBUG = False

Copy = mybir.ActivationFunctionType.Copy
Iden = mybir.ActivationFunctionType.Identity
Gelu = mybir.ActivationFunctionType.Gelu
Sigmoid = mybir.ActivationFunctionType.Sigmoid
Square = mybir.ActivationFunctionType.Square
Sqrt = mybir.ActivationFunctionType.Sqrt
Relu = mybir.ActivationFunctionType.Relu
ADD = mybir.AluOpType.add
SUB = mybir.AluOpType.subtract
MULT = mybir.AluOpType.mult

(S_Y, S_LB1, S_LB2, S_TPB, S_ABV, S_ABO, S_OPB, S_N2W, S_N2B, S_N3W, S_N3B,
 S_GNG, S_GNB, S_MODB, S_MLO, S_MHI) = range(16)


def build_program():
    nc = bacc.Bacc("TRN2", target_bir_lowering=False)

    def dram_in(name, shape, dtype=F32):
        return nc.declare_dram_parameter(name, list(shape), dtype, isOutput=False)

    xpad_in = dram_in("xpad", [C, XPN], BF16)
    xown_in = dram_in("xown", [C, VC], BF16)
    smalls = dram_in("smalls", [C, 16])
    knb1 = dram_in("knb1", [W1R])
    knb2t = dram_in("knb2t", [C, KK])
    ident = dram_in("ident", [128, 128], BF16)
    onesc = dram_in("onesc", [128, 1], BF16)
    onesr_in = dram_in("onesr", [1, 128], BF16)
    gind_in = dram_in("gind6", [CT, 128, G])
    gexpT = dram_in("gexpT", [G, C])
    gemvW = dram_in("gemvW", [6, C, C], BF16)     # lora1,lora2,tp,av,ao,op (all .T)
    modWT = dram_in("modWT", [2 * C, C], BF16)
    w1sT = dram_in("w1sT", [2 * C, W1R], BF16)
    w2sT = dram_in("w2sT", [W1R, KFLAT], BF16)
    wiT = dram_in("wiT", [HT, 128, CT * 128], BF16)
    woT = dram_in("woT", [CT, 128, (HT // 2) * 128], BF16)
    dmlp = dram_in("dmlp", [HT, 128, KK * 128], BF16)
    dwk_in = dram_in("dwk", [HID, KK])
    out = nc.declare_dram_parameter("out", [C, VC], F32, isOutput=True)
    if DEBUG:
        dbg_mod = nc.declare_dram_parameter("dbg_mod", [C, 1], F32, isOutput=True)
        dbg_kern = nc.declare_dram_parameter("dbg_kern", [C, KK], F32, isOutput=True)
        dbg_dyn = nc.declare_dram_parameter("dbg_dyn", [C, V5], F32, isOutput=True)
        dbg_xb = nc.declare_dram_parameter("dbg_xb", [C, V5], F32, isOutput=True)
        dbg_xln = nc.declare_dram_parameter("dbg_xln", [C, V5], F32, isOutput=True)
        dbg_gate = nc.declare_dram_parameter("dbg_gate", [128, VC], F32, isOutput=True)
        dbg_h = nc.declare_dram_parameter("dbg_h", [128, HPN], F32, isOutput=True)
        dbg_c1 = nc.declare_dram_parameter("dbg_c1", [128, VC], F32, isOutput=True)
        dbg_y = nc.declare_dram_parameter("dbg_y", [C, VC], F32, isOutput=True)

    with tile.TileContext(nc) as tc, ExitStack() as ctx:
        persist = ctx.enter_context(tc.tile_pool(name="persist", bufs=1))
        dram = ctx.enter_context(tc.tile_pool(name="dram", bufs=1, space="DRAM"))
        xbpool = ctx.enter_context(tc.tile_pool(name="xbpool", bufs=1))
        ytpool = ctx.enter_context(tc.tile_pool(name="ytpool", bufs=1))

        psA_cm = tc.tile_pool(name="psA", bufs=1, space="PSUM")
        psA = psA_cm.__enter__()
        gpool_cm = tc.tile_pool(name="gemv", bufs=2)
        gpool = gpool_cm.__enter__()
        xpool_cm = tc.tile_pool(name="xpool", bufs=1)
        xpool = xpool_cm.__enter__()

        gwpool_cm = tc.tile_pool(name="gwpool", bufs=2)
        gwpool = gwpool_cm.__enter__()
        w1pool_cm = tc.tile_pool(name="w1pool", bufs=1)
        w1pool = w1pool_cm.__enter__()

        # ------------ phase A: vc partial sums + AR1 (issue-first) ------------
        xvc = [w1pool.tile([128, VC], BF16, name=f"xvc{i}", tag=f"xvc{i}")
               for i in range(CT)]
        for i in range(CT):
            nc.sync.dma_start(xvc[i][:], xown_in[128 * i:128 * (i + 1), :])
        vcs = persist.tile([128, CT], F32, name="vcs", tag="vcs")
        for i in range(CT):
            nc.vector.tensor_reduce(vcs[:, i:i + 1], xvc[i][:],
                                    axis=mybir.AxisListType.X, op=ADD)
        ar1_in = dram.tile([C], F32, name="ar1i", tag="ar1i")
        ar1_out = dram.tile([C], F32, name="ar1o", tag="ar1o",
                            addr_space="Shared")
        nc.sync.dma_start(
            bass.AP(tensor=ar1_in[:].tensor, offset=ar1_in[:].offset,
                    ap=[[1, 128], [128, CT]]), vcs[:])
        nc.gpsimd.collective_compute(
            "AllReduce", ADD, replica_groups=[list(range(NCORES))],
            ins=[ar1_in[:]], outs=[ar1_out[:]])
        sm = [persist.tile([128, 16], F32, name=f"sm{i}", tag=f"sm{i}")
              for i in range(CT)]
        for i in range(CT):
            nc.sync.dma_start(sm[i][:], smalls[128 * i:128 * (i + 1), :])
        xp = [xpool.tile([128, XPN], BF16, name=f"xp{i}", tag=f"xp{i}")
              for i in range(CT)]
        x4 = [t.rearrange("p (z y x) -> p z y x", z=Z7, y=26, x=26) for t in xp]
        id_t = persist.tile([128, 128], BF16, name="identt", tag="identt")
        nc.sync.dma_start(id_t[:], ident[:, :])
        ones_t = persist.tile([128, 1], BF16, name="onest", tag="onest")
        nc.sync.dma_start(ones_t[:], onesc[:, :])
        onesr_t = persist.tile([1, 128], BF16, name="onesrt", tag="onesrt")
        nc.sync.dma_start(onesr_t[:], onesr_in[:, :])
        eps_t = persist.tile([128, 1], F32, name="epst", tag="epst")
        nc.vector.memset(eps_t[:], EPS)
        junk = xpool.tile([128, VC], BF16, name="junk", tag="junk")
        junk5 = junk.rearrange("p (z y x) -> p z y x", z=ZP, y=24, x=24)
        opT = persist.tile([128, CT * C], BF16, name="opT", tag="opT")
        nc.sync.dma_start(
            opT[:], bass.AP(tensor=gemvW, offset=5 * C * C,
                            ap=[[C, 128], [128 * C, CT], [1, C]]))
        gi_all = persist.tile([128, CT * G], F32, name="giall", tag="giall")
        nc.sync.dma_start(
            gi_all[:], bass.AP(tensor=gind_in, offset=0,
                               ap=[[G, 128], [128 * G, CT], [1, G]]))
        knb1_t = persist.tile([128, 3], F32, name="knb1t", tag="knb1t")
        nc.sync.dma_start(
            knb1_t[:], bass.AP(tensor=knb1, offset=0, ap=[[1, 128], [128, 3]]))
        kb2 = [persist.tile([128, KK], F32, name=f"kb2{i}", tag=f"kb2{i}")
               for i in range(CT)]
        for i in range(CT):
            nc.sync.dma_start(kb2[i][:], knb2t[128 * i:128 * (i + 1), :])
        dwk_map = {}
        for t in range(HT):
            if t not in MLP_PE:
                d = persist.tile([128, KK], F32, name=f"dwk{t}", tag=f"dwk{t}")
                nc.sync.dma_start(d[:], dwk_in[128 * t:128 * (t + 1), :])
                dwk_map[t] = d



        def load_gwbuf(dram_t, off):
            t = gwpool.tile([128, CT * C], BF16, name="gwbuf", tag="gwbuf")
            nc.scalar.dma_start(
                t[:], bass.AP(tensor=dram_t, offset=off,
                              ap=[[C, 128], [128 * C, CT], [1, C]]))
            return t

        w1t = [w1pool.tile([128, CT * W1R], BF16, name=f"w1t{h}",
                           tag=f"w1t{h}") for h in range(2)]
        for h in range(2):
            nc.scalar.dma_start(
                w1t[h][:], bass.AP(tensor=w1sT, offset=h * C * W1R,
                                   ap=[[W1R, 128], [128 * W1R, CT], [1, W1R]]))


        # ------------ phase B: text gemv chain (overlaps AR1) ------------
        def gemv(wt, in_cols, nm, act, bias_cols, tag, scale=1.0, odt=BF16):
            outs = []
            for m in range(nm):
                ps = psA.tile([128, 1], F32, name="ps_small", tag="ps_small")
                for k in range(CT):
                    nc.tensor.matmul(
                        ps[:], wt[:, C * k + 128 * m:C * k + 128 * m + 128],
                        in_cols[k][:], start=(k == 0), stop=(k == CT - 1))
                o = gpool.tile([128, 1], odt, name=f"{tag}o{m}",
                               tag=f"{tag}o{m}")
                bias = bias_cols[m] if bias_cols is not None else 0.0
                nc.scalar.activation(o[:], ps[:], act, bias=bias, scale=scale)
                outs.append(o)
            return outs

        t_cols = []
        for i in range(CT):
            t = gpool.tile([128, 1], BF16, name=f"tc{i}", tag=f"tc{i}")
            nc.scalar.activation(t[:], sm[i][:, S_Y:S_Y + 1], Copy)
            t_cols.append(t)
        h1 = gemv(load_gwbuf(gemvW, 0 * C * C), t_cols, CT, Relu,
                  [sm[i][:, S_LB1:S_LB1 + 1] for i in range(CT)], "lw1")
        h2 = gemv(load_gwbuf(gemvW, 1 * C * C), h1, CT, Iden,
                  [sm[i][:, S_LB2:S_LB2 + 1] for i in range(CT)], "lw2")
        tp = gemv(load_gwbuf(gemvW, 2 * C * C), h2, CT, Iden,
                  [sm[i][:, S_TPB:S_TPB + 1] for i in range(CT)], "tpw")
        av = gemv(load_gwbuf(gemvW, 3 * C * C), tp, CT, Iden,
                  [sm[i][:, S_ABV:S_ABV + 1] for i in range(CT)], "avw")
        attn = gemv(load_gwbuf(gemvW, 4 * C * C), av, CT, Iden,
                    [sm[i][:, S_ABO:S_ABO + 1] for i in range(CT)], "aow")
        mw = [load_gwbuf(modWT, h * C * C) for h in range(2)]

        # ------------ phase C: post-AR1 gemvs ------------
        vc_cols = []
        for i in range(CT):
            col = gpool.tile([128, 1], F32, name=f"vcc{i}", tag=f"vcc{i}")
            nc.sync.dma_start(
                col[:], bass.AP(tensor=ar1_out[:].tensor,
                                offset=ar1_out[:].offset + 128 * i,
                                ap=[[1, 128], [128, 1]]))
            cb = gpool.tile([128, 1], BF16, name=f"cmb{i}", tag=f"cmb{i}")
            nc.scalar.activation(cb[:], col[:], Copy, scale=1.0 / NTOT)
            vc_cols.append(cb)
        halves = [vc_cols, attn]

        def gemv2h(wts, stride, nm, act, bias_cols, tag, odt=F32):
            outs = []
            for m in range(nm):
                ps = psA.tile([128, 1], F32, name="ps_small", tag="ps_small")
                for h in range(2):
                    for k in range(CT):
                        nc.tensor.matmul(
                            ps[:], wts[h][:, k * stride + 128 * m:
                                          k * stride + 128 * m + 128],
                            halves[h][k][:],
                            start=(h == 0 and k == 0),
                            stop=(h == 1 and k == CT - 1))
                o = gpool.tile([128, 1], odt, name=f"{tag}o{m}",
                               tag=f"{tag}o{m}")
                nc.scalar.activation(o[:], ps[:], act, bias=bias_cols[m])
                outs.append(o)
            return outs

        mod = gemv2h(mw, C, CT, Sigmoid,
                     [sm[i][:, S_MODB:S_MODB + 1] for i in range(CT)], "modw")
        kp1 = gemv2h(w1t, W1R, 3, Relu,
                     [knb1_t[:, m:m + 1] for m in range(3)], "w1s", odt=BF16)

        w1pool_cm.__exit__(None, None, None)
        gwpool_cm.__exit__(None, None, None)

        # ------------ phase D: kernel_net gemv2 + AR2 ------------
        ko_all = persist.tile([128, KK * CT], F32, name="koall", tag="koall")
        with (tc.tile_pool(name="kseq", bufs=13) as kseq,
              tc.tile_pool(name="kps", bufs=1, space="PSUM") as kps):
            for i in (0, 4, 5):
                nc.sync.dma_start(xp[i][:], xpad_in[128 * i:128 * (i + 1), :])
            kop = kps.tile([128, KK * CT], F32, name="g2ps", tag="g2ps")
            for t in range(KK):
                wt = kseq.tile([128, 3 * C], BF16, name="w2t", tag="w2t")
                nc.gpsimd.dma_start(
                    wt[:], bass.AP(tensor=w2sT, offset=t * C,
                                   ap=[[KFLAT, 128], [128 * KFLAT, 3],
                                       [1, C]]))
                for m in range(CT):
                    for k in range(3):
                        nc.tensor.matmul(
                            kop[:, CT * t + m:CT * t + m + 1],
                            wt[:, k * C + 128 * m:k * C + 128 * m + 128],
                            kp1[k][:], start=(k == 0), stop=(k == 2))
            nc.scalar.activation(ko_all[:], kop[:], Copy)
            for i in (1, 2, 3):
                nc.gpsimd.dma_start(xp[i][:], xpad_in[128 * i:128 * (i + 1), :])
        ar2_in = dram.tile([128, KK * CT], F32, name="ar2i", tag="ar2i")
        ar2_out = dram.tile([128, KK * CT], F32, name="ar2o", tag="ar2o",
                            addr_space="Shared")
        nc.sync.dma_start(ar2_in[:, :], ko_all[:])
        nc.gpsimd.collective_compute(
            "AllReduce", ADD, replica_groups=[list(range(NCORES))],
            ins=[ar2_in[:]], outs=[ar2_out[:]])
        kraw = persist.tile([128, KK * CT], F32, name="kraw", tag="kraw")
        nc.sync.dma_start(kraw[:], ar2_out[:, :])
        kraw3 = kraw.rearrange("p (t i) -> p t i", t=KK, i=CT)


        # kernels: +bias, fold mod
        kernm = []
        for i in range(CT):
            km = persist.tile([128, KK], F32, name=f"kernm{i}", tag=f"kernm{i}")
            nc.vector.tensor_tensor(km[:], kraw3[:, :, i], kb2[i][:], op=ADD)
            nc.vector.tensor_scalar_mul(km[:], km[:], mod[i][:, 0:1])
            kernm.append(km)

        if DEBUG:
            for i in range(CT):
                nc.gpsimd.dma_start(dbg_mod[128 * i:128 * (i + 1), :], mod[i][:])
                nc.gpsimd.dma_start(dbg_kern[128 * i:128 * (i + 1), :], kernm[i][:])

        # ------------ phase E: dynamic depthwise conv + GN stats ------------
        dynpool_cm = tc.tile_pool(name="dynpool", bufs=1)
        dynpool = dynpool_cm.__enter__()
        dyn = [dynpool.tile([128, V5], BF16, name=f"dyn{i}", tag=f"dyn{i}")
               for i in range(CT)]
        dyn4 = [t.rearrange("p (z y x) -> p z y x", z=Z5, y=24, x=24)
                for t in dyn]

        dgpool_cm = tc.tile_pool(name="dgpool", bufs=2)
        dgpool = dgpool_cm.__enter__()

        def build_diag(i):
            d = dgpool.tile([128, KK * 128], BF16, name="dg", tag="dg")
            for t in range(KK):
                nc.vector.tensor_scalar_mul(d[:, 128 * t:128 * (t + 1)],
                                            id_t[:], kernm[i][:, t:t + 1])
            return d

        def dyn_pe_planes(i, dgt, zos, cpool):
            for zo in zos:
                for half in range(2):
                    y0 = 12 * half
                    cp = cpool.tile([128, 288], F32, name=f"dcp{half}",
                                    tag=f"dcp{half}")
                    for ti, (dz, dy, dx) in enumerate(TAPS):
                        nc.tensor.matmul(
                            cp[:], dgt[:, 128 * ti:128 * (ti + 1)],
                            x4[i][:, zo + 1 + dz, 1 + y0 + dy:13 + y0 + dy,
                                  1 + dx:25 + dx],
                            start=(ti == 0), stop=(ti == KK - 1))
                    nc.scalar.activation(
                        dyn4[i][:, zo, y0:y0 + 12, :], cp[:], Copy)

        def dyn_dve_planes(i, zo0, nz, tpool):
            dst = dyn4[i][:, zo0:zo0 + nz, :, :]
            for ti, (dz, dy, dx) in enumerate(TAPS):
                src = x4[i][:, zo0 + 1 + dz:zo0 + 1 + dz + nz,
                            1 + dy:25 + dy, 1 + dx:25 + dx]
                if ti == 0:
                    nc.vector.tensor_scalar_mul(dst, src, kernm[i][:, 0:1])
                else:
                    tmp = tpool.tile([128, 3 * PL], BF16, name="dtmp",
                                     tag="dtmp")
                    t4 = tmp.rearrange("p (z y x) -> p z y x", z=3, y=24,
                                       x=24)[:, 0:nz, :, :]
                    nc.vector.tensor_scalar_mul(t4, src,
                                                kernm[i][:, ti:ti + 1])
                    nc.vector.tensor_tensor(dst, dst, t4, op=ADD)

        gst = persist.tile([128, 2 * CT], F32, name="gst", tag="gst")
        ar3_in = dram.tile([G, 2], F32, name="ar3i", tag="ar3i")
        ar3_out = dram.tile([G, 2], F32, name="ar3o", tag="ar3o",
                            addr_space="Shared")
        gsb = persist.tile([G, 2], F32, name="gsb", tag="gsb")
        with (tc.tile_pool(name="dcpool", bufs=1, space="PSUM") as dcpool,
              tc.tile_pool(name="dtpool", bufs=2) as dtpool,
              tc.tile_pool(name="gnps", bufs=1, space="PSUM") as gnps):
            # own planes (zo 1..3) first, stats, then halo planes under AR3
            dg_live = {}
            for i in range(CT):
                if i in DYN_PE:
                    dg_live[i] = build_diag(i)
                    dyn_pe_planes(i, dg_live[i], (1, 2, 3), dcpool)
                else:
                    dyn_dve_planes(i, 1, 3, dtpool)
                nc.scalar.activation(junk[:], dyn[i][:, PL:4 * PL],
                                     Copy, accum_out=gst[:, 2 * i:2 * i + 1])
                nc.scalar.activation(junk[:], dyn[i][:, PL:4 * PL],
                                     Square,
                                     accum_out=gst[:, 2 * i + 1:2 * i + 2])
            gps = gnps.tile([G, 2], F32, name="gps", tag="gps")
            for i in range(CT):
                nc.tensor.matmul(gps[:], gi_all[:, G * i:G * (i + 1)],
                                 gst[:, 2 * i:2 * i + 2], start=(i == 0),
                                 stop=(i == CT - 1))
            nc.scalar.activation(gsb[:], gps[:], Copy)
            nc.sync.dma_start(ar3_in[:, :], gsb[:])
            nc.gpsimd.collective_compute(
                "AllReduce", ADD, replica_groups=[list(range(NCORES))],
                ins=[ar3_in[:]], outs=[ar3_out[:]])
            for i in range(CT):
                if i in DYN_PE:
                    dg2 = build_diag(i)
                    dyn_pe_planes(i, dg2, (0, 4), dcpool)
                else:
                    dyn_dve_planes(i, 0, 1, dtpool)
                    dyn_dve_planes(i, 4, 1, dtpool)
        dgpool_cm.__exit__(None, None, None)

        if DEBUG:
            for i in range(CT):
                nc.gpsimd.dma_start(dbg_dyn[128 * i:128 * (i + 1), :], dyn[i][:])

        # ------------ GN scale/shift + fold into opT ------------
        gstat = persist.tile([G, 2], F32, name="gstat", tag="gstat")
        nc.sync.dma_start(gstat[:], ar3_out[:, :])
        NGRP = float(GD) * NTOT
        gmr = persist.tile([G, 2], F32, name="gmr", tag="gmr")
        nc.scalar.activation(gmr[:, 0:1], gstat[:, 0:1], Copy, scale=1.0 / NGRP)
        musq = persist.tile([G, 1], F32, name="musq", tag="musq")
        nc.scalar.square(musq[:], gmr[:, 0:1])
        var = persist.tile([G, 1], F32, name="gvar", tag="gvar")
        nc.vector.tensor_scalar(var[:], gstat[:, 1:2], 1.0 / NGRP, None,
                                op0=MULT)
        nc.vector.tensor_sub(var[:], var[:], musq[:])
        nc.scalar.activation(var[:], var[:], Sqrt, bias=eps_t[0:G, 0:1])
        nc.vector.reciprocal(gmr[:, 1:2], var[:])

        cafm_shift = []
        gsc = []
        for i in range(CT):
            ge = gpool.tile([G, 128], F32, name=f"gexp{i}", tag=f"gexp{i}")
            nc.sync.dma_start(ge[:], gexpT[:, 128 * i:128 * (i + 1)])
            ps = psA.tile([128, 2], F32, name="ps_sm2", tag="ps_sm2")
            nc.tensor.matmul(ps[:], ge[:], gmr[:], start=True, stop=True)
            mu_c = persist.tile([128, 2], F32, name=f"muc{i}", tag=f"muc{i}")
            nc.scalar.activation(mu_c[:], ps[:], Copy)
            a = persist.tile([128, 1], F32, name=f"gsc{i}", tag=f"gsc{i}")
            nc.vector.tensor_mul(a[:], sm[i][:, S_GNG:S_GNG + 1], mu_c[:, 1:2])
            b = persist.tile([128, 1], F32, name=f"gsh{i}", tag=f"gsh{i}")
            nc.vector.tensor_mul(b[:], mu_c[:, 0:1], a[:])
            nc.vector.tensor_sub(b[:], sm[i][:, S_GNB:S_GNB + 1], b[:])
            gsc.append(a)
            bb = gpool.tile([128, 1], BF16, name=f"gshb{i}", tag=f"gshb{i}")
            nc.scalar.activation(bb[:], b[:], Copy)
            cafm_shift.append(bb)
        cb_cols = []
        for m in range(CT):
            ps = psA.tile([128, 1], F32, name="ps_small", tag="ps_small")
            for k in range(CT):
                nc.tensor.matmul(ps[:], opT[:, 768 * k + 128 * m:
                                            768 * k + 128 * m + 128],
                                 cafm_shift[k][:], start=(k == 0),
                                 stop=(k == CT - 1))
            o = persist.tile([128, 1], F32, name=f"cbc{m}", tag=f"cbc{m}")
            nc.scalar.activation(o[:], ps[:], Iden,
                                 bias=sm[m][:, S_OPB:S_OPB + 1])
            cb_cols.append(o)
        for k in range(CT):
            nc.vector.tensor_scalar_mul(opT[:, 768 * k:768 * (k + 1)],
                                        opT[:, 768 * k:768 * (k + 1)],
                                        gsc[k][:])

        # ------------ phase F1: cafm matmul + xb ------------
        xb = [xbpool.tile([128, V5], BF16, name=f"xb{m}", tag=f"xb{m}")
              for m in range(CT)]
        xb4 = [t.rearrange("p (z y x) -> p z y x", z=Z5, y=24, x=24)
               for t in xb]
        CH6 = [(o, min(512, V5 - o)) for o in range(0, V5, 512)]
        with (tc.tile_pool(name="f1ps", bufs=2, space="PSUM") as f1ps,
              tc.tile_pool(name="f1t", bufs=2) as f1t):
            for m in range(CT):
                tca = f1t.tile([128, V5], BF16, name="tcafm", tag="tcafm")
                for o, n in CH6:
                    ps = f1ps.tile([128, 512], F32, name="f1p", tag="f1p")
                    for k in range(CT):
                        nc.tensor.matmul(ps[:, 0:n],
                                         opT[:, 768 * k + 128 * m:
                                             768 * k + 128 * m + 128],
                                         dyn[k][:, o:o + n], start=(k == 0),
                                         stop=(k == CT - 1))
                    nc.scalar.activation(tca[:, o:o + n], ps[:, 0:n], Iden,
                                         bias=cb_cols[m][:])
                t4 = tca.rearrange("p (z y x) -> p z y x", z=Z5, y=24, x=24)
                nc.vector.tensor_tensor(xb4[m][:, :, :, :], t4[:, :, :, :],
                                        x4[m][:, 1:6, 1:25, 1:25], op=MULT)
        if DEBUG:
            for m in range(CT):
                nc.gpsimd.dma_start(dbg_xb[128 * m:128 * (m + 1), :], xb[m][:])
        dynpool_cm.__exit__(None, None, None)
        xpool_cm.__exit__(None, None, None)
        gpool_cm.__exit__(None, None, None)
        psA_cm.__exit__(None, None, None)

        # ------------ LN helper ------------
        def ln_stats(tiles, nv, tag, mupool):
            """Per-voxel mean/rstd over channels -> [128, nv] bf16 tiles.
            ones_t carries 1/C so the matmuls produce mean / E[x^2]."""
            nch = [(o, min(480, nv - o)) for o in range(0, nv, 480)]
            muB = mupool.tile([128, nv], BF16, name=f"{tag}muB",
                              tag=f"{tag}muB")
            rsB = mupool.tile([128, nv], BF16, name=f"{tag}rsB",
                              tag=f"{tag}rsB")
            with (tc.tile_pool(name=f"{tag}ps", bufs=1, space="PSUM") as lps,
                  tc.tile_pool(name=f"{tag}sq", bufs=2) as sqp,
                  tc.tile_pool(name=f"{tag}rw", bufs=2) as rwp):
                for o, n in nch:
                    p1 = lps.tile([1, 480], F32, name="s1", tag="s1")
                    p2 = lps.tile([1, 480], F32, name="s2", tag="s2")
                    for k in range(CT):
                        nc.tensor.matmul(p1[:, 0:n], ones_t[:],
                                         tiles[k][:, o:o + n],
                                         start=(k == 0), stop=(k == CT - 1))
                    for k in range(CT):
                        q = sqp.tile([128, 480], BF16, name="sqc", tag="sqc")
                        if k % 2 == 0:
                            nc.scalar.activation(q[:, 0:n],
                                                 tiles[k][:, o:o + n], Square)
                        else:
                            nc.vector.tensor_mul(q[:, 0:n],
                                                 tiles[k][:, o:o + n],
                                                 tiles[k][:, o:o + n])
                        nc.tensor.matmul(p2[:, 0:n], ones_t[:], q[:, 0:n],
                                         start=(k == 0), stop=(k == CT - 1))
                    mubf = rwp.tile([1, 480], BF16, name="mubf", tag="mubf")
                    nc.scalar.activation(mubf[:, 0:n], p1[:, 0:n], Copy)
                    m2 = rwp.tile([1, 480], F32, name="m2", tag="m2")
                    nc.scalar.activation(m2[:, 0:n], p1[:, 0:n], Square)
                    vr = rwp.tile([1, 480], F32, name="vr", tag="vr")
                    nc.vector.tensor_sub(vr[:, 0:n], p2[:, 0:n], m2[:, 0:n])
                    nc.scalar.activation(vr[:, 0:n], vr[:, 0:n], Sqrt,
                                         bias=eps_t[0:1, 0:1])
                    rbf = rwp.tile([1, 480], BF16, name="rbf", tag="rbf")
                    with nc.allow_low_precision(reason="rstd bcast in bf16"):
                        nc.vector.reciprocal(rbf[:, 0:n], vr[:, 0:n])
                    pb = lps.tile([128, 480], F32, name="bc", tag="bc")
                    nc.tensor.matmul(pb[:, 0:n], onesr_t[:], mubf[:, 0:n],
                                     start=True, stop=True)
                    nc.scalar.activation(muB[:, o:o + n], pb[:, 0:n], Copy)
                    pb2 = lps.tile([128, 480], F32, name="bc2", tag="bc2")
                    nc.tensor.matmul(pb2[:, 0:n], onesr_t[:], rbf[:, 0:n],
                                     start=True, stop=True)
                    nc.scalar.activation(rsB[:, o:o + n], pb2[:, 0:n], Copy)
            return muB, rsB

        # ------------ LN2 + edge masks ------------
        gatepool_cm = tc.tile_pool(name="gatepool", bufs=1)
        gatepool = gatepool_cm.__enter__()
        xlnpool_cm = tc.tile_pool(name="xlnpool", bufs=1)
        xlnpool = xlnpool_cm.__enter__()
        xln = [xlnpool.tile([128, V5], BF16, name=f"xln{k}", tag=f"xln{k}")
               for k in range(CT)]
        mupool_cm = tc.tile_pool(name="mupool", bufs=1)
        mupool = mupool_cm.__enter__()
        muB, rsB = ln_stats(xb, V5, "ln2", mupool)
        with tc.tile_pool(name="lnap", bufs=2) as lnap:
            for k in range(CT):
                t1 = lnap.tile([128, V5], BF16, name="lnt1", tag="lnt1")
                nc.vector.tensor_sub(t1[:], xb[k][:], muB[:])
                nc.vector.tensor_mul(t1[:], t1[:], rsB[:])
                nc.vector.tensor_scalar(xln[k][:], t1[:],
                                        sm[k][:, S_N2W:S_N2W + 1],
                                        sm[k][:, S_N2B:S_N2B + 1],
                                        op0=MULT, op1=ADD)
                nc.vector.tensor_scalar_mul(xln[k][:, 0:PL], xln[k][:, 0:PL],
                                            sm[k][:, S_MLO:S_MLO + 1])
                nc.vector.tensor_scalar_mul(xln[k][:, 4 * PL:5 * PL],
                                            xln[k][:, 4 * PL:5 * PL],
                                            sm[k][:, S_MHI:S_MHI + 1])

        # ------------ MLP: Wi + depthwise conv + gate ------------
        if DEBUG:
            for k in range(CT):
                nc.gpsimd.dma_start(dbg_xln[128 * k:128 * (k + 1), :], xln[k][:])
        mupool_cm.__exit__(None, None, None)
        gate = [gatepool.tile([128, VC], BF16, name=f"gate{j}", tag=f"gate{j}")
                for j in range(HT // 2)]
        hppool_cm = tc.tile_pool(name="hppool", bufs=1)
        hppool = hppool_cm.__enter__()
        hpads = [hppool.tile([128, HPN], BF16, name=f"hpad{b}", tag=f"hpad{b}")
                 for b in range(4)]
        for b in range(4):
            nc.vector.memset(hpads[b][:], 0.0)
        hp4s = [t.rearrange("p (z y x) -> p z y x", z=Z5, y=26, x=26)
                for t in hpads]

        with (tc.tile_pool(name="wiw", bufs=2) as wiw,
              tc.tile_pool(name="dga", bufs=2) as dgap,
              tc.tile_pool(name="wips", bufs=2, space="PSUM") as wips,
              tc.tile_pool(name="cvps", bufs=1, space="PSUM") as cvps,
              tc.tile_pool(name="mlpt", bufs=1) as mlpt):

            def wi_pass(t, bi):
                hp4 = hp4s[bi]
                wall = wiw.tile([128, CT * 128], BF16, name="wiall",
                                tag="wiall")
                nc.gpsimd.dma_start(wall[:], wiT[t, :, :])
                for z in range(Z5):
                    for halfy in range(2):
                        y0 = 12 * halfy
                        pz = wips.tile([128, 288], F32, name=f"wip{halfy}",
                                       tag=f"wip{halfy}")
                        for k in range(CT):
                            nc.tensor.matmul(
                                pz[:],
                                wall[:, 128 * k:128 * (k + 1)],
                                xln[k][:, z * PL + y0 * 24:
                                       z * PL + y0 * 24 + 288],
                                start=(k == 0), stop=(k == CT - 1))
                        nc.scalar.activation(
                            hp4[:, z, 1 + y0:13 + y0, 1:25],
                            pz.rearrange("p (y x) -> p y x", y=12, x=24)[
                                :, :, :], Copy)
                if DEBUG and t == 0:
                    nc.gpsimd.dma_start(dbg_h[:, :], hpads[bi][:])

            PTAPS = 7

            def conv_pass(t, bi, dst, act):
                hp4 = hp4s[bi]
                if t in MLP_PE:
                    if dst is None:
                        dst = mlpt.tile([128, VC], BF16, name="conv2",
                                        tag="conv2")
                    dgt = dgap.tile([128, KK * 128], BF16, name="dgall",
                                    tag="dgall")
                    nc.gpsimd.dma_start(dgt[:], dmlp[t, :, :])
                    for zo in range(3):
                        for halfy in range(2):
                            b = 2 * zo + halfy
                            y0 = 12 * halfy
                            cp = cvps.tile([128, 288], F32,
                                           name=f"mcp{b % 2}",
                                           tag=f"mcp{b % 2}")
                            for ti, (dz, dy, dx) in enumerate(TAPS):
                                nc.tensor.matmul(
                                    cp[:], dgt[:, 128 * ti:128 * (ti + 1)],
                                    hp4[:, 1 + zo + dz,
                                        1 + y0 + dy:13 + y0 + dy,
                                        1 + dx:25 + dx],
                                    start=(ti == 0), stop=(ti == KK - 1))
                            nc.scalar.activation(
                                dst[:, 288 * b:288 * (b + 1)], cp[:], act)
                    return dst
                # PE computes the first PTAPS taps into a bf16 partial
                dgp = dgap.tile([128, PTAPS * 128], BF16, name="dgpart",
                                tag="dgpart")
                nc.gpsimd.dma_start(
                    dgp[:], bass.AP(tensor=dmlp, offset=t * 128 * KK * 128,
                                    ap=[[KK * 128, 128], [1, PTAPS * 128]]))
                pcv = mlpt.tile([128, VC], BF16, name="pconv", tag="pconv")
                for zo in range(3):
                    for halfy in range(2):
                        b = 2 * zo + halfy
                        y0 = 12 * halfy
                        cp = cvps.tile([128, 288], F32, name=f"mcp{b % 2}",
                                       tag=f"mcp{b % 2}")
                        for ti in range(PTAPS):
                            dz, dy, dx = TAPS[ti]
                            nc.tensor.matmul(
                                cp[:], dgp[:, 128 * ti:128 * (ti + 1)],
                                hp4[:, 1 + zo + dz, 1 + y0 + dy:13 + y0 + dy,
                                    1 + dx:25 + dx],
                                start=(ti == 0), stop=(ti == PTAPS - 1))
                        nc.scalar.activation(pcv[:, 288 * b:288 * (b + 1)],
                                             cp[:], Copy)
                acc = mlpt.tile([128, VC], BF16, name="macc", tag="macc")
                a4 = acc.rearrange("p (z y x) -> p z y x", z=3, y=24, x=24)
                kw = dwk_map[t]
                for ti in range(PTAPS, KK):
                    dz, dy, dx = TAPS[ti]
                    src = hp4[:, 1 + dz:4 + dz, 1 + dy:25 + dy, 1 + dx:25 + dx]
                    if ti == PTAPS:
                        nc.vector.tensor_scalar_mul(a4[:, :, :, :], src,
                                                    kw[:, ti:ti + 1])
                    else:
                        tmp = mlpt.tile([128, VC], BF16, name="mtmp",
                                        tag="mtmp")
                        t4 = tmp.rearrange("p (z y x) -> p z y x", z=3, y=24,
                                           x=24)
                        nc.vector.tensor_scalar_mul(t4[:, :, :, :], src,
                                                    kw[:, ti:ti + 1])
                        nc.vector.tensor_tensor(a4[:, :, :, :], a4[:, :, :, :],
                                                t4[:, :, :, :], op=ADD)
                nc.vector.tensor_tensor(acc[:], acc[:], pcv[:], op=ADD)
                if act is Gelu:
                    nc.scalar.activation(dst[:], acc[:], Gelu)
                    return dst
                return acc

            for j in range(HT // 2):
                b1, b2 = (2 * j) % 4, (2 * j + 1) % 4
                wi_pass(j, b1)
                wi_pass(j + HT // 2, b2)
                conv_pass(j, b1, gate[j], Gelu)
                if DEBUG and j == 0:
                    nc.gpsimd.dma_start(dbg_c1[:, :], gate[0][:])
                c2 = conv_pass(j + HT // 2, b2, None, Copy)
                nc.vector.tensor_mul(gate[j][:], gate[j][:], c2[:])
        if DEBUG:
            nc.gpsimd.dma_start(dbg_gate[:, :], gate[0][:])
        hppool_cm.__exit__(None, None, None)
        xlnpool_cm.__exit__(None, None, None)

        # ------------ Wo + residual + LN3 (chunk-major, pipelined) ------------
        y_t = [ytpool.tile([128, VC], BF16, name=f"y{m}", tag=f"y{m}")
               for m in range(CT)]
        CH3 = [(0, 480), (480, 480), (960, 480), (1440, 288)]
        with (tc.tile_pool(name="wow", bufs=1) as wow,
              tc.tile_pool(name="wops", bufs=2, space="PSUM") as wops,
              tc.tile_pool(name="ln3ps", bufs=1, space="PSUM") as l3ps,
              tc.tile_pool(name="ln3sq", bufs=2) as l3sq,
              tc.tile_pool(name="ln3rw", bufs=2) as l3rw,
              tc.tile_pool(name="mupool3", bufs=1) as mupool3,
              tc.tile_pool(name="lnap3", bufs=2) as lnap3):
            walls = [wow.tile([128, (HT // 2) * 128], BF16, name=f"wo{m}",
                              tag=f"wo{m}") for m in range(CT)]
            for m in range(CT):
                nc.gpsimd.dma_start(walls[m][:], woT[m, :, :])
            muB3 = mupool3.tile([128, VC], BF16, name="ln3muB", tag="ln3muB")
            rsB3 = mupool3.tile([128, VC], BF16, name="ln3rsB", tag="ln3rsB")
            for o, n in CH3:
                for m in range(CT):
                    ps = wops.tile([128, 512], F32, name="wo_ps", tag="wo_ps")
                    for k in range(HT // 2):
                        nc.tensor.matmul(ps[:, 0:n],
                                         walls[m][:, 128 * k:128 * (k + 1)],
                                         gate[k][:, o:o + n],
                                         start=(k == 0), stop=False)
                    nc.tensor.matmul(ps[:, 0:n], id_t[:],
                                     xb[m][:, PL + o:PL + o + n],
                                     start=False, stop=True)
                    nc.scalar.activation(y_t[m][:, o:o + n], ps[:, 0:n], Copy)
                # LN3 stats for this chunk
                p1 = l3ps.tile([1, 480], F32, name="l3s1", tag="l3s1")
                p2 = l3ps.tile([1, 480], F32, name="l3s2", tag="l3s2")
                for k in range(CT):
                    nc.tensor.matmul(p1[:, 0:n], ones_t[:], y_t[k][:, o:o + n],
                                     start=(k == 0), stop=(k == CT - 1))
                for k in range(CT):
                    q = l3sq.tile([128, 480], BF16, name="l3q", tag="l3q")
                    nc.vector.tensor_mul(q[:, 0:n], y_t[k][:, o:o + n],
                                         y_t[k][:, o:o + n])
                    nc.tensor.matmul(p2[:, 0:n], ones_t[:], q[:, 0:n],
                                     start=(k == 0), stop=(k == CT - 1))
                mubf = l3rw.tile([1, 480], BF16, name="l3mubf", tag="l3mubf")
                nc.scalar.activation(mubf[:, 0:n], p1[:, 0:n], Copy)
                m2 = l3rw.tile([1, 480], F32, name="l3m2", tag="l3m2")
                nc.scalar.activation(m2[:, 0:n], p1[:, 0:n], Square)
                vr = l3rw.tile([1, 480], F32, name="l3vr", tag="l3vr")
                nc.vector.tensor_sub(vr[:, 0:n], p2[:, 0:n], m2[:, 0:n])
                nc.scalar.activation(vr[:, 0:n], vr[:, 0:n], Sqrt,
                                     bias=eps_t[0:1, 0:1])
                rbf = l3rw.tile([1, 480], BF16, name="l3rbf", tag="l3rbf")
                with nc.allow_low_precision(reason="rstd bcast in bf16"):
                    nc.vector.reciprocal(rbf[:, 0:n], vr[:, 0:n])
                pb = l3ps.tile([128, 480], F32, name="l3bc", tag="l3bc")
                nc.tensor.matmul(pb[:, 0:n], onesr_t[:], mubf[:, 0:n],
                                 start=True, stop=True)
                nc.scalar.activation(muB3[:, o:o + n], pb[:, 0:n], Copy)
                pb2 = l3ps.tile([128, 480], F32, name="l3bc2", tag="l3bc2")
                nc.tensor.matmul(pb2[:, 0:n], onesr_t[:], rbf[:, 0:n],
                                 start=True, stop=True)
                nc.scalar.activation(rsB3[:, o:o + n], pb2[:, 0:n], Copy)
                # LN3 apply + store for this chunk
                for k in range(CT):
                    t1 = lnap3.tile([128, 480], BF16, name="ln3t", tag="ln3t")
                    nc.vector.tensor_sub(t1[:, 0:n], y_t[k][:, o:o + n],
                                         muB3[:, o:o + n])
                    nc.vector.tensor_mul(t1[:, 0:n], t1[:, 0:n],
                                         rsB3[:, o:o + n])
                    of = lnap3.tile([128, 480], F32, name="outf", tag="outf")
                    nc.vector.tensor_scalar(of[:, 0:n], t1[:, 0:n],
                                            sm[k][:, S_N3W:S_N3W + 1],
                                            sm[k][:, S_N3B:S_N3B + 1],
                                            op0=MULT, op1=ADD)
                    nc.sync.dma_start(out[128 * k:128 * (k + 1), o:o + n],
                                      of[:, 0:n])
        gatepool_cm.__exit__(None, None, None)

        if DEBUG:
            for m in range(CT):
                nc.gpsimd.dma_start(dbg_y[128 * m:128 * (m + 1), :], y_t[m][:])

    nc.compile()
    return nc


def _prep(inputs):
    bf = lambda a: np.ascontiguousarray(a).astype(BF)
    f32 = lambda a: np.ascontiguousarray(a, dtype=np.float32)
    x = f32(inputs["x"][0]).reshape(C, S, S, S)

    smalls_c = np.zeros((C, 16), np.float32)
    smalls_c[:, 0] = f32(inputs["y"][0, 0])
    for i, k in enumerate(["lora_b1", "lora_b2", "tp_b", "attn_bv", "attn_bo",
                           "op_b", "n2_w", "n2_b", "n3_w", "n3_b", "gn_g",
                           "gn_b", "mod_b"]):
        smalls_c[:, i + 1] = f32(inputs[k])

    gind6 = np.zeros((CT, 128, G), np.float32)
    for j in range(CT):
        for p in range(128):
            gind6[j, p, (128 * j + p) // GD] = 1.0
    gexpT = np.zeros((G, C), np.float32)
    for c in range(C):
        gexpT[c // GD, c] = 1.0

    gemv_stack = np.stack([
        f32(inputs["lora_W1"]).T, f32(inputs["lora_W2"]).T,
        f32(inputs["tp_W"]).T, f32(inputs["attn_Wv"]).T,
        f32(inputs["attn_Wo"]).T, f32(inputs["op_W"]).T])

    kn_W2 = f32(inputs["kn_W2"])
    w2r = kn_W2.reshape(C, KK, HID).transpose(1, 0, 2).reshape(KFLAT, HID)
    w2T = np.ascontiguousarray(w2r.T)          # [HID, KFLAT]
    kn_W1 = f32(inputs["kn_W1"])

    mlp_dw = f32(inputs["mlp_dw"]).reshape(HID, KK)
    dmlp = np.zeros((HT, 128, KK, 128), np.float32)
    idx = np.arange(128)
    for tt in range(HT):
        for ti in range(KK):
            dmlp[tt, idx, ti, idx] = mlp_dw[128 * tt:128 * (tt + 1), ti]
    dmlp = dmlp.reshape(HT, 128, KK * 128)

    com = dict(
        knb2t=f32(inputs["kn_b2"]).reshape(C, KK),
        gind6=gind6, gexpT=gexpT,
        ident=bf(np.eye(128, dtype=np.float32)),
        onesc=bf(np.full((128, 1), 1.0 / C, np.float32)),
        onesr=bf(np.ones((1, 128), np.float32)),
        gemvW=bf(gemv_stack),
        modWT=bf(f32(inputs["mod_W"]).T),
        wiT=bf(f32(inputs["mlp_Wi"]).T.reshape(CT, 128, HT, 128)
               .transpose(2, 1, 0, 3).reshape(HT, 128, CT * 128)),
        woT=bf(f32(inputs["mlp_Wo"]).T.reshape(HT // 2, 128, CT, 128)
               .transpose(2, 1, 0, 3).reshape(CT, 128, (HT // 2) * 128)),
        dmlp=bf(dmlp), dwk=mlp_dw,
    )

    in_maps = []
    for i in range(NCORES):
        z0 = ZP * i
        xh = np.zeros((C, Z7, 26, 26), np.float32)
        lo, hi = max(z0 - 2, 0), min(z0 + ZP + 2, S)
        xh[:, lo - (z0 - 2):lo - (z0 - 2) + (hi - lo), 1:25, 1:25] = \
            x[:, lo:hi]
        smalls = smalls_c.copy()
        smalls[:, S_MLO] = 0.0 if i == 0 else 1.0
        smalls[:, S_MHI] = 0.0 if i == NCORES - 1 else 1.0
        m = dict(com)
        m.update(
            xpad=xh.reshape(C, XPN).astype(BF),
            xown=np.ascontiguousarray(
                x[:, z0:z0 + ZP].reshape(C, VC)).astype(BF),
            smalls=smalls,
            knb1=f32(inputs["kn_b1"][W1R * i:W1R * (i + 1)]),
            w1sT=bf(kn_W1[W1R * i:W1R * (i + 1), :].T),
            w2sT=bf(w2T[W1R * i:W1R * (i + 1), :]),
        )
        in_maps.append(m)
    return in_maps


def kernel(**inputs) -> np.ndarray:
    if "nc" not in _CACHE:
        _CACHE["nc"] = build_program()
    nc = _CACHE["nc"]
    in_maps = _prep(inputs)
    res = run_bass_kernel_spmd(nc, in_maps, list(range(NCORES)))
    outs = [res.results[i]["out"].reshape(C, ZP, PL) for i in range(NCORES)]
    full = np.concatenate(outs, axis=1)
    return full.reshape(1, C, S, S, S).astype(np.float32)


# revision 55
# speedup vs baseline: 2.2143x; 1.0158x over previous
"""Bass/Trainium2 SPMD kernel for nn_Block3D (8 NeuronCores).

Spatial z-shard (24 planes -> 3 per core) with a 2-plane host-prepped halo:
each core computes cafm/xb/xln on 5 planes locally, so the depthwise MLP conv
needs no on-device halo exchange (no AllGather). The per-channel `mod` gate is
folded into the dynamic conv kernels. Depthwise 3x3x3 convs are split between
the PE (diagonal-weight matmuls) and DVE (tensor_scalar 4x + tensor_tensor 2x
chains). All DMAs ride the HWDGE path (nc.sync); kernel_net output is staged
as one [128,162] tile around a single AllReduce.
"""

from contextlib import ExitStack

import numpy as np
import ml_dtypes

import concourse.bass as bass
import concourse.bacc as bacc
import concourse.tile as tile
from concourse import mybir
from concourse.bass_utils import run_bass_kernel_spmd

BF = ml_dtypes.bfloat16
F32 = mybir.dt.float32
BF16 = mybir.dt.bfloat16

C = 768
G = 12
GD = 64
S = 24
HID = 4 * C
KK = 27
EPS = 1e-5
NCORES = 8
ZP = S // NCORES          # 3 own planes
PL = S * S                # 576
Z7 = ZP + 4               # 7 input planes (2-halo each side)
Z5 = ZP + 2               # 5 computed planes (1-halo each side)
PPL = 26 * 26             # 676 padded plane
XPN = Z7 * PPL            # 4732
HPN = Z5 * PPL            # 3380
V5 = Z5 * PL              # 2880
VC = ZP * PL              # 1728
CT = C // 128             # 6
HT = HID // 128           # 24
W1R = HID // NCORES       # 384 hidden rows per core
KFLAT = C * KK
NTOT = float(S * S * S)

TAPS = [(dz, dy, dx) for dz in (-1, 0, 1) for dy in (-1, 0, 1) for dx in (-1, 0, 1)]

# engine split for the depthwise convs (PE diag-matmul vs DVE chains)
DYN_PE = (0, 1, 2, 3)
MLP_PE = tuple([j for j in range(0, 12, 2)] + [j + 12 for j in range(1, 12, 2)])

_CACHE = {}
DEBUG = False

Copy = mybir.ActivationFunctionType.Copy
Iden = mybir.ActivationFunctionType.Identity
Gelu = mybir.ActivationFunctionType.Gelu
Sigmoid = mybir.ActivationFunctionType.Sigmoid
Square = mybir.ActivationFunctionType.Square
Sqrt = mybir.ActivationFunctionType.Sqrt
Relu = mybir.ActivationFunctionType.Relu
ADD = mybir.AluOpType.add
SUB = mybir.AluOpType.subtract
MULT = mybir.AluOpType.mult

(S_Y, S_LB1, S_LB2, S_TPB, S_ABV, S_ABO, S_OPB, S_N2W, S_N2B, S_N3W, S_N3B,
 S_GNG, S_GNB, S_MODB, S_MLO, S_MHI) = range(16)


def build_program():
    nc = bacc.Bacc("TRN2", target_bir_lowering=False)

    def dram_in(name, shape, dtype=F32):
        return nc.declare_dram_parameter(name, list(shape), dtype, isOutput=False)

    xpad_in = dram_in("xpad", [C, XPN], BF16)
    xown_in = dram_in("xown", [C, VC], BF16)
    smalls = dram_in("smalls", [C, 16])
    knb1 = dram_in("knb1", [W1R])
    knb2t = dram_in("knb2t", [C, KK])
    ident = dram_in("ident", [128, 128], BF16)
    onesc = dram_in("onesc", [128, 1], BF16)
    onesr_in = dram_in("onesr", [1, 128], BF16)
    gind_in = dram_in("gind6", [CT, 128, G])
    gexpT = dram_in("gexpT", [G, C])
    gemvW = dram_in("gemvW", [6, C, C], BF16)     # lora1,lora2,tp,av,ao,op (all .T)
    modWT = dram_in("modWT", [2 * C, C], BF16)
    w1sT = dram_in("w1sT", [2 * C, W1R], BF16)
    w2sT = dram_in("w2sT", [W1R, KFLAT], BF16)
    wiT = dram_in("wiT", [HT, 128, CT * 128], BF16)
    woT = dram_in("woT", [CT, 128, (HT // 2) * 128], BF16)
    dmlp = dram_in("dmlp", [HT, 128, KK * 128], BF16)
    dwk_in = dram_in("dwk", [HID, KK])
    out = nc.declare_dram_parameter("out", [C, VC], F32, isOutput=True)
    if DEBUG:
        dbg_mod = nc.declare_dram_parameter("dbg_mod", [C, 1], F32, isOutput=True)
        dbg_kern = nc.declare_dram_parameter("dbg_kern", [C, KK], F32, isOutput=True)
        dbg_dyn = nc.declare_dram_parameter("dbg_dyn", [C, V5], F32, isOutput=True)
        dbg_xb = nc.declare_dram_parameter("dbg_xb", [C, V5], F32, isOutput=True)
        dbg_xln = nc.declare_dram_parameter("dbg_xln", [C, V5], F32, isOutput=True)
        dbg_gate = nc.declare_dram_parameter("dbg_gate", [128, VC], F32, isOutput=True)
        dbg_h = nc.declare_dram_parameter("dbg_h", [128, HPN], F32, isOutput=True)
        dbg_c1 = nc.declare_dram_parameter("dbg_c1", [128, VC], F32, isOutput=True)
        dbg_y = nc.declare_dram_parameter("dbg_y", [C, VC], F32, isOutput=True)

    with tile.TileContext(nc) as tc, ExitStack() as ctx:
        persist = ctx.enter_context(tc.tile_pool(name="persist", bufs=1))
        dram = ctx.enter_context(tc.tile_pool(name="dram", bufs=1, space="DRAM"))
        xbpool = ctx.enter_context(tc.tile_pool(name="xbpool", bufs=1))
        ytpool = ctx.enter_context(tc.tile_pool(name="ytpool", bufs=1))

        psA_cm = tc.tile_pool(name="psA", bufs=1, space="PSUM")
        psA = psA_cm.__enter__()
        gpool_cm = tc.tile_pool(name="gemv", bufs=2)
        gpool = gpool_cm.__enter__()
        xpool_cm = tc.tile_pool(name="xpool", bufs=1)
        xpool = xpool_cm.__enter__()

        gwpool_cm = tc.tile_pool(name="gwpool", bufs=2)
        gwpool = gwpool_cm.__enter__()
        w1pool_cm = tc.tile_pool(name="w1pool", bufs=1)
        w1pool = w1pool_cm.__enter__()

        # ------------ phase A: vc partial sums + AR1 (issue-first) ------------
        xvc = [w1pool.tile([128, VC], BF16, name=f"xvc{i}", tag=f"xvc{i}")
               for i in range(CT)]
        for i in range(CT):
            nc.sync.dma_start(xvc[i][:], xown_in[128 * i:128 * (i + 1), :])
        vcs = persist.tile([128, CT], F32, name="vcs", tag="vcs")
        for i in range(CT):
            nc.vector.tensor_reduce(vcs[:, i:i + 1], xvc[i][:],
                                    axis=mybir.AxisListType.X, op=ADD)
        ar1_in = dram.tile([C], F32, name="ar1i", tag="ar1i")
        ar1_out = dram.tile([C], F32, name="ar1o", tag="ar1o",
                            addr_space="Shared")
        nc.sync.dma_start(
            bass.AP(tensor=ar1_in[:].tensor, offset=ar1_in[:].offset,
                    ap=[[1, 128], [128, CT]]), vcs[:])
        nc.gpsimd.collective_compute(
            "AllReduce", ADD, replica_groups=[list(range(NCORES))],
            ins=[ar1_in[:]], outs=[ar1_out[:]])
        sm = [persist.tile([128, 16], F32, name=f"sm{i}", tag=f"sm{i}")
              for i in range(CT)]
        for i in range(CT):
            nc.sync.dma_start(sm[i][:], smalls[128 * i:128 * (i + 1), :])
        xp = [xpool.tile([128, XPN], BF16, name=f"xp{i}", tag=f"xp{i}")
              for i in range(CT)]
        x4 = [t.rearrange("p (z y x) -> p z y x", z=Z7, y=26, x=26) for t in xp]
        id_t = persist.tile([128, 128], BF16, name="identt", tag="identt")
        nc.sync.dma_start(id_t[:], ident[:, :])
        ones_t = persist.tile([128, 1], BF16, name="onest", tag="onest")
        nc.sync.dma_start(ones_t[:], onesc[:, :])
        onesr_t = persist.tile([1, 128], BF16, name="onesrt", tag="onesrt")
        nc.sync.dma_start(onesr_t[:], onesr_in[:, :])
        eps_t = persist.tile([128, 1], F32, name="epst", tag="epst")
        nc.vector.memset(eps_t[:], EPS)
        junk = xpool.tile([128, VC], BF16, name="junk", tag="junk")
        junk5 = junk.rearrange("p (z y x) -> p z y x", z=ZP, y=24, x=24)
        opT = persist.tile([128, CT * C], BF16, name="opT", tag="opT")
        nc.sync.dma_start(
            opT[:], bass.AP(tensor=gemvW, offset=5 * C * C,
                            ap=[[C, 128], [128 * C, CT], [1, C]]))
        gi_all = persist.tile([128, CT * G], F32, name="giall", tag="giall")
        nc.sync.dma_start(
            gi_all[:], bass.AP(tensor=gind_in, offset=0,
                               ap=[[G, 128], [128 * G, CT], [1, G]]))
        knb1_t = persist.tile([128, 3], F32, name="knb1t", tag="knb1t")
        nc.sync.dma_start(
            knb1_t[:], bass.AP(tensor=knb1, offset=0, ap=[[1, 128], [128, 3]]))
        kb2 = [persist.tile([128, KK], F32, name=f"kb2{i}", tag=f"kb2{i}")
               for i in range(CT)]
        for i in range(CT):
            nc.sync.dma_start(kb2[i][:], knb2t[128 * i:128 * (i + 1), :])
        dwk_map = {}
        for t in range(HT):
            if t not in MLP_PE:
                d = persist.tile([128, KK], F32, name=f"dwk{t}", tag=f"dwk{t}")
                nc.sync.dma_start(d[:], dwk_in[128 * t:128 * (t + 1), :])
                dwk_map[t] = d



        def load_gwbuf(dram_t, off):
            t = gwpool.tile([128, CT * C], BF16, name="gwbuf", tag="gwbuf")
            nc.scalar.dma_start(
                t[:], bass.AP(tensor=dram_t, offset=off,
                              ap=[[C, 128], [128 * C, CT], [1, C]]))
            return t

        w1t = [w1pool.tile([128, CT * W1R], BF16, name=f"w1t{h}",
                           tag=f"w1t{h}") for h in range(2)]
        for h in range(2):
            nc.scalar.dma_start(
                w1t[h][:], bass.AP(tensor=w1sT, offset=h * C * W1R,
                                   ap=[[W1R, 128], [128 * W1R, CT], [1, W1R]]))


        # ------------ phase B: text gemv chain (overlaps AR1) ------------
        def gemv(wt, in_cols, nm, act, bias_cols, tag, scale=1.0, odt=BF16):
            outs = []
            for m in range(nm):
                ps = psA.tile([128, 1], F32, name="ps_small", tag="ps_small")
                for k in range(CT):
                    nc.tensor.matmul(
                        ps[:], wt[:, C * k + 128 * m:C * k + 128 * m + 128],
                        in_cols[k][:], start=(k == 0), stop=(k == CT - 1))
                o = gpool.tile([128, 1], odt, name=f"{tag}o{m}",
                               tag=f"{tag}o{m}")
                bias = bias_cols[m] if bias_cols is not None else 0.0
                nc.scalar.activation(o[:], ps[:], act, bias=bias, scale=scale)
                outs.append(o)
            return outs

        t_cols = []
        for i in range(CT):
            t = gpool.tile([128, 1], BF16, name=f"tc{i}", tag=f"tc{i}")
            nc.scalar.activation(t[:], sm[i][:, S_Y:S_Y + 1], Copy)
            t_cols.append(t)
        h1 = gemv(load_gwbuf(gemvW, 0 * C * C), t_cols, CT, Relu,
                  [sm[i][:, S_LB1:S_LB1 + 1] for i in range(CT)], "lw1")
        h2 = gemv(load_gwbuf(gemvW, 1 * C * C), h1, CT, Iden,
                  [sm[i][:, S_LB2:S_LB2 + 1] for i in range(CT)], "lw2")
        tp = gemv(load_gwbuf(gemvW, 2 * C * C), h2, CT, Iden,
                  [sm[i][:, S_TPB:S_TPB + 1] for i in range(CT)], "tpw")
        av = gemv(load_gwbuf(gemvW, 3 * C * C), tp, CT, Iden,
                  [sm[i][:, S_ABV:S_ABV + 1] for i in range(CT)], "avw")
        attn = gemv(load_gwbuf(gemvW, 4 * C * C), av, CT, Iden,
                    [sm[i][:, S_ABO:S_ABO + 1] for i in range(CT)], "aow")
        mw = [load_gwbuf(modWT, h * C * C) for h in range(2)]

        # ------------ phase C: post-AR1 gemvs ------------
        vc_cols = []
        for i in range(CT):
            col = gpool.tile([128, 1], F32, name=f"vcc{i}", tag=f"vcc{i}")
            nc.sync.dma_start(
                col[:], bass.AP(tensor=ar1_out[:].tensor,
                                offset=ar1_out[:].offset + 128 * i,
                                ap=[[1, 128], [128, 1]]))
            cb = gpool.tile([128, 1], BF16, name=f"cmb{i}", tag=f"cmb{i}")
            nc.scalar.activation(cb[:], col[:], Copy, scale=1.0 / NTOT)
            vc_cols.append(cb)
        halves = [vc_cols, attn]

        def gemv2h(wts, stride, nm, act, bias_cols, tag, odt=F32):
            outs = []
            for m in range(nm):
                ps = psA.tile([128, 1], F32, name="ps_small", tag="ps_small")
                for h in range(2):
                    for k in range(CT):
                        nc.tensor.matmul(
                            ps[:], wts[h][:, k * stride + 128 * m:
                                          k * stride + 128 * m + 128],
                            halves[h][k][:],
                            start=(h == 0 and k == 0),
                            stop=(h == 1 and k == CT - 1))
                o = gpool.tile([128, 1], odt, name=f"{tag}o{m}",
                               tag=f"{tag}o{m}")
                nc.scalar.activation(o[:], ps[:], act, bias=bias_cols[m])
                outs.append(o)
            return outs

        mod = gemv2h(mw, C, CT, Sigmoid,
                     [sm[i][:, S_MODB:S_MODB + 1] for i in range(CT)], "modw")
        kp1 = gemv2h(w1t, W1R, 3, Relu,
                     [knb1_t[:, m:m + 1] for m in range(3)], "w1s", odt=BF16)

        w1pool_cm.__exit__(None, None, None)
        gwpool_cm.__exit__(None, None, None)

        # ------------ phase D: kernel_net gemv2 + AR2 ------------
        ko_all = persist.tile([128, KK * CT], F32, name="koall", tag="koall")
        with (tc.tile_pool(name="kseq", bufs=13) as kseq,
              tc.tile_pool(name="kps", bufs=1, space="PSUM") as kps):
            for i in (0, 4, 5):
                nc.sync.dma_start(xp[i][:], xpad_in[128 * i:128 * (i + 1), :])
            kop = kps.tile([128, KK * CT], F32, name="g2ps", tag="g2ps")
            for t in range(KK):
                wt = kseq.tile([128, 3 * C], BF16, name="w2t", tag="w2t")
                nc.gpsimd.dma_start(
                    wt[:], bass.AP(tensor=w2sT, offset=t * C,
                                   ap=[[KFLAT, 128], [128 * KFLAT, 3],
                                       [1, C]]))
                for m in range(CT):
                    for k in range(3):
                        nc.tensor.matmul(
                            kop[:, CT * t + m:CT * t + m + 1],
                            wt[:, k * C + 128 * m:k * C + 128 * m + 128],
                            kp1[k][:], start=(k == 0), stop=(k == 2))
            nc.scalar.activation(ko_all[:], kop[:], Copy)
            for i in (1, 2, 3):
                nc.gpsimd.dma_start(xp[i][:], xpad_in[128 * i:128 * (i + 1), :])
        ar2_in = dram.tile([128, KK * CT], F32, name="ar2i", tag="ar2i")
        ar2_out = dram.tile([128, KK * CT], F32, name="ar2o", tag="ar2o",
                            addr_space="Shared")
        nc.sync.dma_start(ar2_in[:, :], ko_all[:])
        nc.gpsimd.collective_compute(
            "AllReduce", ADD, replica_groups=[list(range(NCORES))],
            ins=[ar2_in[:]], outs=[ar2_out[:]])
        kraw = persist.tile([128, KK * CT], F32, name="kraw", tag="kraw")
        nc.sync.dma_start(kraw[:], ar2_out[:, :])
        kraw3 = kraw.rearrange("p (t i) -> p t i", t=KK, i=CT)


        # kernels: +bias, fold mod
        kernm = []
        for i in range(CT):
            km = persist.tile([128, KK], F32, name=f"kernm{i}", tag=f"kernm{i}")
            nc.vector.tensor_tensor(km[:], kraw3[:, :, i], kb2[i][:], op=ADD)
            nc.vector.tensor_scalar_mul(km[:], km[:], mod[i][:, 0:1])
            kernm.append(km)

        if DEBUG:
            for i in range(CT):
                nc.gpsimd.dma_start(dbg_mod[128 * i:128 * (i + 1), :], mod[i][:])
                nc.gpsimd.dma_start(dbg_kern[128 * i:128 * (i + 1), :], kernm[i][:])

        # ------------ phase E: dynamic depthwise conv + GN stats ------------
        dynpool_cm = tc.tile_pool(name="dynpool", bufs=1)
        dynpool = dynpool_cm.__enter__()
        dyn = [dynpool.tile([128, V5], BF16, name=f"dyn{i}", tag=f"dyn{i}")
               for i in range(CT)]
        dyn4 = [t.rearrange("p (z y x) -> p z y x", z=Z5, y=24, x=24)
                for t in dyn]

        dgpool_cm = tc.tile_pool(name="dgpool", bufs=2)
        dgpool = dgpool_cm.__enter__()

        def build_diag(i):
            d = dgpool.tile([128, KK * 128], BF16, name="dg", tag="dg")
            for t in range(KK):
                nc.vector.tensor_scalar_mul(d[:, 128 * t:128 * (t + 1)],
                                            id_t[:], kernm[i][:, t:t + 1])
            return d

        def dyn_pe_planes(i, dgt, zos, cpool):
            for zo in zos:
                for half in range(2):
                    y0 = 12 * half
                    cp = cpool.tile([128, 288], F32, name=f"dcp{half}",
                                    tag=f"dcp{half}")
                    for ti, (dz, dy, dx) in enumerate(TAPS):
                        nc.tensor.matmul(
                            cp[:], dgt[:, 128 * ti:128 * (ti + 1)],
                            x4[i][:, zo + 1 + dz, 1 + y0 + dy:13 + y0 + dy,
                                  1 + dx:25 + dx],
                            start=(ti == 0), stop=(ti == KK - 1))
                    nc.scalar.activation(
                        dyn4[i][:, zo, y0:y0 + 12, :], cp[:], Copy)

        DT = 7

        def dyn_assist(i, cpool, tpool):
            # PE computes the first DT taps of a DVE tile's own planes
            dgp = dgpool.tile([128, DT * 128], BF16, name="dgp", tag="dgp")
            for t in range(DT):
                nc.vector.tensor_scalar_mul(dgp[:, 128 * t:128 * (t + 1)],
                                            id_t[:], kernm[i][:, t:t + 1])
            pd = tpool.tile([128, VC], BF16, name="dpart", tag="dpart")
            for zo in (1, 2, 3):
                for half in range(2):
                    y0 = 12 * half
                    cp = cpool.tile([128, 288], F32, name=f"dcp{half}",
                                    tag=f"dcp{half}")
                    for ti in range(DT):
                        dz, dy, dx = TAPS[ti]
                        nc.tensor.matmul(
                            cp[:], dgp[:, 128 * ti:128 * (ti + 1)],
                            x4[i][:, zo + 1 + dz, 1 + y0 + dy:13 + y0 + dy,
                                  1 + dx:25 + dx],
                            start=(ti == 0), stop=(ti == DT - 1))
                    nc.scalar.activation(
                        pd[:, (zo - 1) * PL + y0 * 24:
                           (zo - 1) * PL + y0 * 24 + 288], cp[:], Copy)
            return pd

        def dyn_dve_planes(i, zo0, nz, tpool, t0=0, merge=None):
            dst = dyn4[i][:, zo0:zo0 + nz, :, :]
            for ti in range(t0, KK):
                dz, dy, dx = TAPS[ti]
                src = x4[i][:, zo0 + 1 + dz:zo0 + 1 + dz + nz,
                            1 + dy:25 + dy, 1 + dx:25 + dx]
                if ti == t0:
                    nc.vector.tensor_scalar_mul(dst, src, kernm[i][:, ti:ti + 1])
                else:
                    tmp = tpool.tile([128, 3 * PL], BF16, name="dtmp",
                                     tag="dtmp")
                    t4 = tmp.rearrange("p (z y x) -> p z y x", z=3, y=24,
                                       x=24)[:, 0:nz, :, :]
                    nc.vector.tensor_scalar_mul(t4, src,
                                                kernm[i][:, ti:ti + 1])
                    nc.vector.tensor_tensor(dst, dst, t4, op=ADD)
            if merge is not None:
                nc.vector.tensor_tensor(dyn[i][:, PL:4 * PL],
                                        dyn[i][:, PL:4 * PL], merge[:], op=ADD)

        gst = persist.tile([128, 2 * CT], F32, name="gst", tag="gst")
        ar3_in = dram.tile([G, 2], F32, name="ar3i", tag="ar3i")
        ar3_out = dram.tile([G, 2], F32, name="ar3o", tag="ar3o",
                            addr_space="Shared")
        gsb = persist.tile([G, 2], F32, name="gsb", tag="gsb")
        with (tc.tile_pool(name="dcpool", bufs=1, space="PSUM") as dcpool,
              tc.tile_pool(name="dtpool", bufs=2) as dtpool,
              tc.tile_pool(name="gnps", bufs=1, space="PSUM") as gnps):
            # own planes (zo 1..3) first, stats, then halo planes under AR3
            dg_live = {}
            pdyn = {}
            for i in range(CT):
                if i not in DYN_PE:
                    pdyn[i] = dyn_assist(i, dcpool, dtpool)
            for i in range(CT):
                if i in DYN_PE:
                    dg_live[i] = build_diag(i)
                    dyn_pe_planes(i, dg_live[i], (1, 2, 3), dcpool)
                else:
                    dyn_dve_planes(i, 1, 3, dtpool, t0=DT, merge=pdyn[i])
                nc.scalar.activation(junk[:], dyn[i][:, PL:4 * PL],
                                     Copy, accum_out=gst[:, 2 * i:2 * i + 1])
                nc.scalar.activation(junk[:], dyn[i][:, PL:4 * PL],
                                     Square,
                                     accum_out=gst[:, 2 * i + 1:2 * i + 2])
            gps = gnps.tile([G, 2], F32, name="gps", tag="gps")
            for i in range(CT):
                nc.tensor.matmul(gps[:], gi_all[:, G * i:G * (i + 1)],
                                 gst[:, 2 * i:2 * i + 2], start=(i == 0),
                                 stop=(i == CT - 1))
            nc.scalar.activation(gsb[:], gps[:], Copy)
            nc.sync.dma_start(ar3_in[:, :], gsb[:])
            nc.gpsimd.collective_compute(
                "AllReduce", ADD, replica_groups=[list(range(NCORES))],
                ins=[ar3_in[:]], outs=[ar3_out[:]])
            for i in range(CT):
                if i in DYN_PE:
                    dg2 = build_diag(i)
                    dyn_pe_planes(i, dg2, (0, 4), dcpool)
                else:
                    dyn_dve_planes(i, 0, 1, dtpool)
                    dyn_dve_planes(i, 4, 1, dtpool)
        dgpool_cm.__exit__(None, None, None)

        if DEBUG:
            for i in range(CT):
                nc.gpsimd.dma_start(dbg_dyn[128 * i:128 * (i + 1), :], dyn[i][:])

        # ------------ GN scale/shift + fold into opT ------------
        gstat = persist.tile([G, 2], F32, name="gstat", tag="gstat")
        nc.sync.dma_start(gstat[:], ar3_out[:, :])
        NGRP = float(GD) * NTOT
        gmr = persist.tile([G, 2], F32, name="gmr", tag="gmr")
        nc.scalar.activation(gmr[:, 0:1], gstat[:, 0:1], Copy, scale=1.0 / NGRP)
        musq = persist.tile([G, 1], F32, name="musq", tag="musq")
        nc.scalar.square(musq[:], gmr[:, 0:1])
        var = persist.tile([G, 1], F32, name="gvar", tag="gvar")
        nc.vector.tensor_scalar(var[:], gstat[:, 1:2], 1.0 / NGRP, None,
                                op0=MULT)
        nc.vector.tensor_sub(var[:], var[:], musq[:])
        nc.scalar.activation(var[:], var[:], Sqrt, bias=eps_t[0:G, 0:1])
        nc.vector.reciprocal(gmr[:, 1:2], var[:])

        cafm_shift = []
        gsc = []
        for i in range(CT):
            ge = gpool.tile([G, 128], F32, name=f"gexp{i}", tag=f"gexp{i}")
            nc.sync.dma_start(ge[:], gexpT[:, 128 * i:128 * (i + 1)])
            ps = psA.tile([128, 2], F32, name="ps_sm2", tag="ps_sm2")
            nc.tensor.matmul(ps[:], ge[:], gmr[:], start=True, stop=True)
            mu_c = persist.tile([128, 2], F32, name=f"muc{i}", tag=f"muc{i}")
            nc.scalar.activation(mu_c[:], ps[:], Copy)
            a = persist.tile([128, 1], F32, name=f"gsc{i}", tag=f"gsc{i}")
            nc.vector.tensor_mul(a[:], sm[i][:, S_GNG:S_GNG + 1], mu_c[:, 1:2])
            b = persist.tile([128, 1], F32, name=f"gsh{i}", tag=f"gsh{i}")
            nc.vector.tensor_mul(b[:], mu_c[:, 0:1], a[:])
            nc.vector.tensor_sub(b[:], sm[i][:, S_GNB:S_GNB + 1], b[:])
            gsc.append(a)
            bb = gpool.tile([128, 1], BF16, name=f"gshb{i}", tag=f"gshb{i}")
            nc.scalar.activation(bb[:], b[:], Copy)
            cafm_shift.append(bb)
        cb_cols = []
        for m in range(CT):
            ps = psA.tile([128, 1], F32, name="ps_small", tag="ps_small")
            for k in range(CT):
                nc.tensor.matmul(ps[:], opT[:, 768 * k + 128 * m:
                                            768 * k + 128 * m + 128],
                                 cafm_shift[k][:], start=(k == 0),
                                 stop=(k == CT - 1))
            o = persist.tile([128, 1], F32, name=f"cbc{m}", tag=f"cbc{m}")
            nc.scalar.activation(o[:], ps[:], Iden,
                                 bias=sm[m][:, S_OPB:S_OPB + 1])
            cb_cols.append(o)
        for k in range(CT):
            nc.vector.tensor_scalar_mul(opT[:, 768 * k:768 * (k + 1)],
                                        opT[:, 768 * k:768 * (k + 1)],
                                        gsc[k][:])

        # ------------ phase F1: cafm matmul + xb ------------
        xb = [xbpool.tile([128, V5], BF16, name=f"xb{m}", tag=f"xb{m}")
              for m in range(CT)]
        xb4 = [t.rearrange("p (z y x) -> p z y x", z=Z5, y=24, x=24)
               for t in xb]
        CH6 = [(o, min(512, V5 - o)) for o in range(0, V5, 512)]
        with (tc.tile_pool(name="f1ps", bufs=2, space="PSUM") as f1ps,
              tc.tile_pool(name="f1t", bufs=2) as f1t):
            for m in range(CT):
                tca = f1t.tile([128, V5], BF16, name="tcafm", tag="tcafm")
                for o, n in CH6:
                    ps = f1ps.tile([128, 512], F32, name="f1p", tag="f1p")
                    for k in range(CT):
                        nc.tensor.matmul(ps[:, 0:n],
                                         opT[:, 768 * k + 128 * m:
                                             768 * k + 128 * m + 128],
                                         dyn[k][:, o:o + n], start=(k == 0),
                                         stop=(k == CT - 1))
                    nc.scalar.activation(tca[:, o:o + n], ps[:, 0:n], Iden,
                                         bias=cb_cols[m][:])
                t4 = tca.rearrange("p (z y x) -> p z y x", z=Z5, y=24, x=24)
                nc.vector.tensor_tensor(xb4[m][:, :, :, :], t4[:, :, :, :],
                                        x4[m][:, 1:6, 1:25, 1:25], op=MULT)
        if DEBUG:
            for m in range(CT):
                nc.gpsimd.dma_start(dbg_xb[128 * m:128 * (m + 1), :], xb[m][:])
        dynpool_cm.__exit__(None, None, None)
        xpool_cm.__exit__(None, None, None)
        gpool_cm.__exit__(None, None, None)
        psA_cm.__exit__(None, None, None)

        # ------------ LN helper ------------
        def ln_stats(tiles, nv, tag, mupool):
            """Per-voxel mean/rstd over channels -> [128, nv] bf16 tiles.
            ones_t carries 1/C so the matmuls produce mean / E[x^2]."""
            nch = [(o, min(480, nv - o)) for o in range(0, nv, 480)]
            muB = mupool.tile([128, nv], BF16, name=f"{tag}muB",
                              tag=f"{tag}muB")
            rsB = mupool.tile([128, nv], BF16, name=f"{tag}rsB",
                              tag=f"{tag}rsB")
            with (tc.tile_pool(name=f"{tag}ps", bufs=1, space="PSUM") as lps,
                  tc.tile_pool(name=f"{tag}sq", bufs=2) as sqp,
                  tc.tile_pool(name=f"{tag}rw", bufs=2) as rwp):
                for o, n in nch:
                    p1 = lps.tile([1, 480], F32, name="s1", tag="s1")
                    p2 = lps.tile([1, 480], F32, name="s2", tag="s2")
                    for k in range(CT):
                        nc.tensor.matmul(p1[:, 0:n], ones_t[:],
                                         tiles[k][:, o:o + n],
                                         start=(k == 0), stop=(k == CT - 1))
                    for k in range(CT):
                        q = sqp.tile([128, 480], BF16, name="sqc", tag="sqc")
                        if k % 2 == 0:
                            nc.scalar.activation(q[:, 0:n],
                                                 tiles[k][:, o:o + n], Square)
                        else:
                            nc.vector.tensor_mul(q[:, 0:n],
                                                 tiles[k][:, o:o + n],
                                                 tiles[k][:, o:o + n])
                        nc.tensor.matmul(p2[:, 0:n], ones_t[:], q[:, 0:n],
                                         start=(k == 0), stop=(k == CT - 1))
                    mubf = rwp.tile([1, 480], BF16, name="mubf", tag="mubf")
                    nc.scalar.activation(mubf[:, 0:n], p1[:, 0:n], Copy)
                    m2 = rwp.tile([1, 480], F32, name="m2", tag="m2")
                    nc.scalar.activation(m2[:, 0:n], p1[:, 0:n], Square)
                    vr = rwp.tile([1, 480], F32, name="vr", tag="vr")
                    nc.vector.tensor_sub(vr[:, 0:n], p2[:, 0:n], m2[:, 0:n])
                    nc.scalar.activation(vr[:, 0:n], vr[:, 0:n], Sqrt,
                                         bias=eps_t[0:1, 0:1])
                    rbf = rwp.tile([1, 480], BF16, name="rbf", tag="rbf")
                    with nc.allow_low_precision(reason="rstd bcast in bf16"):
                        nc.vector.reciprocal(rbf[:, 0:n], vr[:, 0:n])
                    pb = lps.tile([128, 480], F32, name="bc", tag="bc")
                    nc.tensor.matmul(pb[:, 0:n], onesr_t[:], mubf[:, 0:n],
                                     start=True, stop=True)
                    nc.scalar.activation(muB[:, o:o + n], pb[:, 0:n], Copy)
                    pb2 = lps.tile([128, 480], F32, name="bc2", tag="bc2")
                    nc.tensor.matmul(pb2[:, 0:n], onesr_t[:], rbf[:, 0:n],
                                     start=True, stop=True)
                    nc.scalar.activation(rsB[:, o:o + n], pb2[:, 0:n], Copy)
            return muB, rsB

        # ------------ LN2 + edge masks ------------
        gatepool_cm = tc.tile_pool(name="gatepool", bufs=1)
        gatepool = gatepool_cm.__enter__()
        xlnpool_cm = tc.tile_pool(name="xlnpool", bufs=1)
        xlnpool = xlnpool_cm.__enter__()
        xln = [xlnpool.tile([128, V5], BF16, name=f"xln{k}", tag=f"xln{k}")
               for k in range(CT)]
        mupool_cm = tc.tile_pool(name="mupool", bufs=1)
        mupool = mupool_cm.__enter__()
        muB, rsB = ln_stats(xb, V5, "ln2", mupool)
        with tc.tile_pool(name="lnap", bufs=2) as lnap:
            for k in range(CT):
                t1 = lnap.tile([128, V5], BF16, name="lnt1", tag="lnt1")
                nc.vector.tensor_sub(t1[:], xb[k][:], muB[:])
                nc.vector.tensor_mul(t1[:], t1[:], rsB[:])
                nc.vector.tensor_scalar(xln[k][:], t1[:],
                                        sm[k][:, S_N2W:S_N2W + 1],
                                        sm[k][:, S_N2B:S_N2B + 1],
                                        op0=MULT, op1=ADD)
                nc.vector.tensor_scalar_mul(xln[k][:, 0:PL], xln[k][:, 0:PL],
                                            sm[k][:, S_MLO:S_MLO + 1])
                nc.vector.tensor_scalar_mul(xln[k][:, 4 * PL:5 * PL],
                                            xln[k][:, 4 * PL:5 * PL],
                                            sm[k][:, S_MHI:S_MHI + 1])

        # ------------ MLP: Wi + depthwise conv + gate ------------
        if DEBUG:
            for k in range(CT):
                nc.gpsimd.dma_start(dbg_xln[128 * k:128 * (k + 1), :], xln[k][:])
        mupool_cm.__exit__(None, None, None)
        gate = [gatepool.tile([128, VC], BF16, name=f"gate{j}", tag=f"gate{j}")
                for j in range(HT // 2)]
        hppool_cm = tc.tile_pool(name="hppool", bufs=1)
        hppool = hppool_cm.__enter__()
        hpads = [hppool.tile([128, HPN], BF16, name=f"hpad{b}", tag=f"hpad{b}")
                 for b in range(4)]
        for b in range(4):
            nc.vector.memset(hpads[b][:], 0.0)
        hp4s = [t.rearrange("p (z y x) -> p z y x", z=Z5, y=26, x=26)
                for t in hpads]

        with (tc.tile_pool(name="wiw", bufs=2) as wiw,
              tc.tile_pool(name="dga", bufs=2) as dgap,
              tc.tile_pool(name="wips", bufs=2, space="PSUM") as wips,
              tc.tile_pool(name="cvps", bufs=1, space="PSUM") as cvps,
              tc.tile_pool(name="mlpt", bufs=1) as mlpt):

            def wi_pass(t, bi):
                hp4 = hp4s[bi]
                wall = wiw.tile([128, CT * 128], BF16, name="wiall",
                                tag="wiall")
                nc.gpsimd.dma_start(wall[:], wiT[t, :, :])
                for z in range(Z5):
                    for halfy in range(2):
                        y0 = 12 * halfy
                        pz = wips.tile([128, 288], F32, name=f"wip{halfy}",
                                       tag=f"wip{halfy}")
                        for k in range(CT):
                            nc.tensor.matmul(
                                pz[:],
                                wall[:, 128 * k:128 * (k + 1)],
                                xln[k][:, z * PL + y0 * 24:
                                       z * PL + y0 * 24 + 288],
                                start=(k == 0), stop=(k == CT - 1))
                        nc.scalar.activation(
                            hp4[:, z, 1 + y0:13 + y0, 1:25],
                            pz.rearrange("p (y x) -> p y x", y=12, x=24)[
                                :, :, :], Copy)
                if DEBUG and t == 0:
                    nc.gpsimd.dma_start(dbg_h[:, :], hpads[bi][:])

            PTAPS = 7

            def conv_pass(t, bi, dst, act):
                hp4 = hp4s[bi]
                if t in MLP_PE:
                    if dst is None:
                        dst = mlpt.tile([128, VC], BF16, name="conv2",
                                        tag="conv2")
                    dgt = dgap.tile([128, KK * 128], BF16, name="dgall",
                                    tag="dgall")
                    nc.gpsimd.dma_start(dgt[:], dmlp[t, :, :])
                    for zo in range(3):
                        for halfy in range(2):
                            b = 2 * zo + halfy
                            y0 = 12 * halfy
                            cp = cvps.tile([128, 288], F32,
                                           name=f"mcp{b % 2}",
                                           tag=f"mcp{b % 2}")
                            for ti, (dz, dy, dx) in enumerate(TAPS):
                                nc.tensor.matmul(
                                    cp[:], dgt[:, 128 * ti:128 * (ti + 1)],
                                    hp4[:, 1 + zo + dz,
                                        1 + y0 + dy:13 + y0 + dy,
                                        1 + dx:25 + dx],
                                    start=(ti == 0), stop=(ti == KK - 1))
                            nc.scalar.activation(
                                dst[:, 288 * b:288 * (b + 1)], cp[:], act)
                    return dst
                # PE computes the first PTAPS taps into a bf16 partial
                dgp = dgap.tile([128, PTAPS * 128], BF16, name="dgpart",
                                tag="dgpart")
                nc.gpsimd.dma_start(
                    dgp[:], bass.AP(tensor=dmlp, offset=t * 128 * KK * 128,
                                    ap=[[KK * 128, 128], [1, PTAPS * 128]]))
                pcv = mlpt.tile([128, VC], BF16, name="pconv", tag="pconv")
                for zo in range(3):
                    for halfy in range(2):
                        b = 2 * zo + halfy
                        y0 = 12 * halfy
                        cp = cvps.tile([128, 288], F32, name=f"mcp{b % 2}",
                                       tag=f"mcp{b % 2}")
                        for ti in range(PTAPS):
                            dz, dy, dx = TAPS[ti]
                            nc.tensor.matmul(
                                cp[:], dgp[:, 128 * ti:128 * (ti + 1)],
                                hp4[:, 1 + zo + dz, 1 + y0 + dy:13 + y0 + dy,
                                    1 + dx:25 + dx],
                                start=(ti == 0), stop=(ti == PTAPS - 1))
                        nc.scalar.activation(pcv[:, 288 * b:288 * (b + 1)],
                                             cp[:], Copy)
                acc = mlpt.tile([128, VC], BF16, name="macc", tag="macc")
                a4 = acc.rearrange("p (z y x) -> p z y x", z=3, y=24, x=24)
                kw = dwk_map[t]
                for ti in range(PTAPS, KK):
                    dz, dy, dx = TAPS[ti]
                    src = hp4[:, 1 + dz:4 + dz, 1 + dy:25 + dy, 1 + dx:25 + dx]
                    if ti == PTAPS:
                        nc.vector.tensor_scalar_mul(a4[:, :, :, :], src,
                                                    kw[:, ti:ti + 1])
                    else:
                        tmp = mlpt.tile([128, VC], BF16, name="mtmp",
                                        tag="mtmp")
                        t4 = tmp.rearrange("p (z y x) -> p z y x", z=3, y=24,
                                           x=24)
                        nc.vector.tensor_scalar_mul(t4[:, :, :, :], src,
                                                    kw[:, ti:ti + 1])
                        nc.vector.tensor_tensor(a4[:, :, :, :], a4[:, :, :, :],
                                                t4[:, :, :, :], op=ADD)
                nc.vector.tensor_tensor(acc[:], acc[:], pcv[:], op=ADD)
                if act is Gelu:
                    nc.scalar.activation(dst[:], acc[:], Gelu)
                    return dst
                return acc

            for j in range(HT // 2):
                b1, b2 = (2 * j) % 4, (2 * j + 1) % 4
                wi_pass(j, b1)
                wi_pass(j + HT // 2, b2)
                conv_pass(j, b1, gate[j], Gelu)
                if DEBUG and j == 0:
                    nc.gpsimd.dma_start(dbg_c1[:, :], gate[0][:])
                c2 = conv_pass(j + HT // 2, b2, None, Copy)
                nc.vector.tensor_mul(gate[j][:], gate[j][:], c2[:])
        if DEBUG:
            nc.gpsimd.dma_start(dbg_gate[:, :], gate[0][:])
        hppool_cm.__exit__(None, None, None)
        xlnpool_cm.__exit__(None, None, None)

        # ------------ Wo + residual + LN3 (chunk-major, pipelined) ------------
        y_t = [ytpool.tile([128, VC], BF16, name=f"y{m}", tag=f"y{m}")
               for m in range(CT)]
        CH3 = [(0, 480), (480, 480), (960, 480), (1440, 288)]
        with (tc.tile_pool(name="wow", bufs=1) as wow,
              tc.tile_pool(name="wops", bufs=2, space="PSUM") as wops,
              tc.tile_pool(name="ln3ps", bufs=1, space="PSUM") as l3ps,
              tc.tile_pool(name="ln3sq", bufs=2) as l3sq,
              tc.tile_pool(name="ln3rw", bufs=2) as l3rw,
              tc.tile_pool(name="mupool3", bufs=1) as mupool3,
              tc.tile_pool(name="lnap3", bufs=2) as lnap3):
            walls = [wow.tile([128, (HT // 2) * 128], BF16, name=f"wo{m}",
                              tag=f"wo{m}") for m in range(CT)]
            for m in range(CT):
                nc.gpsimd.dma_start(walls[m][:], woT[m, :, :])
            muB3 = mupool3.tile([128, VC], BF16, name="ln3muB", tag="ln3muB")
            rsB3 = mupool3.tile([128, VC], BF16, name="ln3rsB", tag="ln3rsB")
            for o, n in CH3:
                for m in range(CT):
                    ps = wops.tile([128, 512], F32, name="wo_ps", tag="wo_ps")
                    for k in range(HT // 2):
                        nc.tensor.matmul(ps[:, 0:n],
                                         walls[m][:, 128 * k:128 * (k + 1)],
                                         gate[k][:, o:o + n],
                                         start=(k == 0), stop=False)
                    nc.tensor.matmul(ps[:, 0:n], id_t[:],
                                     xb[m][:, PL + o:PL + o + n],
                                     start=False, stop=True)
                    nc.scalar.activation(y_t[m][:, o:o + n], ps[:, 0:n], Copy)
                # LN3 stats for this chunk
                p1 = l3ps.tile([1, 480], F32, name="l3s1", tag="l3s1")
                p2 = l3ps.tile([1, 480], F32, name="l3s2", tag="l3s2")
                for k in range(CT):
                    nc.tensor.matmul(p1[:, 0:n], ones_t[:], y_t[k][:, o:o + n],
                                     start=(k == 0), stop=(k == CT - 1))
                for k in range(CT):
                    q = l3sq.tile([128, 480], BF16, name="l3q", tag="l3q")
                    nc.vector.tensor_mul(q[:, 0:n], y_t[k][:, o:o + n],
                                         y_t[k][:, o:o + n])
                    nc.tensor.matmul(p2[:, 0:n], ones_t[:], q[:, 0:n],
                                     start=(k == 0), stop=(k == CT - 1))
                mubf = l3rw.tile([1, 480], BF16, name="l3mubf", tag="l3mubf")
                nc.scalar.activation(mubf[:, 0:n], p1[:, 0:n], Copy)
                m2 = l3rw.tile([1, 480], F32, name="l3m2", tag="l3m2")
                nc.scalar.activation(m2[:, 0:n], p1[:, 0:n], Square)
                vr = l3rw.tile([1, 480], F32, name="l3vr", tag="l3vr")
                nc.vector.tensor_sub(vr[:, 0:n], p2[:, 0:n], m2[:, 0:n])
                nc.scalar.activation(vr[:, 0:n], vr[:, 0:n], Sqrt,
                                     bias=eps_t[0:1, 0:1])
                rbf = l3rw.tile([1, 480], BF16, name="l3rbf", tag="l3rbf")
                with nc.allow_low_precision(reason="rstd bcast in bf16"):
                    nc.vector.reciprocal(rbf[:, 0:n], vr[:, 0:n])
                pb = l3ps.tile([128, 480], F32, name="l3bc", tag="l3bc")
                nc.tensor.matmul(pb[:, 0:n], onesr_t[:], mubf[:, 0:n],
                                 start=True, stop=True)
                nc.scalar.activation(muB3[:, o:o + n], pb[:, 0:n], Copy)
                pb2 = l3ps.tile([128, 480], F32, name="l3bc2", tag="l3bc2")
                nc.tensor.matmul(pb2[:, 0:n], onesr_t[:], rbf[:, 0:n],
                                 start=True, stop=True)
                nc.scalar.activation(rsB3[:, o:o + n], pb2[:, 0:n], Copy)
                # LN3 apply + store for this chunk
                for k in range(CT):
                    t1 = lnap3.tile([128, 480], BF16, name="ln3t", tag="ln3t")
                    nc.vector.tensor_sub(t1[:, 0:n], y_t[k][:, o:o + n],
                                         muB3[:, o:o + n])
                    nc.vector.tensor_mul(t1[:, 0:n], t1[:, 0:n],
                                         rsB3[:, o:o + n])
                    of = lnap3.tile([128, 480], F32, name="outf", tag="outf")
                    nc.vector.tensor_scalar(of[:, 0:n], t1[:, 0:n],
                                            sm[k][:, S_N3W:S_N3W + 1],
                                            sm[k][:, S_N3B:S_N3B + 1],
                                            op0=MULT, op1=ADD)
                    nc.sync.dma_start(out[128 * k:128 * (k + 1), o:o + n],
                                      of[:, 0:n])
        gatepool_cm.__exit__(None, None, None)

        if DEBUG:
            for m in range(CT):
                nc.gpsimd.dma_start(dbg_y[128 * m:128 * (m + 1), :], y_t[m][:])

    nc.compile()
    return nc


def _prep(inputs):
    bf = lambda a: np.ascontiguousarray(a).astype(BF)
    f32 = lambda a: np.ascontiguousarray(a, dtype=np.float32)
    x = f32(inputs["x"][0]).reshape(C, S, S, S)

    smalls_c = np.zeros((C, 16), np.float32)
    smalls_c[:, 0] = f32(inputs["y"][0, 0])
    for i, k in enumerate(["lora_b1", "lora_b2", "tp_b", "attn_bv", "attn_bo",
                           "op_b", "n2_w", "n2_b", "n3_w", "n3_b", "gn_g",
                           "gn_b", "mod_b"]):
        smalls_c[:, i + 1] = f32(inputs[k])

    gind6 = np.zeros((CT, 128, G), np.float32)
    for j in range(CT):
        for p in range(128):
            gind6[j, p, (128 * j + p) // GD] = 1.0
    gexpT = np.zeros((G, C), np.float32)
    for c in range(C):
        gexpT[c // GD, c] = 1.0

    gemv_stack = np.stack([
        f32(inputs["lora_W1"]).T, f32(inputs["lora_W2"]).T,
        f32(inputs["tp_W"]).T, f32(inputs["attn_Wv"]).T,
        f32(inputs["attn_Wo"]).T, f32(inputs["op_W"]).T])

    kn_W2 = f32(inputs["kn_W2"])
    w2r = kn_W2.reshape(C, KK, HID).transpose(1, 0, 2).reshape(KFLAT, HID)
    w2T = np.ascontiguousarray(w2r.T)          # [HID, KFLAT]
    kn_W1 = f32(inputs["kn_W1"])

    mlp_dw = f32(inputs["mlp_dw"]).reshape(HID, KK)
    dmlp = np.zeros((HT, 128, KK, 128), np.float32)
    idx = np.arange(128)
    for tt in range(HT):
        for ti in range(KK):
            dmlp[tt, idx, ti, idx] = mlp_dw[128 * tt:128 * (tt + 1), ti]
    dmlp = dmlp.reshape(HT, 128, KK * 128)

    com = dict(
        knb2t=f32(inputs["kn_b2"]).reshape(C, KK),
        gind6=gind6, gexpT=gexpT,
        ident=bf(np.eye(128, dtype=np.float32)),
        onesc=bf(np.full((128, 1), 1.0 / C, np.float32)),
        onesr=bf(np.ones((1, 128), np.float32)),
        gemvW=bf(gemv_stack),
        modWT=bf(f32(inputs["mod_W"]).T),
        wiT=bf(f32(inputs["mlp_Wi"]).T.reshape(CT, 128, HT, 128)
               .transpose(2, 1, 0, 3).reshape(HT, 128, CT * 128)),
        woT=bf(f32(inputs["mlp_Wo"]).T.reshape(HT // 2, 128, CT, 128)
               .transpose(2, 1, 0, 3).reshape(CT, 128, (HT // 2) * 128)),
        dmlp=bf(dmlp), dwk=mlp_dw,
    )

    in_maps = []
    for i in range(NCORES):
        z0 = ZP * i
        xh = np.zeros((C, Z7, 26, 26), np.float32)
        lo, hi = max(z0 - 2, 0), min(z0 + ZP + 2, S)
        xh[:, lo - (z0 - 2):lo - (z0 - 2) + (hi - lo), 1:25, 1:25] = \
            x[:, lo:hi]
        smalls = smalls_c.copy()
        smalls[:, S_MLO] = 0.0 if i == 0 else 1.0
        smalls[:, S_MHI] = 0.0 if i == NCORES - 1 else 1.0
        m = dict(com)
        m.update(
            xpad=xh.reshape(C, XPN).astype(BF),
            xown=np.ascontiguousarray(
                x[:, z0:z0 + ZP].reshape(C, VC)).astype(BF),
            smalls=smalls,
            knb1=f32(inputs["kn_b1"][W1R * i:W1R * (i + 1)]),
            w1sT=bf(kn_W1[W1R * i:W1R * (i + 1), :].T),
            w2sT=bf(w2T[W1R * i:W1R * (i + 1), :]),
        )
        in_maps.append(m)
    return in_maps


def kernel(**inputs) -> np.ndarray:
    if "nc" not in _CACHE:
        _CACHE["nc"] = build_program()
    nc = _CACHE["nc"]
    in_maps = _prep(inputs)
    res = run_bass_kernel_spmd(nc, in_maps, list(range(NCORES)))
    outs = [res.results[i]["out"].reshape(C, ZP, PL) for i in range(NCORES)]
    full = np.concatenate(outs, axis=1)
    return full.reshape(1, C, S, S, S).astype(np.float32)
